# revision 54
# baseline (speedup 1.0000x reference)
"""Trainium2 Bass kernel for MQA cross-attention (nn_CrossAttention).

Reference computation (fp32):
    q = (x @ Wq).reshape(b, n, 16, 128).transpose(0,2,1,3) * 128**-0.5
    sim = q @ k^T   (k/v shared across heads, MQA)
    out = softmax(sim) @ v
    y = out.merge_heads @ Wo

Sharding: pure sequence-parallel across 8 cores. Each core gets 256 rows
of x per batch (512 rows total), full Wq/Wo/k/v, and produces its 512 rows
of the output. No collectives, no host-side reduction.

Mixed precision (validated vs reference, rel err ~4e-3):
  - qproj / outproj run as fp8e4 DoubleRow matmuls (0.5 cycles/row,
    256-deep contraction) with hi+lo splits of both operands, dropping
    only the lo*lo term. Splits are power-of-2 prescaled on the host so
    the lo residuals clear e4m3's subnormal floor; the prescales are
    folded into on-chip scalars (ACT copy scale, final output scale).
  - sim / attn*v stay bf16 (K=128 per head makes DoubleRow useless for
    sim, and an es hi/lo split would cost a second full ACT/DVE pass).
  - softmax denominators: fp16 DVE partial rowsums (2x DVE mode) +
    gpsimd 128-way partition reduce; normalize+fp8-split of the context
    runs on DVE with the hi-cast offloaded to gpsimd.

Per-core PE cycles: qproj 98304 + sim 131072 + attn*v 131072 +
outproj 98304 = 458752 (vs 524288 all-f32r).

Overlap notes (modeled 219us vs 259us f32r baseline):
  - The ACT exp stream (1038ns per [128,1024] tile) paces the attention
    inner loop, so q PSUM->SBUF copies run on DVE, not ACT.
  - Wo is SBUF-resident; its chunks ride the sync DMA queue behind the
    wq head stream (same-queue order stops the scheduler from hoisting
    them into the startup-critical window - DMA bandwidth is one shared
    ~335GB/s pool, so front-running Wo starves the x/wq/kv stream).
  - Pair-0 qproj defers its batch-1 column halves into the batch-0
    attention stream so the first sim starts ~4us earlier.
  - Pair-7 attention has no qproj filler and would idle PE (the cost
    model's p-state ramp doubles the price of PE gaps): the first
    output-projection tile is trickled in 3-5 matmuls per jg there.
  - The final tile's epilogue is split per column block across the two
    DGE queues to shorten the end drain.
"""

import sys
import numpy as np
import ml_dtypes

for _p in ("/opt/trn_rl_repo", "/root/.axon_site/_ro/trn_rl_repo"):
    if _p not in sys.path:
        sys.path.append(_p)

import concourse.bass as bass  # noqa: E402
import concourse.mybir as mybir  # noqa: E402
import concourse.tile as tile  # noqa: E402
from concourse import bacc, bass_isa  # noqa: E402
from concourse.bass_utils import run_bass_kernel_spmd  # noqa: E402

F32 = mybir.dt.float32
F16 = mybir.dt.float16
BF16 = mybir.dt.bfloat16
F8 = mybir.dt.float8e4
DR = mybir.MatmulPerfMode.DoubleRow
NE4 = ml_dtypes.float8_e4m3
NBF = ml_dtypes.bfloat16

B = 2
N = 2048          # query length (global)
J = 2048          # kv length
E = 2048          # model dim
HEADS = 16
DH = 128          # head dim
NCORES = 8
NC_ROWS = N // NCORES        # 256 query rows per core per batch
R = B * NC_ROWS              # 512 rows per core, col = b*NC_ROWS + i
JT = J // 128                # 16 j-tiles
G = E // 256                 # 8 DoubleRow k-tiles over a 2048 contraction
SCALE = float(DH) ** -0.5

# host-side power-of-2 prescales for the fp8 hi/lo splits
XS = 8.0          # x
WQS = 32.0        # Wq
OS = 64.0         # normalized context (outn)
WOS = 32.0        # Wo
QDESCALE = 1.0 / (XS * WQS)      # folded into the ACT q copy
ODESCALE = 1.0 / (OS * WOS)      # folded into the final output copy

_CACHE = {}


def _build(reps: int = 1):
    nc = bacc.Bacc(name=f"mqa_xattn_dr_r{reps}")
    # x hi/lo: [p, cc(b), g, s, r256] with e = 256g + 128s + p
    xh_d = nc.declare_dram_parameter("xh", [128, 2, G, 2, NC_ROWS], F8,
                                     isOutput=False)
    xl_d = nc.declare_dram_parameter("xl", [128, 2, G, 2, NC_ROWS], F8,
                                     isOutput=False)
    wqh_d = nc.declare_dram_parameter("wqh", [HEADS, 128, G, 2, 128], F8,
                                      isOutput=False)
    wql_d = nc.declare_dram_parameter("wql", [HEADS, 128, G, 2, 128], F8,
                                      isOutput=False)
    kt_d = nc.declare_dram_parameter("kt", [128, B, J], BF16, isOutput=False)
    vt_d = nc.declare_dram_parameter("vt", [128, B, JT, DH], BF16,
                                     isOutput=False)
    woh_d = nc.declare_dram_parameter("woh", [4, 128, G, 2, 512], F8,
                                      isOutput=False)
    wol_d = nc.declare_dram_parameter("wol", [4, 128, G, 2, 512], F8,
                                      isOutput=False)
    o_d = nc.declare_dram_parameter("o", [R, E], F32, isOutput=True)

    with tile.TileContext(nc) as tc:
        for _ in range(reps):
            _emit_once(nc, tc, xh_d, xl_d, wqh_d, wql_d, kt_d, vt_d,
                       woh_d, wol_d, o_d)

    nc.compile()
    return nc


def _emit_once(nc, tc, xh_d, xl_d, wqh_d, wql_d, kt_d, vt_d,
               woh_d, wol_d, o_d):
    with tc.tile_pool(name="persist", bufs=1) as pp:
        kt_sb = pp.tile([128, B, J], BF16)
        v_sb = pp.tile([128, B, JT, DH], BF16)
        qt_all = pp.tile([128, HEADS, R], BF16)
        # context, normalized and fp8 hi/lo split, laid out for DoubleRow
        # outproj: [p, b, g, s, i] with f = 256*g + 128*s + p, i in [0,256)
        on_hi = pp.tile([128, B, G, 2, NC_ROWS], F8)
        on_lo = pp.tile([128, B, G, 2, NC_ROWS], F8)
        # Wo is fully resident; its DMAs stream on the gpsimd queue during
        # phase B so phase C starts without an SBUF/DMA stall.
        woh_sb = pp.tile([128, 4, G, 2, 512], F8)
        wol_sb = pp.tile([128, 4, G, 2, 512], F8)

        # ---- Phase B: q-projection + attention, per head pair ----
        with tc.tile_pool(name="xt_pool", bufs=1) as xtp, \
             tc.tile_pool(name="wq_pool", bufs=3) as wqp, \
             tc.tile_pool(name="es_pool", bufs=6) as esp, \
             tc.tile_pool(name="rb_pool", bufs=2) as rbp, \
             tc.tile_pool(name="qp_ps", bufs=2, space="PSUM") as qp_ps, \
             tc.tile_pool(name="sg_ps", bufs=2, space="PSUM") as sg_ps, \
             tc.tile_pool(name="acc_ps", bufs=2, space="PSUM") as acc_ps:
            xh_sb = xtp.tile([128, 2, G, 2, NC_ROWS], F8)
            xl_sb = xtp.tile([128, 2, G, 2, NC_ROWS], F8)

            # Wo prefetch chunks, paced into the sync DMA queue behind the
            # wq head stream (the scheduler keeps same-queue order, so these
            # can't hoist ahead of the startup-critical transfers).
            wo_chunks = [(dst, src, ec, g0)
                         for ec in range(4)
                         for dst, src in ((woh_sb, woh_d), (wol_sb, wol_d))
                         for g0 in (0, G // 2)]

            def load_wq(h):
                wh = wqp.tile([128, G, 2, 128], F8, tag="wqh",
                              name=f"wqh_sb{h}")
                wl = wqp.tile([128, G, 2, 128], F8, tag="wql",
                              name=f"wql_sb{h}")
                nc.sync.dma_start(wh[:], wqh_d[h])
                nc.sync.dma_start(wl[:], wql_d[h])
                if h >= 2:
                    for _ in range(2):
                        if wo_chunks:
                            dst, src, ec, g0 = wo_chunks.pop(0)
                            nc.sync.dma_start(
                                dst[:, ec, g0:g0 + G // 2],
                                src[ec, :, g0:g0 + G // 2])
                return wh, wl

            # DMA order tuned so the first qproj group starts ~1us in and
            # batch-0 attention is never input-starved.
            wqh0 = wqp.tile([128, G, 2, 128], F8, tag="wqh", name="wqh_sb0")
            wql0 = wqp.tile([128, G, 2, 128], F8, tag="wql", name="wql_sb0")
            # x stream on the scalar-engine DGE queue, weights/kv on sync:
            # transfers share one bandwidth pool but per-DMA issue dead
            # time overlaps across queues
            nc.sync.dma_start(wqh0[:, 0:2], wqh_d[0, :, 0:2])
            nc.sync.dma_start(xh_sb[:, 0, 0:2], xh_d[:, 0, 0:2])
            nc.sync.dma_start(wqh0[:, 2:G], wqh_d[0, :, 2:G])
            nc.sync.dma_start(xh_sb[:, 0, 2:G], xh_d[:, 0, 2:G])
            nc.sync.dma_start(wql0[:], wql_d[0])
            nc.sync.dma_start(xl_sb[:, 0], xl_d[:, 0])
            wq_next = (wqh0, wql0)
            wq_next2 = load_wq(1)
            nc.sync.dma_start(kt_sb[:, 0, 0:1024], kt_d[:, 0, 0:1024])
            nc.sync.dma_start(v_sb[:, 0, 0:8], vt_d[:, 0, 0:8])
            nc.sync.dma_start(xh_sb[:, 1], xh_d[:, 1])
            nc.sync.dma_start(xl_sb[:, 1], xl_d[:, 1])
            nc.sync.dma_start(kt_sb[:, 0, 1024:J], kt_d[:, 0, 1024:J])
            nc.sync.dma_start(v_sb[:, 0, 8:JT], vt_d[:, 0, 8:JT])
            nc.sync.dma_start(kt_sb[:, 1, :], kt_d[:, 1, :])
            nc.sync.dma_start(v_sb[:, 1], vt_d[:, 1])

            def qproj_head_cc(h, wh, wl, q_ps, cc):
                # 3-term hi/lo: Wh@xh + Wl@xh + Wh@xl, one 256-col half
                terms = ((wh, xh_sb), (wl, xh_sb), (wh, xl_sb))
                n_mm = len(terms) * G
                i = 0
                for wt, xt in terms:
                    for g in range(G):
                        nc.tensor.matmul(
                            q_ps[:, cc * 256:(cc + 1) * 256],
                            wt[:, g],
                            xt[:, cc, g],
                            start=(i == 0), stop=(i == n_mm - 1),
                            perf_mode=DR)
                        i += 1

            pending_cc1 = []    # pair-0 cc1 work, interleaved into b0 attn

            def qproj_pair(hp, defer_cc1=False):
                nonlocal wq_next, wq_next2
                pair_w = []
                for hh in range(2):
                    h = 2 * hp + hh
                    pair_w.append(wq_next)
                    wq_next = wq_next2
                    if h + 2 < HEADS:
                        wq_next2 = load_wq(h + 2)
                for hh in range(2):
                    h = 2 * hp + hh
                    wh, wl = pair_w[hh]
                    q_ps = qp_ps.tile([128, R], F32, tag="qp")
                    qproj_head_cc(h, wh, wl, q_ps, 0)
                    # copies on DVE, not ACT: the exp stream paces the
                    # attention tail, so ACT gets nothing extra
                    if defer_cc1:
                        with nc.allow_low_precision(reason="q -> bf16"):
                            nc.vector.tensor_scalar_mul(
                                qt_all[:, h, 0:256], q_ps[:, 0:256],
                                QDESCALE)
                        pending_cc1.append((h, wh, wl, q_ps))
                    else:
                        qproj_head_cc(h, wh, wl, q_ps, 1)
                        with nc.allow_low_precision(reason="q -> bf16"):
                            nc.vector.tensor_scalar_mul(
                                qt_all[:, h, :], q_ps[:], QDESCALE)

            def emit_ctile(ec, b, rt, ps_pool, sb_pool, ps_tag="op",
                           last=False):
                """One output-projection tile [r128, e512] (48 DR matmuls).

                last=True pipelines the epilogue per 256-col half (and
                splits the final half's DMA) to shorten the end drain.
                """
                o_ps = ps_pool.tile([128, 512], F32, tag=ps_tag)
                r0 = rt * 128

                def epilogue(c0, cw, eng=None):
                    o_sb = sb_pool.tile([128, cw], F32, tag=f"ost{cw}")
                    nc.vector.tensor_scalar_mul(o_sb[:], o_ps[:, c0:c0 + cw],
                                                ODESCALE)
                    (eng or nc.sync).dma_start(
                        o_d[b * NC_ROWS + r0:b * NC_ROWS + r0 + 128,
                            ec * 512 + c0:ec * 512 + c0 + cw],
                        o_sb[:])

                for eh in range(2):
                    e0 = eh * 256
                    terms = ((on_hi, woh_sb), (on_lo, woh_sb),
                             (on_hi, wol_sb))
                    n_mm = len(terms) * G
                    i = 0
                    for on_t, wo_t in terms:
                        for g in range(G):
                            nc.tensor.matmul(
                                o_ps[:, e0:e0 + 256],
                                on_t[:, b, g, :, r0:r0 + 128],
                                wo_t[:, ec, g, :, e0:e0 + 256],
                                start=(i == 0), stop=(i == n_mm - 1),
                                perf_mode=DR)
                            i += 1
                    if last and eh == 0:
                        epilogue(0, 256)
                if last:
                    epilogue(256, 128, eng=nc.scalar)
                    epilogue(384, 128)
                else:
                    epilogue(0, 512)

            # Pair-7 units have no qproj filler and run at the ACT exp pace:
            # trickle the first output-projection tile (ec0, b0, rt0) into
            # their PE slack, 3-5 matmuls per jg, g7 terms after pair-7's
            # b0 context exists. Keeps PE continuously busy (the cost
            # model's p-state ramp doubles the price of any PE idle gap).
            CTERMS = lambda: ((on_hi, woh_sb), (on_lo, woh_sb),  # noqa: E731
                              (on_hi, wol_sb))
            trickle = {"q": [], "ops": None}

            def trickle_init():
                trickle["ops"] = qp_ps.tile([128, 512], F32, tag="qp",
                                            name="ct_ops")
                q = []
                for eh in range(2):
                    main = [(eh, t, g) for g in range(G - 1)
                            for t in range(3)]
                    last = [(eh, t, G - 1) for t in range(3)]
                    q += main + last
                trickle["q"] = q

            def trickle_emit(n):
                o_ps = trickle["ops"]
                for _ in range(n):
                    if not trickle["q"]:
                        return
                    i = 48 - len(trickle["q"])
                    eh, t, g = trickle["q"].pop(0)
                    on_t, wo_t = CTERMS()[t]
                    nc.tensor.matmul(
                        o_ps[:, eh * 256:eh * 256 + 256],
                        on_t[:, 0, g, :, 0:128],
                        wo_t[:, 0, g, :, eh * 256:eh * 256 + 256],
                        start=(i % 24 == 0), stop=(i % 24 == 23),
                        perf_mode=DR)

            TRICKLE_SLOTS = {(0, jg): 3 for jg in range(1, 8)}
            TRICKLE_SLOTS.update({(1, 2): 3, (1, 3): 4, (1, 4): 4,
                                  (1, 5): 4, (1, 6): 4, (1, 7): 5})

            qproj_pair(0, defer_cc1=True)
            for hp in range(HEADS // 2):
                for b in range(B):
                    if b == 1 and hp + 1 < HEADS // 2:
                        qproj_pair(hp + 1)
                    if hp == HEADS // 2 - 1 and b == 0:
                        trickle_init()
                    # Both heads of the pair processed together: every matmul
                    # has a 512-wide moving operand laid out as [h2, i256].
                    acc = acc_ps.tile([128, 512], F32, tag="acc")
                    qt_pair = qt_all[:, 2 * hp:2 * hp + 2,
                                     b * NC_ROWS:(b + 1) * NC_ROWS]
                    s1024 = rbp.tile([128, 1024], F16, tag="s128")
                    # during the final attention unit the qproj PSUM banks
                    # are idle and all batch-0 context is split: inject
                    # early output-projection tiles to fill the ACT-paced
                    # tail of phase B
                    inject = False and (hp == HEADS // 2 - 1 and b == 1)
                    for jg in range(JT // 2):
                        if inject and jg in (1, 3, 5, 7):
                            ti = (1, 3, 5, 7).index(jg)
                            emit_ctile(ti // 2, 0, ti % 2, qp_ps, rbp,
                                       ps_tag="qp")
                        if pending_cc1 and hp == 0 and b == 0 \
                                and jg in (1, 3):
                            h, wh, wl, q_ps = pending_cc1.pop(0)
                            qproj_head_cc(h, wh, wl, q_ps, 1)
                            with nc.allow_low_precision(reason="q -> bf16"):
                                nc.vector.tensor_scalar_mul(
                                    qt_all[:, h, 256:512], q_ps[:, 256:512],
                                    QDESCALE)
                        sg = sg_ps.tile([128, 1024], F32, tag="sg")
                        for kk in range(2):
                            jt = jg * 2 + kk
                            nc.tensor.matmul(
                                sg[:, kk * 512:(kk + 1) * 512],
                                kt_sb[:, b, jt * 128:(jt + 1) * 128],
                                qt_pair,
                                start=True, stop=True)
                        es = esp.tile([128, 1024], BF16, tag="es")
                        with nc.allow_low_precision(reason="es bf16"):
                            nc.scalar.activation(
                                es[:], sg[:],
                                mybir.ActivationFunctionType.Exp,
                                scale=SCALE)
                            # softmax denominators: fp16 partial rowsums on
                            # DVE (2x 16-bit mode); partition reduce below
                            if jg == 0:
                                nc.vector.tensor_copy(s1024[:], es[:])
                            else:
                                nc.vector.tensor_add(s1024[:], s1024[:],
                                                     es[:])
                        if hp == HEADS // 2 - 1 and (b, jg) in TRICKLE_SLOTS:
                            trickle_emit(TRICKLE_SLOTS[(b, jg)])
                        for kk in range(2):
                            jt = jg * 2 + kk
                            esk = es[:, kk * 512:(kk + 1) * 512]
                            nc.tensor.matmul(acc[:], v_sb[:, b, jt, :],
                                             esk, start=(jt == 0),
                                             stop=(jt == JT - 1))
                    # softmax-denominator tail + context fp8 hi/lo split
                    s512 = rbp.tile([128, 512], F32, tag="s512", bufs=1)
                    sB = rbp.tile([128, 512], F32, tag="sB", bufs=1)
                    rb_sb = rbp.tile([128, 512], F32, tag="rbs")
                    t32 = rbp.tile([128, 512], F32, tag="t32")
                    hi_ap = on_hi[:, b, hp].rearrange("p a b -> p (a b)")
                    lo_ap = on_lo[:, b, hp].rearrange("p a b -> p (a b)")
                    with nc.allow_low_precision(reason="denominator tail"):
                        nc.vector.tensor_add(s512[:], s1024[:, 0:512],
                                             s1024[:, 512:1024])
                        nc.gpsimd.partition_all_reduce(
                            sB[:], s512[:], channels=128,
                            reduce_op=bass_isa.ReduceOp.add)
                        nc.vector.reciprocal(rb_sb[:], sB[:])
                        nc.vector.tensor_mul(t32[:], acc[:], rb_sb[:])
                        nc.gpsimd.tensor_scalar_mul(hi_ap, t32[:], OS)
                        nc.vector.scalar_tensor_tensor(
                            lo_ap, t32[:], OS, hi_ap,
                            mybir.AluOpType.mult,
                            mybir.AluOpType.subtract)
                    if hp == HEADS // 2 - 1 and b == 1:
                        trickle_emit(3)  # leftover g7 terms of tile A
                        o_sb = rbp.tile([128, 512], F32, tag="ost512")
                        nc.vector.tensor_scalar_mul(
                            o_sb[:], trickle["ops"][:], ODESCALE)
                        nc.sync.dma_start(o_d[0:128, 0:512], o_sb[:])

        # ---- Phase C: remaining output-projection tiles ----
        # (ec0/ec1, b0, *) were injected into the tail of phase B above.
        with tc.tile_pool(name="ost_pool", bufs=4) as ostp, \
             tc.tile_pool(name="op_ps", bufs=4, space="PSUM") as op_ps:
            tiles = [(ec, b, rt) for ec in range(4) for b in (0, 1)
                     for rt in (0, 1) if (ec, b, rt) != (0, 0, 0)]
            for ti, (ec, b, rt) in enumerate(tiles):
                emit_ctile(ec, b, rt, op_ps, ostp,
                           last=(ti == len(tiles) - 1))


def _get_nc(reps: int = 1):
    if reps not in _CACHE:
        _CACHE[reps] = _build(reps)
    return _CACHE[reps]


def _hilo(a, pre):
    s = (a * pre).astype(np.float32)
    hi = s.astype(NE4)
    lo = (s - hi.astype(np.float32)).astype(NE4)
    return hi, lo


def _make_in_maps(x, k, v, Wq, Wo):
    # Wq [E, inner] -> [h, p, g, s, f] with e = 256g + 128s + p
    wq_t = Wq.reshape(G, 2, 128, HEADS, 128).transpose(3, 2, 0, 1, 4)
    wqh, wql = _hilo(np.ascontiguousarray(wq_t), WQS)
    # Wo [inner, E] -> [ec, p, g, s, e'] with f = 256g + 128s + p
    wo_t = Wo.reshape(G, 2, 128, 4, 512).transpose(3, 2, 0, 1, 4)
    woh, wol = _hilo(np.ascontiguousarray(wo_t), WOS)
    # k [B, J, DH] -> kT [d, b, j]
    kt = np.ascontiguousarray(k.transpose(2, 0, 1)).astype(NBF)
    # v [B, J, DH] -> [p, b, jt, d]
    vt = np.ascontiguousarray(
        v.reshape(B, JT, 128, DH).transpose(2, 0, 1, 3)).astype(NBF)
    in_maps = []
    for c in range(NCORES):
        xs = x[:, c * NC_ROWS:(c + 1) * NC_ROWS, :]
        # [E, cc, r256] -> [p, cc, g, s, r]
        xt = np.stack([xs[0].T, xs[1].T], axis=1)
        xt = np.ascontiguousarray(
            xt.reshape(G, 2, 128, 2, NC_ROWS).transpose(2, 3, 0, 1, 4))
        xh, xl = _hilo(xt, XS)
        in_maps.append({"xh": xh, "xl": xl, "wqh": wqh, "wql": wql,
                        "kt": kt, "vt": vt, "woh": woh, "wol": wol})
    return in_maps


def run_on_device(x, k, v, Wq, Wo, reps: int = 1):
    nc = _get_nc(reps)
    in_maps = _make_in_maps(x, k, v, Wq, Wo)
    res = run_bass_kernel_spmd(nc, in_maps, list(range(NCORES)))
    parts = [res.results[c]["o"].reshape(B, NC_ROWS, E) for c in range(NCORES)]
    return np.concatenate(parts, axis=1)


def kernel(x, k, v, Wq, Wo):
    x = np.asarray(x, dtype=np.float32)
    k = np.asarray(k, dtype=np.float32)
    v = np.asarray(v, dtype=np.float32)
    Wq = np.asarray(Wq, dtype=np.float32)
    Wo = np.asarray(Wo, dtype=np.float32)
    return run_on_device(x, k, v, Wq, Wo, reps=1)


# revision 74
# speedup vs baseline: 1.0207x; 1.0207x over previous
"""Trainium2 Bass kernel for MQA cross-attention (nn_CrossAttention).

Reference computation (fp32):
    q = (x @ Wq).reshape(b, n, 16, 128).transpose(0,2,1,3) * 128**-0.5
    sim = q @ k^T   (k/v shared across heads, MQA)
    out = softmax(sim) @ v
    y = out.merge_heads @ Wo

Sharding: pure sequence-parallel across 8 cores. Each core gets 256 rows
of x per batch (512 rows total), full Wq/Wo/k/v, and produces its 512 rows
of the output. No collectives, no host-side reduction.

Mixed precision (validated vs reference, rel err ~4e-3):
  - qproj / outproj run as fp8e4 DoubleRow matmuls (0.5 cycles/row,
    256-deep contraction) with hi+lo splits of both operands, dropping
    only the lo*lo term. Splits are power-of-2 prescaled on the host so
    the lo residuals clear e4m3's subnormal floor; the prescales are
    folded into on-chip scalars (ACT copy scale, final output scale).
  - sim / attn*v stay bf16 (K=128 per head makes DoubleRow useless for
    sim, and an es hi/lo split would cost a second full ACT/DVE pass).
  - softmax denominators: fp16 DVE partial rowsums (2x DVE mode) +
    gpsimd 128-way partition reduce; normalize+fp8-split of the context
    runs on DVE with the hi-cast offloaded to gpsimd.

Per-core PE cycles: qproj 98304 + sim 131072 + attn*v 131072 +
outproj 98304 = 458752 (vs 524288 all-f32r).

Overlap notes (modeled 214us vs 259us f32r baseline):
  - The ACT exp stream (1038ns per [128,1024] tile) paces the attention
    inner loop, so q PSUM->SBUF copies run on DVE, not ACT.
  - qproj for pair hp+1 is drip-fed a few matmuls per jg into pair hp's
    attention stream instead of bursting: during a burst ACT starves
    (sg double-buffering banks only 2 jg of sim backlog) and loses the
    lead it needs to cover the per-jg exp deficit.
  - Wo is SBUF-resident; its chunks ride the sync DMA queue behind the
    wq head stream (same-queue order stops the scheduler from hoisting
    them into the startup-critical window - DMA bandwidth is one shared
    ~335GB/s pool, so front-running Wo starves the x/wq/kv stream).
  - Pair-0 qproj defers its batch-1 column halves into the batch-0
    attention stream so the first sim starts ~4us earlier.
  - Pair-7 has no qproj filler: the first two output-projection tiles
    are trickled into its PE slack (3 matmuls per jg, g7 terms queued
    last since they need pair-7's own context). PE gaps are doubly
    expensive under the cost model's p-state ramp.
  - The final tile's epilogue is split per column block across the two
    DGE queues to shorten the end drain.
"""

import sys
import numpy as np
import ml_dtypes

for _p in ("/opt/trn_rl_repo", "/root/.axon_site/_ro/trn_rl_repo"):
    if _p not in sys.path:
        sys.path.append(_p)

import concourse.bass as bass  # noqa: E402
import concourse.mybir as mybir  # noqa: E402
import concourse.tile as tile  # noqa: E402
from concourse import bacc, bass_isa  # noqa: E402
from concourse.bass_utils import run_bass_kernel_spmd  # noqa: E402

F32 = mybir.dt.float32
F16 = mybir.dt.float16
BF16 = mybir.dt.bfloat16
F8 = mybir.dt.float8e4
DR = mybir.MatmulPerfMode.DoubleRow
NE4 = ml_dtypes.float8_e4m3
NBF = ml_dtypes.bfloat16

B = 2
N = 2048          # query length (global)
J = 2048          # kv length
E = 2048          # model dim
HEADS = 16
DH = 128          # head dim
NCORES = 8
NC_ROWS = N // NCORES        # 256 query rows per core per batch
R = B * NC_ROWS              # 512 rows per core, col = b*NC_ROWS + i
JT = J // 128                # 16 j-tiles
G = E // 256                 # 8 DoubleRow k-tiles over a 2048 contraction
SCALE = float(DH) ** -0.5

# host-side power-of-2 prescales for the fp8 hi/lo splits
XS = 8.0          # x
WQS = 32.0        # Wq
OS = 64.0         # normalized context (outn)
WOS = 32.0        # Wo
QDESCALE = 1.0 / (XS * WQS)      # folded into the ACT q copy
ODESCALE = 1.0 / (OS * WOS)      # folded into the final output copy

_CACHE = {}


def _build(reps: int = 1):
    nc = bacc.Bacc(name=f"mqa_xattn_dr_r{reps}")
    # x hi/lo: [p, cc(b), g, s, r256] with e = 256g + 128s + p
    xh_d = nc.declare_dram_parameter("xh", [128, 2, G, 2, NC_ROWS], F8,
                                     isOutput=False)
    xl_d = nc.declare_dram_parameter("xl", [128, 2, G, 2, NC_ROWS], F8,
                                     isOutput=False)
    wqh_d = nc.declare_dram_parameter("wqh", [HEADS, 128, G, 2, 128], F8,
                                      isOutput=False)
    wql_d = nc.declare_dram_parameter("wql", [HEADS, 128, G, 2, 128], F8,
                                      isOutput=False)
    kt_d = nc.declare_dram_parameter("kt", [128, B, J], BF16, isOutput=False)
    vt_d = nc.declare_dram_parameter("vt", [128, B, JT, DH], BF16,
                                     isOutput=False)
    woh_d = nc.declare_dram_parameter("woh", [4, 128, G, 2, 512], F8,
                                      isOutput=False)
    wol_d = nc.declare_dram_parameter("wol", [4, 128, G, 2, 512], F8,
                                      isOutput=False)
    o_d = nc.declare_dram_parameter("o", [R, E], F32, isOutput=True)

    with tile.TileContext(nc) as tc:
        for _ in range(reps):
            _emit_once(nc, tc, xh_d, xl_d, wqh_d, wql_d, kt_d, vt_d,
                       woh_d, wol_d, o_d)

    nc.compile()
    return nc


def _emit_once(nc, tc, xh_d, xl_d, wqh_d, wql_d, kt_d, vt_d,
               woh_d, wol_d, o_d):
    with tc.tile_pool(name="persist", bufs=1) as pp:
        kt_sb = pp.tile([128, B, J], BF16)
        v_sb = pp.tile([128, B, JT, DH], BF16)
        qt_all = pp.tile([128, HEADS, R], BF16)
        # context, normalized and fp8 hi/lo split, laid out for DoubleRow
        # outproj: [p, b, g, s, i] with f = 256*g + 128*s + p, i in [0,256)
        on_hi = pp.tile([128, B, G, 2, NC_ROWS], F8)
        on_lo = pp.tile([128, B, G, 2, NC_ROWS], F8)
        # Wo is fully resident; its DMAs stream on the gpsimd queue during
        # phase B so phase C starts without an SBUF/DMA stall.
        woh_sb = pp.tile([128, 4, G, 2, 512], F8)
        wol_sb = pp.tile([128, 4, G, 2, 512], F8)

        # ---- Phase B: q-projection + attention, per head pair ----
        with tc.tile_pool(name="xt_pool", bufs=1) as xtp, \
             tc.tile_pool(name="wq_pool", bufs=3) as wqp, \
             tc.tile_pool(name="es_pool", bufs=6) as esp, \
             tc.tile_pool(name="rb_pool", bufs=2) as rbp, \
             tc.tile_pool(name="qp_ps", bufs=2, space="PSUM") as qp_ps, \
             tc.tile_pool(name="sg_ps", bufs=2, space="PSUM") as sg_ps, \
             tc.tile_pool(name="acc_ps", bufs=2, space="PSUM") as acc_ps:
            xh_sb = xtp.tile([128, 2, G, 2, NC_ROWS], F8)
            xl_sb = xtp.tile([128, 2, G, 2, NC_ROWS], F8)

            # Wo prefetch chunks, paced into the sync DMA queue behind the
            # wq head stream (the scheduler keeps same-queue order, so these
            # can't hoist ahead of the startup-critical transfers).
            wo_chunks = [(dst, src, ec, g0)
                         for ec in range(4)
                         for dst, src in ((woh_sb, woh_d), (wol_sb, wol_d))
                         for g0 in (0, G // 2)]

            def load_wq(h):
                wh = wqp.tile([128, G, 2, 128], F8, tag="wqh",
                              name=f"wqh_sb{h}")
                wl = wqp.tile([128, G, 2, 128], F8, tag="wql",
                              name=f"wql_sb{h}")
                nc.sync.dma_start(wh[:], wqh_d[h])
                nc.sync.dma_start(wl[:], wql_d[h])
                if h >= 2:
                    for _ in range(2):
                        if wo_chunks:
                            dst, src, ec, g0 = wo_chunks.pop(0)
                            nc.sync.dma_start(
                                dst[:, ec, g0:g0 + G // 2],
                                src[ec, :, g0:g0 + G // 2])
                return wh, wl

            # DMA order tuned so the first qproj group starts ~1us in and
            # batch-0 attention is never input-starved.
            wqh0 = wqp.tile([128, G, 2, 128], F8, tag="wqh", name="wqh_sb0")
            wql0 = wqp.tile([128, G, 2, 128], F8, tag="wql", name="wql_sb0")
            # x stream on the scalar-engine DGE queue, weights/kv on sync:
            # transfers share one bandwidth pool but per-DMA issue dead
            # time overlaps across queues
            nc.sync.dma_start(wqh0[:, 0:2], wqh_d[0, :, 0:2])
            nc.sync.dma_start(xh_sb[:, 0, 0:2], xh_d[:, 0, 0:2])
            nc.sync.dma_start(wqh0[:, 2:G], wqh_d[0, :, 2:G])
            nc.sync.dma_start(xh_sb[:, 0, 2:G], xh_d[:, 0, 2:G])
            nc.sync.dma_start(wql0[:], wql_d[0])
            nc.sync.dma_start(xl_sb[:, 0], xl_d[:, 0])
            wq_next = (wqh0, wql0)
            wq_next2 = load_wq(1)
            nc.sync.dma_start(kt_sb[:, 0, 0:1024], kt_d[:, 0, 0:1024])
            nc.sync.dma_start(v_sb[:, 0, 0:8], vt_d[:, 0, 0:8])
            nc.sync.dma_start(xh_sb[:, 1], xh_d[:, 1])
            nc.sync.dma_start(xl_sb[:, 1], xl_d[:, 1])
            nc.sync.dma_start(kt_sb[:, 0, 1024:J], kt_d[:, 0, 1024:J])
            nc.sync.dma_start(v_sb[:, 0, 8:JT], vt_d[:, 0, 8:JT])
            nc.sync.dma_start(kt_sb[:, 1, :], kt_d[:, 1, :])
            nc.sync.dma_start(v_sb[:, 1], vt_d[:, 1])

            def qproj_head_cc(h, wh, wl, q_ps, cc):
                # 3-term hi/lo: Wh@xh + Wl@xh + Wh@xl, one 256-col half
                terms = ((wh, xh_sb), (wl, xh_sb), (wh, xl_sb))
                n_mm = len(terms) * G
                i = 0
                for wt, xt in terms:
                    for g in range(G):
                        nc.tensor.matmul(
                            q_ps[:, cc * 256:(cc + 1) * 256],
                            wt[:, g],
                            xt[:, cc, g],
                            start=(i == 0), stop=(i == n_mm - 1),
                            perf_mode=DR)
                        i += 1

            pending_cc1 = []    # pair-0 cc1 work, interleaved into b0 attn

            # qproj for pair hp+1 is not emitted as a burst (ACT starves
            # during bursts: sg double-buffering banks only 2 jg of sim
            # backlog, so the exp stream idles and loses its lead). It is
            # drip-fed 3 matmuls at a time into pair hp's attention stream,
            # matching the per-jg ACT deficit.
            qtrickle = {"q": []}

            def build_qtrickle(hp1):
                nonlocal wq_next, wq_next2
                pw = []
                for hh in range(2):
                    h = 2 * hp1 + hh
                    pw.append(wq_next)
                    wq_next = wq_next2
                    if h + 2 < HEADS:
                        wq_next2 = load_wq(h + 2)
                tiles_ = [qp_ps.tile([128, R], F32, tag="qp",
                                     name=f"qpt{hp1}_{i}") for i in range(2)]
                q = []
                # cc0 groups (both heads) first: the next pair's batch-0
                # sim needs only the cc0 halves of qt
                for cc in range(2):
                    for hh in range(2):
                        h = 2 * hp1 + hh
                        wh, wl = pw[hh]
                        terms = ((wh, xh_sb), (wl, xh_sb), (wh, xl_sb))
                        n = 0
                        for wt, xt in terms:
                            for g in range(G):
                                q.append(("mm", tiles_[hh], wt, xt, cc, g,
                                          n == 0, n == 3 * G - 1))
                                n += 1
                        q.append(("copy", tiles_[hh], h, cc))
                qtrickle["q"] = q

            def qdrip(nmm):
                done = 0
                while qtrickle["q"] and done < nmm:
                    e = qtrickle["q"].pop(0)
                    if e[0] == "copy":
                        _, t, h, cc = e
                        with nc.allow_low_precision(reason="q -> bf16"):
                            nc.vector.tensor_scalar_mul(
                                qt_all[:, h, cc * 256:(cc + 1) * 256],
                                t[:, cc * 256:(cc + 1) * 256], QDESCALE)
                        continue
                    _, t, wt, xt, cc, g, st, sp = e
                    nc.tensor.matmul(t[:, cc * 256:(cc + 1) * 256],
                                     wt[:, g], xt[:, cc, g],
                                     start=st, stop=sp, perf_mode=DR)
                    done += 1

            def qproj_pair(hp, defer_cc1=False):
                nonlocal wq_next, wq_next2
                pair_w = []
                for hh in range(2):
                    h = 2 * hp + hh
                    pair_w.append(wq_next)
                    wq_next = wq_next2
                    if h + 2 < HEADS:
                        wq_next2 = load_wq(h + 2)
                for hh in range(2):
                    h = 2 * hp + hh
                    wh, wl = pair_w[hh]
                    q_ps = qp_ps.tile([128, R], F32, tag="qp")
                    qproj_head_cc(h, wh, wl, q_ps, 0)
                    # copies on DVE, not ACT: the exp stream paces the
                    # attention tail, so ACT gets nothing extra
                    if defer_cc1:
                        with nc.allow_low_precision(reason="q -> bf16"):
                            nc.vector.tensor_scalar_mul(
                                qt_all[:, h, 0:256], q_ps[:, 0:256],
                                QDESCALE)
                        pending_cc1.append((h, wh, wl, q_ps))
                    else:
                        qproj_head_cc(h, wh, wl, q_ps, 1)
                        with nc.allow_low_precision(reason="q -> bf16"):
                            nc.vector.tensor_scalar_mul(
                                qt_all[:, h, :], q_ps[:], QDESCALE)

            def emit_ctile(ec, b, rt, ps_pool, sb_pool, ps_tag="op",
                           last=False):
                """One output-projection tile [r128, e512] (48 DR matmuls).

                last=True pipelines the epilogue per 256-col half (and
                splits the final half's DMA) to shorten the end drain.
                """
                o_ps = ps_pool.tile([128, 512], F32, tag=ps_tag)
                r0 = rt * 128

                def epilogue(c0, cw, eng=None):
                    o_sb = sb_pool.tile([128, cw], F32, tag=f"ost{cw}")
                    nc.vector.tensor_scalar_mul(o_sb[:], o_ps[:, c0:c0 + cw],
                                                ODESCALE)
                    (eng or nc.sync).dma_start(
                        o_d[b * NC_ROWS + r0:b * NC_ROWS + r0 + 128,
                            ec * 512 + c0:ec * 512 + c0 + cw],
                        o_sb[:])

                for eh in range(2):
                    e0 = eh * 256
                    terms = ((on_hi, woh_sb), (on_lo, woh_sb),
                             (on_hi, wol_sb))
                    n_mm = len(terms) * G
                    i = 0
                    for on_t, wo_t in terms:
                        for g in range(G):
                            nc.tensor.matmul(
                                o_ps[:, e0:e0 + 256],
                                on_t[:, b, g, :, r0:r0 + 128],
                                wo_t[:, ec, g, :, e0:e0 + 256],
                                start=(i == 0), stop=(i == n_mm - 1),
                                perf_mode=DR)
                            i += 1
                    if last and eh == 0:
                        epilogue(0, 256)
                if last:
                    epilogue(256, 128, eng=nc.scalar)
                    epilogue(384, 128)
                else:
                    epilogue(0, 512)

            # Pair-7 units have no qproj filler and run at the ACT exp pace:
            # trickle the first output-projection tile (ec0, b0, rt0) into
            # their PE slack, 3-5 matmuls per jg, g7 terms after pair-7's
            # b0 context exists. Keeps PE continuously busy (the cost
            # model's p-state ramp doubles the price of any PE idle gap).
            CTERMS = lambda: ((on_hi, woh_sb), (on_lo, woh_sb),  # noqa: E731
                              (on_hi, wol_sb))
            trickle = {"q": [], "ops": None}

            CTRICKLE_TILES = [(0, 0, 0), (0, 0, 1)]

            def trickle_init():
                q = []
                trickle["tiles"] = []
                for ec, tb, rt in CTRICKLE_TILES:
                    ops = qp_ps.tile([128, 512], F32, tag="qp",
                                     name=f"ct_ops{ec}{tb}{rt}")
                    trickle["tiles"].append((ops, ec, tb, rt))
                    for eh in range(2):
                        idx = [(t, g) for g in range(G - 1)
                               for t in range(3)]
                        idx += [(t, G - 1) for t in range(3)]
                        for i, (t, g) in enumerate(idx):
                            q.append((ops, ec, tb, rt, eh, t, g,
                                      i == 0, i == 3 * G - 1))
                trickle["q"] = q

            def trickle_emit(n):
                for _ in range(n):
                    if not trickle["q"]:
                        return
                    ops, ec, tb, rt, eh, t, g, st, sp = \
                        trickle["q"].pop(0)
                    on_t, wo_t = CTERMS()[t]
                    nc.tensor.matmul(
                        ops[:, eh * 256:eh * 256 + 256],
                        on_t[:, tb, g, :, rt * 128:rt * 128 + 128],
                        wo_t[:, ec, g, :, eh * 256:eh * 256 + 256],
                        start=st, stop=sp, perf_mode=DR)

            # b0 may drain at most 21 entries (tile A's eh0 g0-6): anything
            # later in the queue reads pair-7's own context, written by the
            # b0 tail which is EMITTED after b0's jg slots - an earlier
            # read would see uninitialized SBUF with no semaphore guard.
            TRICKLE_SLOTS = {(0, jg): 3 for jg in range(1, 8)}
            TRICKLE_SLOTS.update({(1, jg): 3 for jg in range(8)})

            qproj_pair(0, defer_cc1=True)
            for hp in range(HEADS // 2):
                for b in range(B):
                    if b == 0 and hp + 1 < HEADS // 2:
                        build_qtrickle(hp + 1)
                    if hp == HEADS // 2 - 1 and b == 0:
                        trickle_init()
                    # Both heads of the pair processed together: every matmul
                    # has a 512-wide moving operand laid out as [h2, i256].
                    acc = acc_ps.tile([128, 512], F32, tag="acc")
                    qt_pair = qt_all[:, 2 * hp:2 * hp + 2,
                                     b * NC_ROWS:(b + 1) * NC_ROWS]
                    s1024 = rbp.tile([128, 1024], F16, tag="s128")
                    # during the final attention unit the qproj PSUM banks
                    # are idle and all batch-0 context is split: inject
                    # early output-projection tiles to fill the ACT-paced
                    # tail of phase B
                    inject = False and (hp == HEADS // 2 - 1 and b == 1)
                    for jg in range(JT // 2):
                        if inject and jg in (1, 3, 5, 7):
                            ti = (1, 3, 5, 7).index(jg)
                            emit_ctile(ti // 2, 0, ti % 2, qp_ps, rbp,
                                       ps_tag="qp")
                        if pending_cc1 and hp == 0 and b == 0 \
                                and jg in (1, 3):
                            h, wh, wl, q_ps = pending_cc1.pop(0)
                            qproj_head_cc(h, wh, wl, q_ps, 1)
                            with nc.allow_low_precision(reason="q -> bf16"):
                                nc.vector.tensor_scalar_mul(
                                    qt_all[:, h, 256:512], q_ps[:, 256:512],
                                    QDESCALE)
                        sg = sg_ps.tile([128, 1024], F32, tag="sg")
                        for kk in range(2):
                            jt = jg * 2 + kk
                            nc.tensor.matmul(
                                sg[:, kk * 512:(kk + 1) * 512],
                                kt_sb[:, b, jt * 128:(jt + 1) * 128],
                                qt_pair,
                                start=True, stop=True)
                        es = esp.tile([128, 1024], BF16, tag="es")
                        with nc.allow_low_precision(reason="es bf16"):
                            nc.scalar.activation(
                                es[:], sg[:],
                                mybir.ActivationFunctionType.Exp,
                                scale=SCALE)
                            # softmax denominators: fp16 partial rowsums on
                            # DVE (2x 16-bit mode); partition reduce below
                            if jg == 0:
                                nc.vector.tensor_copy(s1024[:], es[:])
                            else:
                                nc.vector.tensor_add(s1024[:], s1024[:],
                                                     es[:])
                        if hp == HEADS // 2 - 1 and (b, jg) in TRICKLE_SLOTS:
                            trickle_emit(TRICKLE_SLOTS[(b, jg)])
                        qdrip(2)
                        for kk in range(2):
                            jt = jg * 2 + kk
                            esk = es[:, kk * 512:(kk + 1) * 512]
                            nc.tensor.matmul(acc[:], v_sb[:, b, jt, :],
                                             esk, start=(jt == 0),
                                             stop=(jt == JT - 1))
                        qdrip(4)
                    if b == 1:
                        qdrip(10 ** 6)  # force-drain before the next pair
                    # softmax-denominator tail + context fp8 hi/lo split
                    s512 = rbp.tile([128, 512], F32, tag="s512", bufs=1)
                    sB = rbp.tile([128, 512], F32, tag="sB", bufs=1)
                    rb_sb = rbp.tile([128, 512], F32, tag="rbs")
                    t32 = rbp.tile([128, 512], F32, tag="t32")
                    hi_ap = on_hi[:, b, hp].rearrange("p a b -> p (a b)")
                    lo_ap = on_lo[:, b, hp].rearrange("p a b -> p (a b)")
                    with nc.allow_low_precision(reason="denominator tail"):
                        nc.vector.tensor_add(s512[:], s1024[:, 0:512],
                                             s1024[:, 512:1024])
                        nc.gpsimd.partition_all_reduce(
                            sB[:], s512[:], channels=128,
                            reduce_op=bass_isa.ReduceOp.add)
                        nc.vector.reciprocal(rb_sb[:], sB[:])
                        nc.vector.tensor_mul(t32[:], acc[:], rb_sb[:])
                        nc.gpsimd.tensor_scalar_mul(hi_ap, t32[:], OS)
                        nc.vector.scalar_tensor_tensor(
                            lo_ap, t32[:], OS, hi_ap,
                            mybir.AluOpType.mult,
                            mybir.AluOpType.subtract)
                    if hp == HEADS // 2 - 1 and b == 1:
                        trickle_emit(10 ** 6)  # drain leftover tile work
                        for ops, ec, tb, rt in trickle["tiles"]:
                            o_sb = rbp.tile([128, 512], F32, tag="ost512")
                            nc.vector.tensor_scalar_mul(o_sb[:], ops[:],
                                                        ODESCALE)
                            nc.sync.dma_start(
                                o_d[tb * NC_ROWS + rt * 128:
                                    tb * NC_ROWS + rt * 128 + 128,
                                    ec * 512:(ec + 1) * 512],
                                o_sb[:])

        # ---- Phase C: remaining output-projection tiles ----
        # (ec0/ec1, b0, *) were injected into the tail of phase B above.
        with tc.tile_pool(name="ost_pool", bufs=4) as ostp, \
             tc.tile_pool(name="op_ps", bufs=4, space="PSUM") as op_ps:
            tiles = [(ec, 0, rt) for ec in range(4) for rt in (0, 1)
                     if (ec, 0, rt) not in ((0, 0, 0), (0, 0, 1))]
            tiles += [(ec, 1, rt) for ec in range(4) for rt in (0, 1)]
            for ti, (ec, b, rt) in enumerate(tiles):
                emit_ctile(ec, b, rt, op_ps, ostp,
                           last=(ti == len(tiles) - 1))


def _get_nc(reps: int = 1):
    if reps not in _CACHE:
        _CACHE[reps] = _build(reps)
    return _CACHE[reps]


def _hilo(a, pre):
    s = (a * pre).astype(np.float32)
    hi = s.astype(NE4)
    lo = (s - hi.astype(np.float32)).astype(NE4)
    return hi, lo


def _make_in_maps(x, k, v, Wq, Wo):
    # Wq [E, inner] -> [h, p, g, s, f] with e = 256g + 128s + p
    wq_t = Wq.reshape(G, 2, 128, HEADS, 128).transpose(3, 2, 0, 1, 4)
    wqh, wql = _hilo(np.ascontiguousarray(wq_t), WQS)
    # Wo [inner, E] -> [ec, p, g, s, e'] with f = 256g + 128s + p
    wo_t = Wo.reshape(G, 2, 128, 4, 512).transpose(3, 2, 0, 1, 4)
    woh, wol = _hilo(np.ascontiguousarray(wo_t), WOS)
    # k [B, J, DH] -> kT [d, b, j]
    kt = np.ascontiguousarray(k.transpose(2, 0, 1)).astype(NBF)
    # v [B, J, DH] -> [p, b, jt, d]
    vt = np.ascontiguousarray(
        v.reshape(B, JT, 128, DH).transpose(2, 0, 1, 3)).astype(NBF)
    in_maps = []
    for c in range(NCORES):
        xs = x[:, c * NC_ROWS:(c + 1) * NC_ROWS, :]
        # [E, cc, r256] -> [p, cc, g, s, r]
        xt = np.stack([xs[0].T, xs[1].T], axis=1)
        xt = np.ascontiguousarray(
            xt.reshape(G, 2, 128, 2, NC_ROWS).transpose(2, 3, 0, 1, 4))
        xh, xl = _hilo(xt, XS)
        in_maps.append({"xh": xh, "xl": xl, "wqh": wqh, "wql": wql,
                        "kt": kt, "vt": vt, "woh": woh, "wol": wol})
    return in_maps


def run_on_device(x, k, v, Wq, Wo, reps: int = 1):
    nc = _get_nc(reps)
    in_maps = _make_in_maps(x, k, v, Wq, Wo)
    res = run_bass_kernel_spmd(nc, in_maps, list(range(NCORES)))
    parts = [res.results[c]["o"].reshape(B, NC_ROWS, E) for c in range(NCORES)]
    return np.concatenate(parts, axis=1)


def kernel(x, k, v, Wq, Wo):
    x = np.asarray(x, dtype=np.float32)
    k = np.asarray(k, dtype=np.float32)
    v = np.asarray(v, dtype=np.float32)
    Wq = np.asarray(Wq, dtype=np.float32)
    Wo = np.asarray(Wo, dtype=np.float32)
    return run_on_device(x, k, v, Wq, Wo, reps=1)


# revision 84
# speedup vs baseline: 1.0260x; 1.0052x over previous
"""Trainium2 Bass kernel for MQA cross-attention (nn_CrossAttention).

Reference computation (fp32):
    q = (x @ Wq).reshape(b, n, 16, 128).transpose(0,2,1,3) * 128**-0.5
    sim = q @ k^T   (k/v shared across heads, MQA)
    out = softmax(sim) @ v
    y = out.merge_heads @ Wo

Sharding: pure sequence-parallel across 8 cores. Each core gets 256 rows
of x per batch (512 rows total), full Wq/Wo/k/v, and produces its 512 rows
of the output. No collectives, no host-side reduction.

Mixed precision (validated vs reference, rel err ~4e-3):
  - qproj / outproj run as fp8e4 DoubleRow matmuls (0.5 cycles/row,
    256-deep contraction) with hi+lo splits of both operands, dropping
    only the lo*lo term. Splits are power-of-2 prescaled on the host so
    the lo residuals clear e4m3's subnormal floor; the prescales are
    folded into on-chip scalars (ACT copy scale, final output scale).
  - sim / attn*v stay bf16 (K=128 per head makes DoubleRow useless for
    sim, and an es hi/lo split would cost a second full ACT/DVE pass).
  - softmax denominators: fp16 DVE partial rowsums (2x DVE mode) +
    gpsimd 128-way partition reduce; normalize+fp8-split of the context
    runs on DVE with the hi-cast offloaded to gpsimd.

Per-core PE cycles: qproj 98304 + sim 131072 + attn*v 131072 +
outproj 98304 = 458752 (vs 524288 all-f32r).

Overlap notes (modeled 214us vs 259us f32r baseline):
  - The ACT exp stream (1038ns per [128,1024] tile) paces the attention
    inner loop, so q PSUM->SBUF copies run on DVE, not ACT.
  - qproj for pair hp+1 is drip-fed a few matmuls per jg into pair hp's
    attention stream instead of bursting: during a burst ACT starves
    (sg double-buffering banks only 2 jg of sim backlog) and loses the
    lead it needs to cover the per-jg exp deficit.
  - Wo is SBUF-resident; its chunks ride the sync DMA queue behind the
    wq head stream (same-queue order stops the scheduler from hoisting
    them into the startup-critical window - DMA bandwidth is one shared
    ~335GB/s pool, so front-running Wo starves the x/wq/kv stream).
  - Pair-0 qproj defers its batch-1 column halves into the batch-0
    attention stream so the first sim starts ~4us earlier.
  - Pair-7 has no qproj filler: the first two output-projection tiles
    are trickled into its PE slack (3 matmuls per jg, g7 terms queued
    last since they need pair-7's own context). PE gaps are doubly
    expensive under the cost model's p-state ramp.
  - The final tile's epilogue is split per column block across the two
    DGE queues to shorten the end drain.
"""

import sys
import numpy as np
import ml_dtypes

for _p in ("/opt/trn_rl_repo", "/root/.axon_site/_ro/trn_rl_repo"):
    if _p not in sys.path:
        sys.path.append(_p)

import concourse.bass as bass  # noqa: E402
import concourse.mybir as mybir  # noqa: E402
import concourse.tile as tile  # noqa: E402
from concourse import bacc, bass_isa  # noqa: E402
from concourse.bass_utils import run_bass_kernel_spmd  # noqa: E402

F32 = mybir.dt.float32
F16 = mybir.dt.float16
BF16 = mybir.dt.bfloat16
F8 = mybir.dt.float8e4
DR = mybir.MatmulPerfMode.DoubleRow
NE4 = ml_dtypes.float8_e4m3
NBF = ml_dtypes.bfloat16

B = 2
N = 2048          # query length (global)
J = 2048          # kv length
E = 2048          # model dim
HEADS = 16
DH = 128          # head dim
NCORES = 8
NC_ROWS = N // NCORES        # 256 query rows per core per batch
R = B * NC_ROWS              # 512 rows per core, col = b*NC_ROWS + i
JT = J // 128                # 16 j-tiles
G = E // 256                 # 8 DoubleRow k-tiles over a 2048 contraction
SCALE = float(DH) ** -0.5

# host-side power-of-2 prescales for the fp8 hi/lo splits
XS = 8.0          # x
WQS = 32.0        # Wq
OS = 64.0         # normalized context (outn)
WOS = 32.0        # Wo
QDESCALE = 1.0 / (XS * WQS)      # folded into the ACT q copy
ODESCALE = 1.0 / (OS * WOS)      # folded into the final output copy

_CACHE = {}


def _build(reps: int = 1):
    nc = bacc.Bacc(name=f"mqa_xattn_dr_r{reps}")
    # x hi/lo: [p, cc(b), g, s, r256] with e = 256g + 128s + p
    xh_d = nc.declare_dram_parameter("xh", [128, 2, G, 2, NC_ROWS], F8,
                                     isOutput=False)
    xl_d = nc.declare_dram_parameter("xl", [128, 2, G, 2, NC_ROWS], F8,
                                     isOutput=False)
    wqh_d = nc.declare_dram_parameter("wqh", [HEADS, 128, G, 2, 128], F8,
                                      isOutput=False)
    wql_d = nc.declare_dram_parameter("wql", [HEADS, 128, G, 2, 128], F8,
                                      isOutput=False)
    kt_d = nc.declare_dram_parameter("kt", [128, B, J], BF16, isOutput=False)
    vt_d = nc.declare_dram_parameter("vt", [128, B, JT, DH], BF16,
                                     isOutput=False)
    woh_d = nc.declare_dram_parameter("woh", [4, 128, G, 2, 512], F8,
                                      isOutput=False)
    wol_d = nc.declare_dram_parameter("wol", [4, 128, G, 2, 512], F8,
                                      isOutput=False)
    o_d = nc.declare_dram_parameter("o", [R, E], F32, isOutput=True)

    with tile.TileContext(nc) as tc:
        for _ in range(reps):
            _emit_once(nc, tc, xh_d, xl_d, wqh_d, wql_d, kt_d, vt_d,
                       woh_d, wol_d, o_d)

    nc.compile()
    return nc


def _emit_once(nc, tc, xh_d, xl_d, wqh_d, wql_d, kt_d, vt_d,
               woh_d, wol_d, o_d):
    with tc.tile_pool(name="persist", bufs=1) as pp:
        kt_sb = pp.tile([128, B, J], BF16)
        v_sb = pp.tile([128, B, JT, DH], BF16)
        qt_all = pp.tile([128, HEADS, R], BF16)
        # context, normalized and fp8 hi/lo split, laid out for DoubleRow
        # outproj: [p, b, g, s, i] with f = 256*g + 128*s + p, i in [0,256)
        on_hi = pp.tile([128, B, G, 2, NC_ROWS], F8)
        on_lo = pp.tile([128, B, G, 2, NC_ROWS], F8)
        # Wo is fully resident; its DMAs stream on the gpsimd queue during
        # phase B so phase C starts without an SBUF/DMA stall.
        woh_sb = pp.tile([128, 4, G, 2, 512], F8)
        wol_sb = pp.tile([128, 4, G, 2, 512], F8)

        # ---- Phase B: q-projection + attention, per head pair ----
        with tc.tile_pool(name="xt_pool", bufs=1) as xtp, \
             tc.tile_pool(name="wq_pool", bufs=3) as wqp, \
             tc.tile_pool(name="es_pool", bufs=6) as esp, \
             tc.tile_pool(name="rb_pool", bufs=2) as rbp, \
             tc.tile_pool(name="qp_ps", bufs=2, space="PSUM") as qp_ps, \
             tc.tile_pool(name="sg_ps", bufs=2, space="PSUM") as sg_ps, \
             tc.tile_pool(name="acc_ps", bufs=2, space="PSUM") as acc_ps:
            xh_sb = xtp.tile([128, 2, G, 2, NC_ROWS], F8)
            xl_sb = xtp.tile([128, 2, G, 2, NC_ROWS], F8)

            # Wo prefetch chunks, paced into the sync DMA queue behind the
            # wq head stream (the scheduler keeps same-queue order, so these
            # can't hoist ahead of the startup-critical transfers).
            wo_chunks = [(dst, src, ec, g0)
                         for ec in range(4)
                         for dst, src in ((woh_sb, woh_d), (wol_sb, wol_d))
                         for g0 in (0, G // 2)]

            def load_wq(h):
                wh = wqp.tile([128, G, 2, 128], F8, tag="wqh",
                              name=f"wqh_sb{h}")
                wl = wqp.tile([128, G, 2, 128], F8, tag="wql",
                              name=f"wql_sb{h}")
                nc.sync.dma_start(wh[:], wqh_d[h])
                nc.sync.dma_start(wl[:], wql_d[h])
                if h >= 2:
                    for _ in range(2):
                        if wo_chunks:
                            dst, src, ec, g0 = wo_chunks.pop(0)
                            nc.sync.dma_start(
                                dst[:, ec, g0:g0 + G // 2],
                                src[ec, :, g0:g0 + G // 2])
                return wh, wl

            # DMA order tuned so the first qproj group starts ~1us in and
            # batch-0 attention is never input-starved.
            wqh0 = wqp.tile([128, G, 2, 128], F8, tag="wqh", name="wqh_sb0")
            wql0 = wqp.tile([128, G, 2, 128], F8, tag="wql", name="wql_sb0")
            # x stream on the scalar-engine DGE queue, weights/kv on sync:
            # transfers share one bandwidth pool but per-DMA issue dead
            # time overlaps across queues
            nc.sync.dma_start(wqh0[:, 0:2], wqh_d[0, :, 0:2])
            nc.sync.dma_start(xh_sb[:, 0, 0:2], xh_d[:, 0, 0:2])
            nc.sync.dma_start(wqh0[:, 2:G], wqh_d[0, :, 2:G])
            nc.sync.dma_start(xh_sb[:, 0, 2:G], xh_d[:, 0, 2:G])
            nc.sync.dma_start(wql0[:], wql_d[0])
            nc.sync.dma_start(xl_sb[:, 0], xl_d[:, 0])
            wq_next = (wqh0, wql0)
            wq_next2 = load_wq(1)
            nc.sync.dma_start(kt_sb[:, 0, 0:1024], kt_d[:, 0, 0:1024])
            nc.sync.dma_start(v_sb[:, 0, 0:8], vt_d[:, 0, 0:8])
            nc.sync.dma_start(xh_sb[:, 1], xh_d[:, 1])
            nc.sync.dma_start(xl_sb[:, 1], xl_d[:, 1])
            nc.sync.dma_start(kt_sb[:, 0, 1024:J], kt_d[:, 0, 1024:J])
            nc.sync.dma_start(v_sb[:, 0, 8:JT], vt_d[:, 0, 8:JT])
            nc.sync.dma_start(kt_sb[:, 1, :], kt_d[:, 1, :])
            nc.sync.dma_start(v_sb[:, 1], vt_d[:, 1])

            def qproj_head_cc(h, wh, wl, q_ps, cc):
                # 3-term hi/lo: Wh@xh + Wl@xh + Wh@xl, one 256-col half
                terms = ((wh, xh_sb), (wl, xh_sb), (wh, xl_sb))
                n_mm = len(terms) * G
                i = 0
                for wt, xt in terms:
                    for g in range(G):
                        nc.tensor.matmul(
                            q_ps[:, cc * 256:(cc + 1) * 256],
                            wt[:, g],
                            xt[:, cc, g],
                            start=(i == 0), stop=(i == n_mm - 1),
                            perf_mode=DR)
                        i += 1

            pending_cc1 = []    # pair-0 cc1 work, interleaved into b0 attn

            # qproj for pair hp+1 is not emitted as a burst (ACT starves
            # during bursts: sg double-buffering banks only 2 jg of sim
            # backlog, so the exp stream idles and loses its lead). It is
            # drip-fed 3 matmuls at a time into pair hp's attention stream,
            # matching the per-jg ACT deficit.
            qtrickle = {"q": []}

            def build_qtrickle(hp1):
                nonlocal wq_next, wq_next2
                pw = []
                for hh in range(2):
                    h = 2 * hp1 + hh
                    pw.append(wq_next)
                    wq_next = wq_next2
                    if h + 2 < HEADS:
                        wq_next2 = load_wq(h + 2)
                tiles_ = [qp_ps.tile([128, R], F32, tag="qp",
                                     name=f"qpt{hp1}_{i}") for i in range(2)]
                q = []
                # cc0 groups (both heads) first: the next pair's batch-0
                # sim needs only the cc0 halves of qt
                for cc in range(2):
                    for hh in range(2):
                        h = 2 * hp1 + hh
                        wh, wl = pw[hh]
                        terms = ((wh, xh_sb), (wl, xh_sb), (wh, xl_sb))
                        n = 0
                        for wt, xt in terms:
                            for g in range(G):
                                q.append(("mm", tiles_[hh], wt, xt, cc, g,
                                          n == 0, n == 3 * G - 1))
                                n += 1
                        q.append(("copy", tiles_[hh], h, cc))
                qtrickle["q"] = q

            def qdrip(nmm):
                done = 0
                while qtrickle["q"] and done < nmm:
                    e = qtrickle["q"].pop(0)
                    if e[0] == "copy":
                        _, t, h, cc = e
                        with nc.allow_low_precision(reason="q -> bf16"):
                            nc.vector.tensor_scalar_mul(
                                qt_all[:, h, cc * 256:(cc + 1) * 256],
                                t[:, cc * 256:(cc + 1) * 256], QDESCALE)
                        continue
                    _, t, wt, xt, cc, g, st, sp = e
                    nc.tensor.matmul(t[:, cc * 256:(cc + 1) * 256],
                                     wt[:, g], xt[:, cc, g],
                                     start=st, stop=sp, perf_mode=DR)
                    done += 1

            def qproj_pair(hp, defer_cc1=False):
                nonlocal wq_next, wq_next2
                pair_w = []
                for hh in range(2):
                    h = 2 * hp + hh
                    pair_w.append(wq_next)
                    wq_next = wq_next2
                    if h + 2 < HEADS:
                        wq_next2 = load_wq(h + 2)
                for hh in range(2):
                    h = 2 * hp + hh
                    wh, wl = pair_w[hh]
                    q_ps = qp_ps.tile([128, R], F32, tag="qp")
                    qproj_head_cc(h, wh, wl, q_ps, 0)
                    # copies on DVE, not ACT: the exp stream paces the
                    # attention tail, so ACT gets nothing extra
                    if defer_cc1:
                        with nc.allow_low_precision(reason="q -> bf16"):
                            nc.vector.tensor_scalar_mul(
                                qt_all[:, h, 0:256], q_ps[:, 0:256],
                                QDESCALE)
                        pending_cc1.append((h, wh, wl, q_ps))
                    else:
                        qproj_head_cc(h, wh, wl, q_ps, 1)
                        with nc.allow_low_precision(reason="q -> bf16"):
                            nc.vector.tensor_scalar_mul(
                                qt_all[:, h, :], q_ps[:], QDESCALE)

            def emit_ctile(ec, b, rt, ps_pool, sb_pool, ps_tag="op",
                           last=False):
                """One output-projection tile [r128, e512] (48 DR matmuls).

                last=True pipelines the epilogue per 256-col half (and
                splits the final half's DMA) to shorten the end drain.
                """
                o_ps = ps_pool.tile([128, 512], F32, tag=ps_tag)
                # last tile: separate PSUM banks per eh half so the eh0
                # epilogue copy doesn't serialize against eh1's group start
                if last:
                    o_ps2 = ps_pool.tile([128, 512], F32, tag=ps_tag,
                                         name="ops_last2")
                    ps_eh = [o_ps, o_ps2]
                else:
                    ps_eh = [o_ps, o_ps]
                r0 = rt * 128

                def epilogue(c0, cw, eng=None, ceng=None):
                    src = ps_eh[c0 // 256 if c0 < 512 else 1]
                    o_sb = sb_pool.tile([128, cw], F32, tag=f"ost{cw}")
                    (ceng or nc.vector).tensor_scalar_mul(
                        o_sb[:], src[:, c0:c0 + cw], ODESCALE)
                    (eng or nc.sync).dma_start(
                        o_d[b * NC_ROWS + r0:b * NC_ROWS + r0 + 128,
                            ec * 512 + c0:ec * 512 + c0 + cw],
                        o_sb[:])

                for eh in range(2):
                    e0 = eh * 256
                    terms = ((on_hi, woh_sb), (on_lo, woh_sb),
                             (on_hi, wol_sb))
                    n_mm = len(terms) * G
                    i = 0
                    for on_t, wo_t in terms:
                        for g in range(G):
                            nc.tensor.matmul(
                                ps_eh[eh][:, e0:e0 + 256],
                                on_t[:, b, g, :, r0:r0 + 128],
                                wo_t[:, ec, g, :, e0:e0 + 256],
                                start=(i == 0), stop=(i == n_mm - 1),
                                perf_mode=DR)
                            i += 1
                    if last and eh == 0:
                        epilogue(0, 256)
                if last:
                    epilogue(256, 128, eng=nc.scalar)
                    epilogue(384, 128)
                else:
                    epilogue(0, 512)

            # Pair-7 units have no qproj filler and run at the ACT exp pace:
            # trickle the first output-projection tile (ec0, b0, rt0) into
            # their PE slack, 3-5 matmuls per jg, g7 terms after pair-7's
            # b0 context exists. Keeps PE continuously busy (the cost
            # model's p-state ramp doubles the price of any PE idle gap).
            CTERMS = lambda: ((on_hi, woh_sb), (on_lo, woh_sb),  # noqa: E731
                              (on_hi, wol_sb))
            trickle = {"q": [], "ops": None}

            CTRICKLE_TILES = [(0, 0, 0), (0, 0, 1)]

            def trickle_init():
                q = []
                trickle["tiles"] = []
                for ec, tb, rt in CTRICKLE_TILES:
                    ops = qp_ps.tile([128, 512], F32, tag="qp",
                                     name=f"ct_ops{ec}{tb}{rt}")
                    trickle["tiles"].append((ops, ec, tb, rt))
                    for eh in range(2):
                        idx = [(t, g) for g in range(G - 1)
                               for t in range(3)]
                        idx += [(t, G - 1) for t in range(3)]
                        for i, (t, g) in enumerate(idx):
                            q.append((ops, ec, tb, rt, eh, t, g,
                                      i == 0, i == 3 * G - 1))
                trickle["q"] = q

            def trickle_emit(n):
                for _ in range(n):
                    if not trickle["q"]:
                        return
                    ops, ec, tb, rt, eh, t, g, st, sp = \
                        trickle["q"].pop(0)
                    on_t, wo_t = CTERMS()[t]
                    nc.tensor.matmul(
                        ops[:, eh * 256:eh * 256 + 256],
                        on_t[:, tb, g, :, rt * 128:rt * 128 + 128],
                        wo_t[:, ec, g, :, eh * 256:eh * 256 + 256],
                        start=st, stop=sp, perf_mode=DR)

            # b0 may drain at most 21 entries (tile A's eh0 g0-6): anything
            # later in the queue reads pair-7's own context, written by the
            # b0 tail which is EMITTED after b0's jg slots - an earlier
            # read would see uninitialized SBUF with no semaphore guard.
            TRICKLE_SLOTS = {(0, jg): 3 for jg in range(1, 8)}
            TRICKLE_SLOTS.update({(1, jg): 5 for jg in range(8)})

            qproj_pair(0, defer_cc1=True)
            for hp in range(HEADS // 2):
                for b in range(B):
                    if b == 0 and hp + 1 < HEADS // 2:
                        build_qtrickle(hp + 1)
                    if hp == HEADS // 2 - 1 and b == 0:
                        trickle_init()
                    # Both heads of the pair processed together: every matmul
                    # has a 512-wide moving operand laid out as [h2, i256].
                    acc = acc_ps.tile([128, 512], F32, tag="acc")
                    qt_pair = qt_all[:, 2 * hp:2 * hp + 2,
                                     b * NC_ROWS:(b + 1) * NC_ROWS]
                    s1024 = rbp.tile([128, 1024], F16, tag="s128")
                    # during the final attention unit the qproj PSUM banks
                    # are idle and all batch-0 context is split: inject
                    # early output-projection tiles to fill the ACT-paced
                    # tail of phase B
                    inject = False and (hp == HEADS // 2 - 1 and b == 1)
                    for jg in range(JT // 2):
                        if inject and jg in (1, 3, 5, 7):
                            ti = (1, 3, 5, 7).index(jg)
                            emit_ctile(ti // 2, 0, ti % 2, qp_ps, rbp,
                                       ps_tag="qp")
                        if pending_cc1 and hp == 0 and b == 0 \
                                and jg in (1, 3):
                            h, wh, wl, q_ps = pending_cc1.pop(0)
                            qproj_head_cc(h, wh, wl, q_ps, 1)
                            with nc.allow_low_precision(reason="q -> bf16"):
                                nc.vector.tensor_scalar_mul(
                                    qt_all[:, h, 256:512], q_ps[:, 256:512],
                                    QDESCALE)
                        sg = sg_ps.tile([128, 1024], F32, tag="sg")
                        for kk in range(2):
                            jt = jg * 2 + kk
                            nc.tensor.matmul(
                                sg[:, kk * 512:(kk + 1) * 512],
                                kt_sb[:, b, jt * 128:(jt + 1) * 128],
                                qt_pair,
                                start=True, stop=True)
                        es = esp.tile([128, 1024], BF16, tag="es")
                        with nc.allow_low_precision(reason="es bf16"):
                            nc.scalar.activation(
                                es[:], sg[:],
                                mybir.ActivationFunctionType.Exp,
                                scale=SCALE)
                            # softmax denominators: fp16 partial rowsums on
                            # DVE (2x 16-bit mode); partition reduce below
                            if jg == 0:
                                nc.vector.tensor_copy(s1024[:], es[:])
                            else:
                                nc.vector.tensor_add(s1024[:], s1024[:],
                                                     es[:])
                        if hp == HEADS // 2 - 1 and (b, jg) in TRICKLE_SLOTS:
                            trickle_emit(TRICKLE_SLOTS[(b, jg)])
                        qdrip(2)
                        for kk in range(2):
                            jt = jg * 2 + kk
                            esk = es[:, kk * 512:(kk + 1) * 512]
                            nc.tensor.matmul(acc[:], v_sb[:, b, jt, :],
                                             esk, start=(jt == 0),
                                             stop=(jt == JT - 1))
                        qdrip(4)
                    if b == 1:
                        qdrip(10 ** 6)  # force-drain before the next pair
                    # softmax-denominator tail + context fp8 hi/lo split
                    s512 = rbp.tile([128, 512], F32, tag="s512", bufs=1)
                    sB = rbp.tile([128, 512], F32, tag="sB", bufs=1)
                    rb_sb = rbp.tile([128, 512], F32, tag="rbs")
                    t32 = rbp.tile([128, 512], F32, tag="t32")
                    hi_ap = on_hi[:, b, hp].rearrange("p a b -> p (a b)")
                    lo_ap = on_lo[:, b, hp].rearrange("p a b -> p (a b)")
                    with nc.allow_low_precision(reason="denominator tail"):
                        nc.vector.tensor_add(s512[:], s1024[:, 0:512],
                                             s1024[:, 512:1024])
                        nc.gpsimd.partition_all_reduce(
                            sB[:], s512[:], channels=128,
                            reduce_op=bass_isa.ReduceOp.add)
                        nc.vector.reciprocal(rb_sb[:], sB[:])
                        nc.vector.tensor_mul(t32[:], acc[:], rb_sb[:])
                        nc.gpsimd.tensor_scalar_mul(hi_ap, t32[:], OS)
                        nc.vector.scalar_tensor_tensor(
                            lo_ap, t32[:], OS, hi_ap,
                            mybir.AluOpType.mult,
                            mybir.AluOpType.subtract)
                    if hp == HEADS // 2 - 1 and b == 1:
                        trickle_emit(10 ** 6)  # drain leftover tile work
                        for ops, ec, tb, rt in trickle["tiles"]:
                            o_sb = rbp.tile([128, 512], F32, tag="ost512")
                            nc.vector.tensor_scalar_mul(o_sb[:], ops[:],
                                                        ODESCALE)
                            nc.sync.dma_start(
                                o_d[tb * NC_ROWS + rt * 128:
                                    tb * NC_ROWS + rt * 128 + 128,
                                    ec * 512:(ec + 1) * 512],
                                o_sb[:])

        # ---- Phase C: remaining output-projection tiles ----
        # (ec0/ec1, b0, *) were injected into the tail of phase B above.
        with tc.tile_pool(name="ost_pool", bufs=4) as ostp, \
             tc.tile_pool(name="op_ps", bufs=4, space="PSUM") as op_ps:
            tiles = [(ec, 0, rt) for ec in range(4) for rt in (0, 1)
                     if (ec, 0, rt) not in ((0, 0, 0), (0, 0, 1))]
            tiles += [(ec, 1, rt) for ec in range(4) for rt in (0, 1)]
            for ti, (ec, b, rt) in enumerate(tiles):
                emit_ctile(ec, b, rt, op_ps, ostp,
                           last=(ti == len(tiles) - 1))


def _get_nc(reps: int = 1):
    if reps not in _CACHE:
        _CACHE[reps] = _build(reps)
    return _CACHE[reps]


def _hilo(a, pre):
    s = (a * pre).astype(np.float32)
    hi = s.astype(NE4)
    lo = (s - hi.astype(np.float32)).astype(NE4)
    return hi, lo


def _make_in_maps(x, k, v, Wq, Wo):
    # Wq [E, inner] -> [h, p, g, s, f] with e = 256g + 128s + p
    wq_t = Wq.reshape(G, 2, 128, HEADS, 128).transpose(3, 2, 0, 1, 4)
    wqh, wql = _hilo(np.ascontiguousarray(wq_t), WQS)
    # Wo [inner, E] -> [ec, p, g, s, e'] with f = 256g + 128s + p
    wo_t = Wo.reshape(G, 2, 128, 4, 512).transpose(3, 2, 0, 1, 4)
    woh, wol = _hilo(np.ascontiguousarray(wo_t), WOS)
    # k [B, J, DH] -> kT [d, b, j]
    kt = np.ascontiguousarray(k.transpose(2, 0, 1)).astype(NBF)
    # v [B, J, DH] -> [p, b, jt, d]
    vt = np.ascontiguousarray(
        v.reshape(B, JT, 128, DH).transpose(2, 0, 1, 3)).astype(NBF)
    in_maps = []
    for c in range(NCORES):
        xs = x[:, c * NC_ROWS:(c + 1) * NC_ROWS, :]
        # [E, cc, r256] -> [p, cc, g, s, r]
        xt = np.stack([xs[0].T, xs[1].T], axis=1)
        xt = np.ascontiguousarray(
            xt.reshape(G, 2, 128, 2, NC_ROWS).transpose(2, 3, 0, 1, 4))
        xh, xl = _hilo(xt, XS)
        in_maps.append({"xh": xh, "xl": xl, "wqh": wqh, "wql": wql,
                        "kt": kt, "vt": vt, "woh": woh, "wol": wol})
    return in_maps


def run_on_device(x, k, v, Wq, Wo, reps: int = 1):
    nc = _get_nc(reps)
    in_maps = _make_in_maps(x, k, v, Wq, Wo)
    res = run_bass_kernel_spmd(nc, in_maps, list(range(NCORES)))
    parts = [res.results[c]["o"].reshape(B, NC_ROWS, E) for c in range(NCORES)]
    return np.concatenate(parts, axis=1)


def kernel(x, k, v, Wq, Wo):
    x = np.asarray(x, dtype=np.float32)
    k = np.asarray(k, dtype=np.float32)
    v = np.asarray(v, dtype=np.float32)
    Wq = np.asarray(Wq, dtype=np.float32)
    Wo = np.asarray(Wo, dtype=np.float32)
    return run_on_device(x, k, v, Wq, Wo, reps=1)


# revision 85
# speedup vs baseline: 1.0262x; 1.0002x over previous
"""Trainium2 Bass kernel for MQA cross-attention (nn_CrossAttention).

Reference computation (fp32):
    q = (x @ Wq).reshape(b, n, 16, 128).transpose(0,2,1,3) * 128**-0.5
    sim = q @ k^T   (k/v shared across heads, MQA)
    out = softmax(sim) @ v
    y = out.merge_heads @ Wo

Sharding: pure sequence-parallel across 8 cores. Each core gets 256 rows
of x per batch (512 rows total), full Wq/Wo/k/v, and produces its 512 rows
of the output. No collectives, no host-side reduction.

Mixed precision (validated vs reference, rel err ~4e-3):
  - qproj / outproj run as fp8e4 DoubleRow matmuls (0.5 cycles/row,
    256-deep contraction) with hi+lo splits of both operands, dropping
    only the lo*lo term. Splits are power-of-2 prescaled on the host so
    the lo residuals clear e4m3's subnormal floor; the prescales are
    folded into on-chip scalars (ACT copy scale, final output scale).
  - sim / attn*v stay bf16 (K=128 per head makes DoubleRow useless for
    sim, and an es hi/lo split would cost a second full ACT/DVE pass).
  - softmax denominators: fp16 DVE partial rowsums (2x DVE mode) +
    gpsimd 128-way partition reduce; normalize+fp8-split of the context
    runs on DVE with the hi-cast offloaded to gpsimd.

Per-core PE cycles: qproj 98304 + sim 131072 + attn*v 131072 +
outproj 98304 = 458752 (vs 524288 all-f32r).

Overlap notes (modeled 214us vs 259us f32r baseline):
  - The ACT exp stream (1038ns per [128,1024] tile) paces the attention
    inner loop, so q PSUM->SBUF copies run on DVE, not ACT.
  - qproj for pair hp+1 is drip-fed a few matmuls per jg into pair hp's
    attention stream instead of bursting: during a burst ACT starves
    (sg double-buffering banks only 2 jg of sim backlog) and loses the
    lead it needs to cover the per-jg exp deficit.
  - Wo is SBUF-resident; its chunks ride the sync DMA queue behind the
    wq head stream (same-queue order stops the scheduler from hoisting
    them into the startup-critical window - DMA bandwidth is one shared
    ~335GB/s pool, so front-running Wo starves the x/wq/kv stream).
  - Pair-0 qproj defers its batch-1 column halves into the batch-0
    attention stream so the first sim starts ~4us earlier.
  - Pair-7 has no qproj filler: the first two output-projection tiles
    are trickled into its PE slack (3 matmuls per jg, g7 terms queued
    last since they need pair-7's own context). PE gaps are doubly
    expensive under the cost model's p-state ramp.
  - The final tile's epilogue is split per column block across the two
    DGE queues to shorten the end drain.
"""

import sys
import numpy as np
import ml_dtypes

for _p in ("/opt/trn_rl_repo", "/root/.axon_site/_ro/trn_rl_repo"):
    if _p not in sys.path:
        sys.path.append(_p)

import concourse.bass as bass  # noqa: E402
import concourse.mybir as mybir  # noqa: E402
import concourse.tile as tile  # noqa: E402
from concourse import bacc, bass_isa  # noqa: E402
from concourse.bass_utils import run_bass_kernel_spmd  # noqa: E402

F32 = mybir.dt.float32
F16 = mybir.dt.float16
BF16 = mybir.dt.bfloat16
F8 = mybir.dt.float8e4
DR = mybir.MatmulPerfMode.DoubleRow
NE4 = ml_dtypes.float8_e4m3
NBF = ml_dtypes.bfloat16

B = 2
N = 2048          # query length (global)
J = 2048          # kv length
E = 2048          # model dim
HEADS = 16
DH = 128          # head dim
NCORES = 8
NC_ROWS = N // NCORES        # 256 query rows per core per batch
R = B * NC_ROWS              # 512 rows per core, col = b*NC_ROWS + i
JT = J // 128                # 16 j-tiles
G = E // 256                 # 8 DoubleRow k-tiles over a 2048 contraction
SCALE = float(DH) ** -0.5

# host-side power-of-2 prescales for the fp8 hi/lo splits
XS = 8.0          # x
WQS = 32.0        # Wq
OS = 64.0         # normalized context (outn)
WOS = 32.0        # Wo
QDESCALE = 1.0 / (XS * WQS)      # folded into the ACT q copy
ODESCALE = 1.0 / (OS * WOS)      # folded into the final output copy

_CACHE = {}


def _build(reps: int = 1):
    nc = bacc.Bacc(name=f"mqa_xattn_dr_r{reps}")
    # x hi/lo: [p, cc(b), g, s, r256] with e = 256g + 128s + p
    xh_d = nc.declare_dram_parameter("xh", [128, 2, G, 2, NC_ROWS], F8,
                                     isOutput=False)
    xl_d = nc.declare_dram_parameter("xl", [128, 2, G, 2, NC_ROWS], F8,
                                     isOutput=False)
    wqh_d = nc.declare_dram_parameter("wqh", [HEADS, 128, G, 2, 128], F8,
                                      isOutput=False)
    wql_d = nc.declare_dram_parameter("wql", [HEADS, 128, G, 2, 128], F8,
                                      isOutput=False)
    kt_d = nc.declare_dram_parameter("kt", [128, B, J], BF16, isOutput=False)
    vt_d = nc.declare_dram_parameter("vt", [128, B, JT, DH], BF16,
                                     isOutput=False)
    woh_d = nc.declare_dram_parameter("woh", [4, 128, G, 2, 512], F8,
                                      isOutput=False)
    wol_d = nc.declare_dram_parameter("wol", [4, 128, G, 2, 512], F8,
                                      isOutput=False)
    o_d = nc.declare_dram_parameter("o", [R, E], F32, isOutput=True)

    with tile.TileContext(nc) as tc:
        for _ in range(reps):
            _emit_once(nc, tc, xh_d, xl_d, wqh_d, wql_d, kt_d, vt_d,
                       woh_d, wol_d, o_d)

    nc.compile()
    return nc


def _emit_once(nc, tc, xh_d, xl_d, wqh_d, wql_d, kt_d, vt_d,
               woh_d, wol_d, o_d):
    with tc.tile_pool(name="persist", bufs=1) as pp:
        kt_sb = pp.tile([128, B, J], BF16)
        v_sb = pp.tile([128, B, JT, DH], BF16)
        qt_all = pp.tile([128, HEADS, R], BF16)
        # context, normalized and fp8 hi/lo split, laid out for DoubleRow
        # outproj: [p, b, g, s, i] with f = 256*g + 128*s + p, i in [0,256)
        on_hi = pp.tile([128, B, G, 2, NC_ROWS], F8)
        on_lo = pp.tile([128, B, G, 2, NC_ROWS], F8)
        # Wo is fully resident; its DMAs stream on the gpsimd queue during
        # phase B so phase C starts without an SBUF/DMA stall.
        woh_sb = pp.tile([128, 4, G, 2, 512], F8)
        wol_sb = pp.tile([128, 4, G, 2, 512], F8)

        # ---- Phase B: q-projection + attention, per head pair ----
        with tc.tile_pool(name="xt_pool", bufs=1) as xtp, \
             tc.tile_pool(name="wq_pool", bufs=3) as wqp, \
             tc.tile_pool(name="es_pool", bufs=6) as esp, \
             tc.tile_pool(name="rb_pool", bufs=2) as rbp, \
             tc.tile_pool(name="qp_ps", bufs=2, space="PSUM") as qp_ps, \
             tc.tile_pool(name="sg_ps", bufs=2, space="PSUM") as sg_ps, \
             tc.tile_pool(name="acc_ps", bufs=2, space="PSUM") as acc_ps:
            xh_sb = xtp.tile([128, 2, G, 2, NC_ROWS], F8)
            xl_sb = xtp.tile([128, 2, G, 2, NC_ROWS], F8)

            # Wo prefetch chunks, paced into the sync DMA queue behind the
            # wq head stream (the scheduler keeps same-queue order, so these
            # can't hoist ahead of the startup-critical transfers).
            wo_chunks = [(dst, src, ec, g0)
                         for ec in range(4)
                         for dst, src in ((woh_sb, woh_d), (wol_sb, wol_d))
                         for g0 in (0, G // 2)]

            def load_wq(h):
                wh = wqp.tile([128, G, 2, 128], F8, tag="wqh",
                              name=f"wqh_sb{h}")
                wl = wqp.tile([128, G, 2, 128], F8, tag="wql",
                              name=f"wql_sb{h}")
                nc.sync.dma_start(wh[:], wqh_d[h])
                nc.sync.dma_start(wl[:], wql_d[h])
                if h >= 2:
                    for _ in range(2):
                        if wo_chunks:
                            dst, src, ec, g0 = wo_chunks.pop(0)
                            nc.sync.dma_start(
                                dst[:, ec, g0:g0 + G // 2],
                                src[ec, :, g0:g0 + G // 2])
                return wh, wl

            # DMA order tuned so the first qproj group starts ~1us in and
            # batch-0 attention is never input-starved.
            wqh0 = wqp.tile([128, G, 2, 128], F8, tag="wqh", name="wqh_sb0")
            wql0 = wqp.tile([128, G, 2, 128], F8, tag="wql", name="wql_sb0")
            # x stream on the scalar-engine DGE queue, weights/kv on sync:
            # transfers share one bandwidth pool but per-DMA issue dead
            # time overlaps across queues
            nc.sync.dma_start(wqh0[:, 0:2], wqh_d[0, :, 0:2])
            nc.sync.dma_start(xh_sb[:, 0, 0:2], xh_d[:, 0, 0:2])
            nc.sync.dma_start(wqh0[:, 2:G], wqh_d[0, :, 2:G])
            nc.sync.dma_start(xh_sb[:, 0, 2:G], xh_d[:, 0, 2:G])
            nc.sync.dma_start(wql0[:, 0:4], wql_d[0, :, 0:4])
            nc.sync.dma_start(xl_sb[:, 0, 0:4], xl_d[:, 0, 0:4])
            nc.sync.dma_start(wql0[:, 4:G], wql_d[0, :, 4:G])
            nc.sync.dma_start(xl_sb[:, 0, 4:G], xl_d[:, 0, 4:G])
            wq_next = (wqh0, wql0)
            wq_next2 = load_wq(1)
            nc.sync.dma_start(kt_sb[:, 0, 0:1024], kt_d[:, 0, 0:1024])
            nc.sync.dma_start(v_sb[:, 0, 0:8], vt_d[:, 0, 0:8])
            nc.sync.dma_start(xh_sb[:, 1], xh_d[:, 1])
            nc.sync.dma_start(xl_sb[:, 1], xl_d[:, 1])
            nc.sync.dma_start(kt_sb[:, 0, 1024:J], kt_d[:, 0, 1024:J])
            nc.sync.dma_start(v_sb[:, 0, 8:JT], vt_d[:, 0, 8:JT])
            nc.sync.dma_start(kt_sb[:, 1, :], kt_d[:, 1, :])
            nc.sync.dma_start(v_sb[:, 1], vt_d[:, 1])

            def qproj_head_cc(h, wh, wl, q_ps, cc):
                # 3-term hi/lo: Wh@xh + Wl@xh + Wh@xl, one 256-col half
                terms = ((wh, xh_sb), (wl, xh_sb), (wh, xl_sb))
                n_mm = len(terms) * G
                i = 0
                for wt, xt in terms:
                    for g in range(G):
                        nc.tensor.matmul(
                            q_ps[:, cc * 256:(cc + 1) * 256],
                            wt[:, g],
                            xt[:, cc, g],
                            start=(i == 0), stop=(i == n_mm - 1),
                            perf_mode=DR)
                        i += 1

            pending_cc1 = []    # pair-0 cc1 work, interleaved into b0 attn

            # qproj for pair hp+1 is not emitted as a burst (ACT starves
            # during bursts: sg double-buffering banks only 2 jg of sim
            # backlog, so the exp stream idles and loses its lead). It is
            # drip-fed 3 matmuls at a time into pair hp's attention stream,
            # matching the per-jg ACT deficit.
            qtrickle = {"q": []}

            def build_qtrickle(hp1):
                nonlocal wq_next, wq_next2
                pw = []
                for hh in range(2):
                    h = 2 * hp1 + hh
                    pw.append(wq_next)
                    wq_next = wq_next2
                    if h + 2 < HEADS:
                        wq_next2 = load_wq(h + 2)
                tiles_ = [qp_ps.tile([128, R], F32, tag="qp",
                                     name=f"qpt{hp1}_{i}") for i in range(2)]
                q = []
                # cc0 groups (both heads) first: the next pair's batch-0
                # sim needs only the cc0 halves of qt
                for cc in range(2):
                    for hh in range(2):
                        h = 2 * hp1 + hh
                        wh, wl = pw[hh]
                        terms = ((wh, xh_sb), (wl, xh_sb), (wh, xl_sb))
                        n = 0
                        for wt, xt in terms:
                            for g in range(G):
                                q.append(("mm", tiles_[hh], wt, xt, cc, g,
                                          n == 0, n == 3 * G - 1))
                                n += 1
                        q.append(("copy", tiles_[hh], h, cc))
                qtrickle["q"] = q

            def qdrip(nmm):
                done = 0
                while qtrickle["q"] and done < nmm:
                    e = qtrickle["q"].pop(0)
                    if e[0] == "copy":
                        _, t, h, cc = e
                        with nc.allow_low_precision(reason="q -> bf16"):
                            nc.vector.tensor_scalar_mul(
                                qt_all[:, h, cc * 256:(cc + 1) * 256],
                                t[:, cc * 256:(cc + 1) * 256], QDESCALE)
                        continue
                    _, t, wt, xt, cc, g, st, sp = e
                    nc.tensor.matmul(t[:, cc * 256:(cc + 1) * 256],
                                     wt[:, g], xt[:, cc, g],
                                     start=st, stop=sp, perf_mode=DR)
                    done += 1

            def qproj_pair(hp, defer_cc1=False):
                nonlocal wq_next, wq_next2
                pair_w = []
                for hh in range(2):
                    h = 2 * hp + hh
                    pair_w.append(wq_next)
                    wq_next = wq_next2
                    if h + 2 < HEADS:
                        wq_next2 = load_wq(h + 2)
                for hh in range(2):
                    h = 2 * hp + hh
                    wh, wl = pair_w[hh]
                    q_ps = qp_ps.tile([128, R], F32, tag="qp")
                    qproj_head_cc(h, wh, wl, q_ps, 0)
                    # copies on DVE, not ACT: the exp stream paces the
                    # attention tail, so ACT gets nothing extra
                    if defer_cc1:
                        with nc.allow_low_precision(reason="q -> bf16"):
                            nc.vector.tensor_scalar_mul(
                                qt_all[:, h, 0:256], q_ps[:, 0:256],
                                QDESCALE)
                        pending_cc1.append((h, wh, wl, q_ps))
                    else:
                        qproj_head_cc(h, wh, wl, q_ps, 1)
                        with nc.allow_low_precision(reason="q -> bf16"):
                            nc.vector.tensor_scalar_mul(
                                qt_all[:, h, :], q_ps[:], QDESCALE)

            def emit_ctile(ec, b, rt, ps_pool, sb_pool, ps_tag="op",
                           last=False):
                """One output-projection tile [r128, e512] (48 DR matmuls).

                last=True pipelines the epilogue per 256-col half (and
                splits the final half's DMA) to shorten the end drain.
                """
                o_ps = ps_pool.tile([128, 512], F32, tag=ps_tag)
                # last tile: separate PSUM banks per eh half so the eh0
                # epilogue copy doesn't serialize against eh1's group start
                if last:
                    o_ps2 = ps_pool.tile([128, 512], F32, tag=ps_tag,
                                         name="ops_last2")
                    ps_eh = [o_ps, o_ps2]
                else:
                    ps_eh = [o_ps, o_ps]
                r0 = rt * 128

                def epilogue(c0, cw, eng=None, ceng=None):
                    src = ps_eh[c0 // 256 if c0 < 512 else 1]
                    o_sb = sb_pool.tile([128, cw], F32, tag=f"ost{cw}")
                    (ceng or nc.vector).tensor_scalar_mul(
                        o_sb[:], src[:, c0:c0 + cw], ODESCALE)
                    (eng or nc.sync).dma_start(
                        o_d[b * NC_ROWS + r0:b * NC_ROWS + r0 + 128,
                            ec * 512 + c0:ec * 512 + c0 + cw],
                        o_sb[:])

                for eh in range(2):
                    e0 = eh * 256
                    terms = ((on_hi, woh_sb), (on_lo, woh_sb),
                             (on_hi, wol_sb))
                    n_mm = len(terms) * G
                    i = 0
                    for on_t, wo_t in terms:
                        for g in range(G):
                            nc.tensor.matmul(
                                ps_eh[eh][:, e0:e0 + 256],
                                on_t[:, b, g, :, r0:r0 + 128],
                                wo_t[:, ec, g, :, e0:e0 + 256],
                                start=(i == 0), stop=(i == n_mm - 1),
                                perf_mode=DR)
                            i += 1
                    if last and eh == 0:
                        epilogue(0, 256)
                if last:
                    epilogue(256, 128, eng=nc.scalar)
                    epilogue(384, 128)
                else:
                    epilogue(0, 512)

            # Pair-7 units have no qproj filler and run at the ACT exp pace:
            # trickle the first output-projection tile (ec0, b0, rt0) into
            # their PE slack, 3-5 matmuls per jg, g7 terms after pair-7's
            # b0 context exists. Keeps PE continuously busy (the cost
            # model's p-state ramp doubles the price of any PE idle gap).
            CTERMS = lambda: ((on_hi, woh_sb), (on_lo, woh_sb),  # noqa: E731
                              (on_hi, wol_sb))
            trickle = {"q": [], "ops": None}

            CTRICKLE_TILES = [(0, 0, 0), (0, 0, 1)]

            def trickle_init():
                q = []
                trickle["tiles"] = []
                for ec, tb, rt in CTRICKLE_TILES:
                    ops = qp_ps.tile([128, 512], F32, tag="qp",
                                     name=f"ct_ops{ec}{tb}{rt}")
                    trickle["tiles"].append((ops, ec, tb, rt))
                    for eh in range(2):
                        idx = [(t, g) for g in range(G - 1)
                               for t in range(3)]
                        idx += [(t, G - 1) for t in range(3)]
                        for i, (t, g) in enumerate(idx):
                            q.append((ops, ec, tb, rt, eh, t, g,
                                      i == 0, i == 3 * G - 1))
                trickle["q"] = q

            def trickle_emit(n):
                for _ in range(n):
                    if not trickle["q"]:
                        return
                    ops, ec, tb, rt, eh, t, g, st, sp = \
                        trickle["q"].pop(0)
                    on_t, wo_t = CTERMS()[t]
                    nc.tensor.matmul(
                        ops[:, eh * 256:eh * 256 + 256],
                        on_t[:, tb, g, :, rt * 128:rt * 128 + 128],
                        wo_t[:, ec, g, :, eh * 256:eh * 256 + 256],
                        start=st, stop=sp, perf_mode=DR)

            # b0 may drain at most 21 entries (tile A's eh0 g0-6): anything
            # later in the queue reads pair-7's own context, written by the
            # b0 tail which is EMITTED after b0's jg slots - an earlier
            # read would see uninitialized SBUF with no semaphore guard.
            TRICKLE_SLOTS = {(0, jg): 3 for jg in range(1, 8)}
            TRICKLE_SLOTS.update({(1, jg): 5 for jg in range(8)})

            qproj_pair(0, defer_cc1=True)
            for hp in range(HEADS // 2):
                for b in range(B):
                    if b == 0 and hp + 1 < HEADS // 2:
                        build_qtrickle(hp + 1)
                    if hp == HEADS // 2 - 1 and b == 0:
                        trickle_init()
                    # Both heads of the pair processed together: every matmul
                    # has a 512-wide moving operand laid out as [h2, i256].
                    acc = acc_ps.tile([128, 512], F32, tag="acc")
                    qt_pair = qt_all[:, 2 * hp:2 * hp + 2,
                                     b * NC_ROWS:(b + 1) * NC_ROWS]
                    s1024 = rbp.tile([128, 1024], F16, tag="s128")
                    # during the final attention unit the qproj PSUM banks
                    # are idle and all batch-0 context is split: inject
                    # early output-projection tiles to fill the ACT-paced
                    # tail of phase B
                    inject = False and (hp == HEADS // 2 - 1 and b == 1)
                    for jg in range(JT // 2):
                        if inject and jg in (1, 3, 5, 7):
                            ti = (1, 3, 5, 7).index(jg)
                            emit_ctile(ti // 2, 0, ti % 2, qp_ps, rbp,
                                       ps_tag="qp")
                        if pending_cc1 and hp == 0 and b == 0 \
                                and jg in (1, 3):
                            h, wh, wl, q_ps = pending_cc1.pop(0)
                            qproj_head_cc(h, wh, wl, q_ps, 1)
                            with nc.allow_low_precision(reason="q -> bf16"):
                                nc.vector.tensor_scalar_mul(
                                    qt_all[:, h, 256:512], q_ps[:, 256:512],
                                    QDESCALE)
                        sg = sg_ps.tile([128, 1024], F32, tag="sg")
                        for kk in range(2):
                            jt = jg * 2 + kk
                            nc.tensor.matmul(
                                sg[:, kk * 512:(kk + 1) * 512],
                                kt_sb[:, b, jt * 128:(jt + 1) * 128],
                                qt_pair,
                                start=True, stop=True)
                        es = esp.tile([128, 1024], BF16, tag="es")
                        with nc.allow_low_precision(reason="es bf16"):
                            nc.scalar.activation(
                                es[:], sg[:],
                                mybir.ActivationFunctionType.Exp,
                                scale=SCALE)
                            # softmax denominators: fp16 partial rowsums on
                            # DVE (2x 16-bit mode); partition reduce below
                            if jg == 0:
                                nc.vector.tensor_copy(s1024[:], es[:])
                            else:
                                nc.vector.tensor_add(s1024[:], s1024[:],
                                                     es[:])
                        if hp == HEADS // 2 - 1 and (b, jg) in TRICKLE_SLOTS:
                            trickle_emit(TRICKLE_SLOTS[(b, jg)])
                        qdrip(2)
                        for kk in range(2):
                            jt = jg * 2 + kk
                            esk = es[:, kk * 512:(kk + 1) * 512]
                            nc.tensor.matmul(acc[:], v_sb[:, b, jt, :],
                                             esk, start=(jt == 0),
                                             stop=(jt == JT - 1))
                        qdrip(4)
                    if b == 1:
                        qdrip(10 ** 6)  # force-drain before the next pair
                    # softmax-denominator tail + context fp8 hi/lo split
                    s512 = rbp.tile([128, 512], F32, tag="s512", bufs=1)
                    sB = rbp.tile([128, 512], F32, tag="sB", bufs=1)
                    rb_sb = rbp.tile([128, 512], F32, tag="rbs")
                    t32 = rbp.tile([128, 512], F32, tag="t32")
                    hi_ap = on_hi[:, b, hp].rearrange("p a b -> p (a b)")
                    lo_ap = on_lo[:, b, hp].rearrange("p a b -> p (a b)")
                    with nc.allow_low_precision(reason="denominator tail"):
                        nc.vector.tensor_add(s512[:], s1024[:, 0:512],
                                             s1024[:, 512:1024])
                        nc.gpsimd.partition_all_reduce(
                            sB[:], s512[:], channels=128,
                            reduce_op=bass_isa.ReduceOp.add)
                        nc.vector.reciprocal(rb_sb[:], sB[:])
                        nc.vector.tensor_mul(t32[:], acc[:], rb_sb[:])
                        nc.gpsimd.tensor_scalar_mul(hi_ap, t32[:], OS)
                        nc.vector.scalar_tensor_tensor(
                            lo_ap, t32[:], OS, hi_ap,
                            mybir.AluOpType.mult,
                            mybir.AluOpType.subtract)
                    if hp == HEADS // 2 - 1 and b == 1:
                        trickle_emit(10 ** 6)  # drain leftover tile work
                        for ops, ec, tb, rt in trickle["tiles"]:
                            o_sb = rbp.tile([128, 512], F32, tag="ost512")
                            nc.vector.tensor_scalar_mul(o_sb[:], ops[:],
                                                        ODESCALE)
                            nc.sync.dma_start(
                                o_d[tb * NC_ROWS + rt * 128:
                                    tb * NC_ROWS + rt * 128 + 128,
                                    ec * 512:(ec + 1) * 512],
                                o_sb[:])

        # ---- Phase C: remaining output-projection tiles ----
        # (ec0/ec1, b0, *) were injected into the tail of phase B above.
        with tc.tile_pool(name="ost_pool", bufs=4) as ostp, \
             tc.tile_pool(name="op_ps", bufs=4, space="PSUM") as op_ps:
            tiles = [(ec, 0, rt) for ec in range(4) for rt in (0, 1)
                     if (ec, 0, rt) not in ((0, 0, 0), (0, 0, 1))]
            tiles += [(ec, 1, rt) for ec in range(4) for rt in (0, 1)]
            for ti, (ec, b, rt) in enumerate(tiles):
                emit_ctile(ec, b, rt, op_ps, ostp,
                           last=(ti == len(tiles) - 1))


def _get_nc(reps: int = 1):
    if reps not in _CACHE:
        _CACHE[reps] = _build(reps)
    return _CACHE[reps]


def _hilo(a, pre):
    s = (a * pre).astype(np.float32)
    hi = s.astype(NE4)
    lo = (s - hi.astype(np.float32)).astype(NE4)
    return hi, lo


def _make_in_maps(x, k, v, Wq, Wo):
    # Wq [E, inner] -> [h, p, g, s, f] with e = 256g + 128s + p
    wq_t = Wq.reshape(G, 2, 128, HEADS, 128).transpose(3, 2, 0, 1, 4)
    wqh, wql = _hilo(np.ascontiguousarray(wq_t), WQS)
    # Wo [inner, E] -> [ec, p, g, s, e'] with f = 256g + 128s + p
    wo_t = Wo.reshape(G, 2, 128, 4, 512).transpose(3, 2, 0, 1, 4)
    woh, wol = _hilo(np.ascontiguousarray(wo_t), WOS)
    # k [B, J, DH] -> kT [d, b, j]
    kt = np.ascontiguousarray(k.transpose(2, 0, 1)).astype(NBF)
    # v [B, J, DH] -> [p, b, jt, d]
    vt = np.ascontiguousarray(
        v.reshape(B, JT, 128, DH).transpose(2, 0, 1, 3)).astype(NBF)
    in_maps = []
    for c in range(NCORES):
        xs = x[:, c * NC_ROWS:(c + 1) * NC_ROWS, :]
        # [E, cc, r256] -> [p, cc, g, s, r]
        xt = np.stack([xs[0].T, xs[1].T], axis=1)
        xt = np.ascontiguousarray(
            xt.reshape(G, 2, 128, 2, NC_ROWS).transpose(2, 3, 0, 1, 4))
        xh, xl = _hilo(xt, XS)
        in_maps.append({"xh": xh, "xl": xl, "wqh": wqh, "wql": wql,
                        "kt": kt, "vt": vt, "woh": woh, "wol": wol})
    return in_maps


def run_on_device(x, k, v, Wq, Wo, reps: int = 1):
    nc = _get_nc(reps)
    in_maps = _make_in_maps(x, k, v, Wq, Wo)
    res = run_bass_kernel_spmd(nc, in_maps, list(range(NCORES)))
    parts = [res.results[c]["o"].reshape(B, NC_ROWS, E) for c in range(NCORES)]
    return np.concatenate(parts, axis=1)


def kernel(x, k, v, Wq, Wo):
    x = np.asarray(x, dtype=np.float32)
    k = np.asarray(k, dtype=np.float32)
    v = np.asarray(v, dtype=np.float32)
    Wq = np.asarray(Wq, dtype=np.float32)
    Wo = np.asarray(Wo, dtype=np.float32)
    return run_on_device(x, k, v, Wq, Wo, reps=1)


# revision 88
# speedup vs baseline: 1.0280x; 1.0017x over previous
"""Trainium2 Bass kernel for MQA cross-attention (nn_CrossAttention).

Reference computation (fp32):
    q = (x @ Wq).reshape(b, n, 16, 128).transpose(0,2,1,3) * 128**-0.5
    sim = q @ k^T   (k/v shared across heads, MQA)
    out = softmax(sim) @ v
    y = out.merge_heads @ Wo

Sharding: pure sequence-parallel across 8 cores. Each core gets 256 rows
of x per batch (512 rows total), full Wq/Wo/k/v, and produces its 512 rows
of the output. No collectives, no host-side reduction.

Mixed precision (validated vs reference, rel err ~4e-3):
  - qproj / outproj run as fp8e4 DoubleRow matmuls (0.5 cycles/row,
    256-deep contraction) with hi+lo splits of both operands, dropping
    only the lo*lo term. Splits are power-of-2 prescaled on the host so
    the lo residuals clear e4m3's subnormal floor; the prescales are
    folded into on-chip scalars (ACT copy scale, final output scale).
  - sim / attn*v stay bf16 (K=128 per head makes DoubleRow useless for
    sim, and an es hi/lo split would cost a second full ACT/DVE pass).
  - softmax denominators: fp16 DVE partial rowsums (2x DVE mode) +
    gpsimd 128-way partition reduce; normalize+fp8-split of the context
    runs on DVE with the hi-cast offloaded to gpsimd.

Per-core PE cycles: qproj 98304 + sim 131072 + attn*v 131072 +
outproj 98304 = 458752 (vs 524288 all-f32r).

Overlap notes (modeled 214us vs 259us f32r baseline):
  - The ACT exp stream (1038ns per [128,1024] tile) paces the attention
    inner loop, so q PSUM->SBUF copies run on DVE, not ACT.
  - qproj for pair hp+1 is drip-fed a few matmuls per jg into pair hp's
    attention stream instead of bursting: during a burst ACT starves
    (sg double-buffering banks only 2 jg of sim backlog) and loses the
    lead it needs to cover the per-jg exp deficit.
  - Wo is SBUF-resident; its chunks ride the sync DMA queue behind the
    wq head stream (same-queue order stops the scheduler from hoisting
    them into the startup-critical window - DMA bandwidth is one shared
    ~335GB/s pool, so front-running Wo starves the x/wq/kv stream).
  - Pair-0 qproj defers its batch-1 column halves into the batch-0
    attention stream so the first sim starts ~4us earlier.
  - Pair-7 has no qproj filler: the first two output-projection tiles
    are trickled into its PE slack (3 matmuls per jg, g7 terms queued
    last since they need pair-7's own context). PE gaps are doubly
    expensive under the cost model's p-state ramp.
  - The final tile's epilogue is split per column block across the two
    DGE queues to shorten the end drain.
"""

import sys
import numpy as np
import ml_dtypes

for _p in ("/opt/trn_rl_repo", "/root/.axon_site/_ro/trn_rl_repo"):
    if _p not in sys.path:
        sys.path.append(_p)

import concourse.bass as bass  # noqa: E402
import concourse.mybir as mybir  # noqa: E402
import concourse.tile as tile  # noqa: E402
from concourse import bacc, bass_isa  # noqa: E402
from concourse.bass_utils import run_bass_kernel_spmd  # noqa: E402

F32 = mybir.dt.float32
F16 = mybir.dt.float16
BF16 = mybir.dt.bfloat16
F8 = mybir.dt.float8e4
DR = mybir.MatmulPerfMode.DoubleRow
NE4 = ml_dtypes.float8_e4m3
NBF = ml_dtypes.bfloat16

B = 2
N = 2048          # query length (global)
J = 2048          # kv length
E = 2048          # model dim
HEADS = 16
DH = 128          # head dim
NCORES = 8
NC_ROWS = N // NCORES        # 256 query rows per core per batch
R = B * NC_ROWS              # 512 rows per core, col = b*NC_ROWS + i
JT = J // 128                # 16 j-tiles
G = E // 256                 # 8 DoubleRow k-tiles over a 2048 contraction
SCALE = float(DH) ** -0.5

# host-side power-of-2 prescales for the fp8 hi/lo splits
XS = 8.0          # x
WQS = 32.0        # Wq
OS = 64.0         # normalized context (outn)
WOS = 32.0        # Wo
QDESCALE = 1.0 / (XS * WQS)      # folded into the ACT q copy
ODESCALE = 1.0 / (OS * WOS)      # folded into the final output copy

_CACHE = {}


def _build(reps: int = 1):
    nc = bacc.Bacc(name=f"mqa_xattn_dr_r{reps}")
    # x hi/lo: [p, cc(b), g, s, r256] with e = 256g + 128s + p
    xh_d = nc.declare_dram_parameter("xh", [128, 2, G, 2, NC_ROWS], F8,
                                     isOutput=False)
    xl_d = nc.declare_dram_parameter("xl", [128, 2, G, 2, NC_ROWS], F8,
                                     isOutput=False)
    wqh_d = nc.declare_dram_parameter("wqh", [HEADS, 128, G, 2, 128], F8,
                                      isOutput=False)
    wql_d = nc.declare_dram_parameter("wql", [HEADS, 128, G, 2, 128], F8,
                                      isOutput=False)
    kt_d = nc.declare_dram_parameter("kt", [128, B, J], BF16, isOutput=False)
    vt_d = nc.declare_dram_parameter("vt", [128, B, JT, DH], BF16,
                                     isOutput=False)
    woh_d = nc.declare_dram_parameter("woh", [4, 128, G, 2, 512], F8,
                                      isOutput=False)
    wol_d = nc.declare_dram_parameter("wol", [4, 128, G, 2, 512], F8,
                                      isOutput=False)
    o_d = nc.declare_dram_parameter("o", [R, E], F32, isOutput=True)

    with tile.TileContext(nc) as tc:
        for _ in range(reps):
            _emit_once(nc, tc, xh_d, xl_d, wqh_d, wql_d, kt_d, vt_d,
                       woh_d, wol_d, o_d)

    nc.compile()
    return nc


def _emit_once(nc, tc, xh_d, xl_d, wqh_d, wql_d, kt_d, vt_d,
               woh_d, wol_d, o_d):
    with tc.tile_pool(name="persist", bufs=1) as pp:
        kt_sb = pp.tile([128, B, J], BF16)
        v_sb = pp.tile([128, B, JT, DH], BF16)
        qt_all = pp.tile([128, HEADS, R], BF16)
        # context, normalized and fp8 hi/lo split, laid out for DoubleRow
        # outproj: [p, b, g, s, i] with f = 256*g + 128*s + p, i in [0,256)
        on_hi = pp.tile([128, B, G, 2, NC_ROWS], F8)
        on_lo = pp.tile([128, B, G, 2, NC_ROWS], F8)
        # Wo is fully resident; its DMAs stream on the gpsimd queue during
        # phase B so phase C starts without an SBUF/DMA stall.
        woh_sb = pp.tile([128, 4, G, 2, 512], F8)
        wol_sb = pp.tile([128, 4, G, 2, 512], F8)

        # ---- Phase B: q-projection + attention, per head pair ----
        with tc.tile_pool(name="xt_pool", bufs=1) as xtp, \
             tc.tile_pool(name="wq_pool", bufs=3) as wqp, \
             tc.tile_pool(name="es_pool", bufs=6) as esp, \
             tc.tile_pool(name="rb_pool", bufs=2) as rbp, \
             tc.tile_pool(name="qp_ps", bufs=2, space="PSUM") as qp_ps, \
             tc.tile_pool(name="sg_ps", bufs=2, space="PSUM") as sg_ps, \
             tc.tile_pool(name="acc_ps", bufs=2, space="PSUM") as acc_ps:
            xh_sb = xtp.tile([128, 2, G, 2, NC_ROWS], F8)
            xl_sb = xtp.tile([128, 2, G, 2, NC_ROWS], F8)

            # Wo prefetch chunks, paced into the sync DMA queue behind the
            # wq head stream (the scheduler keeps same-queue order, so these
            # can't hoist ahead of the startup-critical transfers).
            wo_chunks = [(dst, src, ec, g0)
                         for ec in range(4)
                         for dst, src in ((woh_sb, woh_d), (wol_sb, wol_d))
                         for g0 in (0, G // 2)]

            def load_wq(h):
                wh = wqp.tile([128, G, 2, 128], F8, tag="wqh",
                              name=f"wqh_sb{h}")
                wl = wqp.tile([128, G, 2, 128], F8, tag="wql",
                              name=f"wql_sb{h}")
                nc.sync.dma_start(wh[:], wqh_d[h])
                nc.sync.dma_start(wl[:], wql_d[h])
                if h >= 2:
                    for _ in range(2):
                        if wo_chunks:
                            dst, src, ec, g0 = wo_chunks.pop(0)
                            nc.sync.dma_start(
                                dst[:, ec, g0:g0 + G // 2],
                                src[ec, :, g0:g0 + G // 2])
                return wh, wl

            # DMA order tuned so the first qproj group starts ~1us in and
            # batch-0 attention is never input-starved.
            wqh0 = wqp.tile([128, G, 2, 128], F8, tag="wqh", name="wqh_sb0")
            wql0 = wqp.tile([128, G, 2, 128], F8, tag="wql", name="wql_sb0")
            # x stream on the scalar-engine DGE queue, weights/kv on sync:
            # transfers share one bandwidth pool but per-DMA issue dead
            # time overlaps across queues
            nc.sync.dma_start(wqh0[:, 0:2], wqh_d[0, :, 0:2])
            nc.sync.dma_start(xh_sb[:, 0, 0:2], xh_d[:, 0, 0:2])
            nc.sync.dma_start(wqh0[:, 2:G], wqh_d[0, :, 2:G])
            nc.sync.dma_start(xh_sb[:, 0, 2:G], xh_d[:, 0, 2:G])
            nc.sync.dma_start(wql0[:, 0:4], wql_d[0, :, 0:4])
            nc.sync.dma_start(xl_sb[:, 0, 0:4], xl_d[:, 0, 0:4])
            nc.sync.dma_start(wql0[:, 4:G], wql_d[0, :, 4:G])
            nc.sync.dma_start(xl_sb[:, 0, 4:G], xl_d[:, 0, 4:G])
            wq_next = (wqh0, wql0)
            wq_next2 = load_wq(1)
            nc.sync.dma_start(kt_sb[:, 0, 0:1024], kt_d[:, 0, 0:1024])
            nc.sync.dma_start(v_sb[:, 0, 0:8], vt_d[:, 0, 0:8])
            nc.sync.dma_start(xh_sb[:, 1], xh_d[:, 1])
            nc.sync.dma_start(xl_sb[:, 1], xl_d[:, 1])
            nc.sync.dma_start(kt_sb[:, 0, 1024:J], kt_d[:, 0, 1024:J])
            nc.sync.dma_start(v_sb[:, 0, 8:JT], vt_d[:, 0, 8:JT])
            nc.sync.dma_start(kt_sb[:, 1, :], kt_d[:, 1, :])
            nc.sync.dma_start(v_sb[:, 1], vt_d[:, 1])

            def qproj_head_cc(h, wh, wl, q_ps, cc):
                # 3-term hi/lo: Wh@xh + Wl@xh + Wh@xl, one 256-col half
                terms = ((wh, xh_sb), (wl, xh_sb), (wh, xl_sb))
                n_mm = len(terms) * G
                i = 0
                for wt, xt in terms:
                    for g in range(G):
                        nc.tensor.matmul(
                            q_ps[:, cc * 256:(cc + 1) * 256],
                            wt[:, g],
                            xt[:, cc, g],
                            start=(i == 0), stop=(i == n_mm - 1),
                            perf_mode=DR)
                        i += 1

            pending_cc1 = []    # pair-0 cc1 work, interleaved into b0 attn

            # qproj for pair hp+1 is not emitted as a burst (ACT starves
            # during bursts: sg double-buffering banks only 2 jg of sim
            # backlog, so the exp stream idles and loses its lead). It is
            # drip-fed 3 matmuls at a time into pair hp's attention stream,
            # matching the per-jg ACT deficit.
            qtrickle = {"q": []}

            def build_qtrickle(hp1):
                nonlocal wq_next, wq_next2
                pw = []
                for hh in range(2):
                    h = 2 * hp1 + hh
                    pw.append(wq_next)
                    wq_next = wq_next2
                    if h + 2 < HEADS:
                        wq_next2 = load_wq(h + 2)
                tiles_ = [qp_ps.tile([128, R], F32, tag="qp",
                                     name=f"qpt{hp1}_{i}") for i in range(2)]
                q = []
                # cc0 groups (both heads) first: the next pair's batch-0
                # sim needs only the cc0 halves of qt
                for cc in range(2):
                    for hh in range(2):
                        h = 2 * hp1 + hh
                        wh, wl = pw[hh]
                        terms = ((wh, xh_sb), (wl, xh_sb), (wh, xl_sb))
                        n = 0
                        for wt, xt in terms:
                            for g in range(G):
                                q.append(("mm", tiles_[hh], wt, xt, cc, g,
                                          n == 0, n == 3 * G - 1))
                                n += 1
                        q.append(("copy", tiles_[hh], h, cc))
                qtrickle["q"] = q

            def qdrip(nmm):
                done = 0
                while qtrickle["q"] and done < nmm:
                    e = qtrickle["q"].pop(0)
                    if e[0] == "copy":
                        _, t, h, cc = e
                        with nc.allow_low_precision(reason="q -> bf16"):
                            nc.vector.tensor_scalar_mul(
                                qt_all[:, h, cc * 256:(cc + 1) * 256],
                                t[:, cc * 256:(cc + 1) * 256], QDESCALE)
                        continue
                    _, t, wt, xt, cc, g, st, sp = e
                    nc.tensor.matmul(t[:, cc * 256:(cc + 1) * 256],
                                     wt[:, g], xt[:, cc, g],
                                     start=st, stop=sp, perf_mode=DR)
                    done += 1

            def qproj_pair(hp, defer_cc1=False):
                nonlocal wq_next, wq_next2
                pair_w = []
                for hh in range(2):
                    h = 2 * hp + hh
                    pair_w.append(wq_next)
                    wq_next = wq_next2
                    if h + 2 < HEADS:
                        wq_next2 = load_wq(h + 2)
                for hh in range(2):
                    h = 2 * hp + hh
                    wh, wl = pair_w[hh]
                    q_ps = qp_ps.tile([128, R], F32, tag="qp")
                    qproj_head_cc(h, wh, wl, q_ps, 0)
                    # copies on DVE, not ACT: the exp stream paces the
                    # attention tail, so ACT gets nothing extra
                    if defer_cc1:
                        with nc.allow_low_precision(reason="q -> bf16"):
                            nc.vector.tensor_scalar_mul(
                                qt_all[:, h, 0:256], q_ps[:, 0:256],
                                QDESCALE)
                        pending_cc1.append((h, wh, wl, q_ps))
                    else:
                        qproj_head_cc(h, wh, wl, q_ps, 1)
                        with nc.allow_low_precision(reason="q -> bf16"):
                            nc.vector.tensor_scalar_mul(
                                qt_all[:, h, :], q_ps[:], QDESCALE)

            def emit_ctile(ec, b, rt, ps_pool, sb_pool, ps_tag="op",
                           last=False):
                """One output-projection tile [r128, e512] (48 DR matmuls).

                last=True pipelines the epilogue per 256-col half (and
                splits the final half's DMA) to shorten the end drain.
                """
                o_ps = ps_pool.tile([128, 512], F32, tag=ps_tag)
                r0 = rt * 128
                terms = ((on_hi, woh_sb), (on_lo, woh_sb),
                         (on_hi, wol_sb))

                def group(dst, e0, ew):
                    n_mm = len(terms) * G
                    i = 0
                    for on_t, wo_t in terms:
                        for g in range(G):
                            nc.tensor.matmul(
                                dst[:, e0:e0 + ew],
                                on_t[:, b, g, :, r0:r0 + 128],
                                wo_t[:, ec, g, :, e0:e0 + ew],
                                start=(i == 0), stop=(i == n_mm - 1),
                                perf_mode=DR)
                            i += 1

                def epilogue(src, c0, cw, eng=None):
                    o_sb = sb_pool.tile([128, cw], F32, tag=f"ost{cw}")
                    nc.vector.tensor_scalar_mul(
                        o_sb[:], src[:, c0:c0 + cw], ODESCALE)
                    (eng or nc.sync).dma_start(
                        o_d[b * NC_ROWS + r0:b * NC_ROWS + r0 + 128,
                            ec * 512 + c0:ec * 512 + c0 + cw],
                        o_sb[:])

                if last:
                    # staircase drain: three column groups on separate
                    # PSUM banks, each epilogue overlapping the next
                    # group's matmuls
                    o_ps2 = ps_pool.tile([128, 512], F32, tag=ps_tag,
                                         name="ops_last2")
                    o_ps3 = ps_pool.tile([128, 512], F32, tag=ps_tag,
                                         name="ops_last3")
                    group(o_ps, 0, 256)
                    epilogue(o_ps, 0, 256, eng=nc.scalar)
                    group(o_ps2, 256, 128)
                    epilogue(o_ps2, 256, 128)
                    group(o_ps3, 384, 128)
                    epilogue(o_ps3, 384, 128, eng=nc.scalar)
                else:
                    group(o_ps, 0, 256)
                    group(o_ps, 256, 256)
                    epilogue(o_ps, 0, 512)

            # Pair-7 units have no qproj filler and run at the ACT exp pace:
            # trickle the first output-projection tile (ec0, b0, rt0) into
            # their PE slack, 3-5 matmuls per jg, g7 terms after pair-7's
            # b0 context exists. Keeps PE continuously busy (the cost
            # model's p-state ramp doubles the price of any PE idle gap).
            CTERMS = lambda: ((on_hi, woh_sb), (on_lo, woh_sb),  # noqa: E731
                              (on_hi, wol_sb))
            trickle = {"q": [], "ops": None}

            CTRICKLE_TILES = [(0, 0, 0), (0, 0, 1)]

            def trickle_init():
                trickle["tiles"] = []
                parts = []     # per tile: (eh0_main, eh0_g7, eh1_all)
                for ec, tb, rt in CTRICKLE_TILES:
                    ops = qp_ps.tile([128, 512], F32, tag="qp",
                                     name=f"ct_ops{ec}{tb}{rt}")
                    trickle["tiles"].append((ops, ec, tb, rt))
                    seg = []
                    for eh in range(2):
                        idx = [(t, g) for g in range(G - 1)
                               for t in range(3)]
                        idx += [(t, G - 1) for t in range(3)]
                        ent = [(ops, ec, tb, rt, eh, t, g,
                                i == 0, i == 3 * G - 1)
                               for i, (t, g) in enumerate(idx)]
                        seg.append(ent)
                    parts.append((seg[0][:21], seg[0][21:], seg[1]))
                # batch-0 slots drain only the first 21 entries (tile A's
                # g0-6): everything later reads pair-7's own context and
                # must be emitted after the b0 tail (batch-1 slots onward)
                (a0m, a0l, a1), (b0m, b0l, b1) = parts
                trickle["q"] = a0m + a0l + a1 + b0m + b0l + b1

            def trickle_emit(n):
                for _ in range(n):
                    if not trickle["q"]:
                        return
                    ops, ec, tb, rt, eh, t, g, st, sp = \
                        trickle["q"].pop(0)
                    on_t, wo_t = CTERMS()[t]
                    nc.tensor.matmul(
                        ops[:, eh * 256:eh * 256 + 256],
                        on_t[:, tb, g, :, rt * 128:rt * 128 + 128],
                        wo_t[:, ec, g, :, eh * 256:eh * 256 + 256],
                        start=st, stop=sp, perf_mode=DR)

            # b0 may drain at most 21 entries (tile A's eh0 g0-6): anything
            # later in the queue reads pair-7's own context, written by the
            # b0 tail which is EMITTED after b0's jg slots - an earlier
            # read would see uninitialized SBUF with no semaphore guard.
            TRICKLE_SLOTS = {(0, jg): 3 for jg in range(1, 8)}
            TRICKLE_SLOTS.update({(1, jg): 5 for jg in range(8)})

            qproj_pair(0, defer_cc1=True)
            for hp in range(HEADS // 2):
                for b in range(B):
                    if b == 0 and hp + 1 < HEADS // 2:
                        build_qtrickle(hp + 1)
                    if hp == HEADS // 2 - 1 and b == 0:
                        trickle_init()
                    # Both heads of the pair processed together: every matmul
                    # has a 512-wide moving operand laid out as [h2, i256].
                    acc = acc_ps.tile([128, 512], F32, tag="acc")
                    qt_pair = qt_all[:, 2 * hp:2 * hp + 2,
                                     b * NC_ROWS:(b + 1) * NC_ROWS]
                    s1024 = rbp.tile([128, 1024], F16, tag="s128")
                    # during the final attention unit the qproj PSUM banks
                    # are idle and all batch-0 context is split: inject
                    # early output-projection tiles to fill the ACT-paced
                    # tail of phase B
                    inject = False and (hp == HEADS // 2 - 1 and b == 1)
                    for jg in range(JT // 2):
                        if inject and jg in (1, 3, 5, 7):
                            ti = (1, 3, 5, 7).index(jg)
                            emit_ctile(ti // 2, 0, ti % 2, qp_ps, rbp,
                                       ps_tag="qp")
                        if pending_cc1 and hp == 0 and b == 0 \
                                and jg in (1, 3):
                            h, wh, wl, q_ps = pending_cc1.pop(0)
                            qproj_head_cc(h, wh, wl, q_ps, 1)
                            with nc.allow_low_precision(reason="q -> bf16"):
                                nc.vector.tensor_scalar_mul(
                                    qt_all[:, h, 256:512], q_ps[:, 256:512],
                                    QDESCALE)
                        sg = sg_ps.tile([128, 1024], F32, tag="sg")
                        for kk in range(2):
                            jt = jg * 2 + kk
                            nc.tensor.matmul(
                                sg[:, kk * 512:(kk + 1) * 512],
                                kt_sb[:, b, jt * 128:(jt + 1) * 128],
                                qt_pair,
                                start=True, stop=True)
                        es = esp.tile([128, 1024], BF16, tag="es")
                        with nc.allow_low_precision(reason="es bf16"):
                            nc.scalar.activation(
                                es[:], sg[:],
                                mybir.ActivationFunctionType.Exp,
                                scale=SCALE)
                            # softmax denominators: fp16 partial rowsums on
                            # DVE (2x 16-bit mode); partition reduce below
                            if jg == 0:
                                nc.vector.tensor_copy(s1024[:], es[:])
                            else:
                                nc.vector.tensor_add(s1024[:], s1024[:],
                                                     es[:])
                        if hp == HEADS // 2 - 1 and (b, jg) in TRICKLE_SLOTS:
                            trickle_emit(TRICKLE_SLOTS[(b, jg)])
                        qdrip(2)
                        for kk in range(2):
                            jt = jg * 2 + kk
                            esk = es[:, kk * 512:(kk + 1) * 512]
                            nc.tensor.matmul(acc[:], v_sb[:, b, jt, :],
                                             esk, start=(jt == 0),
                                             stop=(jt == JT - 1))
                        qdrip(4)
                    if b == 1:
                        qdrip(10 ** 6)  # force-drain before the next pair
                    # softmax-denominator tail + context fp8 hi/lo split
                    s512 = rbp.tile([128, 512], F32, tag="s512", bufs=1)
                    sB = rbp.tile([128, 512], F32, tag="sB", bufs=1)
                    rb_sb = rbp.tile([128, 512], F32, tag="rbs")
                    t32 = rbp.tile([128, 512], F32, tag="t32")
                    hi_ap = on_hi[:, b, hp].rearrange("p a b -> p (a b)")
                    lo_ap = on_lo[:, b, hp].rearrange("p a b -> p (a b)")
                    with nc.allow_low_precision(reason="denominator tail"):
                        nc.vector.tensor_add(s512[:], s1024[:, 0:512],
                                             s1024[:, 512:1024])
                        nc.gpsimd.partition_all_reduce(
                            sB[:], s512[:], channels=128,
                            reduce_op=bass_isa.ReduceOp.add)
                        nc.vector.reciprocal(rb_sb[:], sB[:])
                        nc.vector.tensor_mul(t32[:], acc[:], rb_sb[:])
                        nc.gpsimd.tensor_scalar_mul(hi_ap, t32[:], OS)
                        nc.vector.scalar_tensor_tensor(
                            lo_ap, t32[:], OS, hi_ap,
                            mybir.AluOpType.mult,
                            mybir.AluOpType.subtract)
                    if hp == HEADS // 2 - 1 and b == 1:
                        trickle_emit(10 ** 6)  # drain leftover tile work
                        for ops, ec, tb, rt in trickle["tiles"]:
                            o_sb = rbp.tile([128, 512], F32, tag="ost512")
                            nc.vector.tensor_scalar_mul(o_sb[:], ops[:],
                                                        ODESCALE)
                            nc.sync.dma_start(
                                o_d[tb * NC_ROWS + rt * 128:
                                    tb * NC_ROWS + rt * 128 + 128,
                                    ec * 512:(ec + 1) * 512],
                                o_sb[:])

        # ---- Phase C: remaining output-projection tiles ----
        # (ec0/ec1, b0, *) were injected into the tail of phase B above.
        with tc.tile_pool(name="ost_pool", bufs=4) as ostp, \
             tc.tile_pool(name="op_ps", bufs=4, space="PSUM") as op_ps:
            tiles = [(ec, 0, rt) for ec in range(4) for rt in (0, 1)
                     if (ec, 0, rt) not in ((0, 0, 0), (0, 0, 1))]
            tiles += [(ec, 1, rt) for ec in range(4) for rt in (0, 1)]
            for ti, (ec, b, rt) in enumerate(tiles):
                emit_ctile(ec, b, rt, op_ps, ostp,
                           last=(ti == len(tiles) - 1))


def _get_nc(reps: int = 1):
    if reps not in _CACHE:
        _CACHE[reps] = _build(reps)
    return _CACHE[reps]


def _hilo(a, pre):
    s = (a * pre).astype(np.float32)
    hi = s.astype(NE4)
    lo = (s - hi.astype(np.float32)).astype(NE4)
    return hi, lo


def _make_in_maps(x, k, v, Wq, Wo):
    # Wq [E, inner] -> [h, p, g, s, f] with e = 256g + 128s + p
    wq_t = Wq.reshape(G, 2, 128, HEADS, 128).transpose(3, 2, 0, 1, 4)
    wqh, wql = _hilo(np.ascontiguousarray(wq_t), WQS)
    # Wo [inner, E] -> [ec, p, g, s, e'] with f = 256g + 128s + p
    wo_t = Wo.reshape(G, 2, 128, 4, 512).transpose(3, 2, 0, 1, 4)
    woh, wol = _hilo(np.ascontiguousarray(wo_t), WOS)
    # k [B, J, DH] -> kT [d, b, j]
    kt = np.ascontiguousarray(k.transpose(2, 0, 1)).astype(NBF)
    # v [B, J, DH] -> [p, b, jt, d]
    vt = np.ascontiguousarray(
        v.reshape(B, JT, 128, DH).transpose(2, 0, 1, 3)).astype(NBF)
    in_maps = []
    for c in range(NCORES):
        xs = x[:, c * NC_ROWS:(c + 1) * NC_ROWS, :]
        # [E, cc, r256] -> [p, cc, g, s, r]
        xt = np.stack([xs[0].T, xs[1].T], axis=1)
        xt = np.ascontiguousarray(
            xt.reshape(G, 2, 128, 2, NC_ROWS).transpose(2, 3, 0, 1, 4))
        xh, xl = _hilo(xt, XS)
        in_maps.append({"xh": xh, "xl": xl, "wqh": wqh, "wql": wql,
                        "kt": kt, "vt": vt, "woh": woh, "wol": wol})
    return in_maps


def run_on_device(x, k, v, Wq, Wo, reps: int = 1):
    nc = _get_nc(reps)
    in_maps = _make_in_maps(x, k, v, Wq, Wo)
    res = run_bass_kernel_spmd(nc, in_maps, list(range(NCORES)))
    parts = [res.results[c]["o"].reshape(B, NC_ROWS, E) for c in range(NCORES)]
    return np.concatenate(parts, axis=1)


def kernel(x, k, v, Wq, Wo):
    x = np.asarray(x, dtype=np.float32)
    k = np.asarray(k, dtype=np.float32)
    v = np.asarray(v, dtype=np.float32)
    Wq = np.asarray(Wq, dtype=np.float32)
    Wo = np.asarray(Wo, dtype=np.float32)
    return run_on_device(x, k, v, Wq, Wo, reps=1)


# revision 92
# speedup vs baseline: 1.0283x; 1.0003x over previous
"""Trainium2 Bass kernel for MQA cross-attention (nn_CrossAttention).

Reference computation (fp32):
    q = (x @ Wq).reshape(b, n, 16, 128).transpose(0,2,1,3) * 128**-0.5
    sim = q @ k^T   (k/v shared across heads, MQA)
    out = softmax(sim) @ v
    y = out.merge_heads @ Wo

Sharding: pure sequence-parallel across 8 cores. Each core gets 256 rows
of x per batch (512 rows total), full Wq/Wo/k/v, and produces its 512 rows
of the output. No collectives, no host-side reduction.

Mixed precision (validated vs reference, rel err ~4e-3):
  - qproj / outproj run as fp8e4 DoubleRow matmuls (0.5 cycles/row,
    256-deep contraction) with hi+lo splits of both operands, dropping
    only the lo*lo term. Splits are power-of-2 prescaled on the host so
    the lo residuals clear e4m3's subnormal floor; the prescales are
    folded into on-chip scalars (ACT copy scale, final output scale).
  - sim / attn*v stay bf16 (K=128 per head makes DoubleRow useless for
    sim, and an es hi/lo split would cost a second full ACT/DVE pass).
  - softmax denominators: fp16 DVE partial rowsums (2x DVE mode) +
    gpsimd 128-way partition reduce; normalize+fp8-split of the context
    runs on DVE with the hi-cast offloaded to gpsimd.

Per-core PE cycles: qproj 98304 + sim 131072 + attn*v 131072 +
outproj 98304 = 458752 (vs 524288 all-f32r).

Overlap notes (modeled 214us vs 259us f32r baseline):
  - The ACT exp stream (1038ns per [128,1024] tile) paces the attention
    inner loop, so q PSUM->SBUF copies run on DVE, not ACT.
  - qproj for pair hp+1 is drip-fed a few matmuls per jg into pair hp's
    attention stream instead of bursting: during a burst ACT starves
    (sg double-buffering banks only 2 jg of sim backlog) and loses the
    lead it needs to cover the per-jg exp deficit.
  - Wo is SBUF-resident; its chunks ride the sync DMA queue behind the
    wq head stream (same-queue order stops the scheduler from hoisting
    them into the startup-critical window - DMA bandwidth is one shared
    ~335GB/s pool, so front-running Wo starves the x/wq/kv stream).
  - Pair-0 qproj defers its batch-1 column halves into the batch-0
    attention stream so the first sim starts ~4us earlier.
  - Pair-7 has no qproj filler: the first two output-projection tiles
    are trickled into its PE slack (3 matmuls per jg, g7 terms queued
    last since they need pair-7's own context). PE gaps are doubly
    expensive under the cost model's p-state ramp.
  - The final tile's epilogue is split per column block across the two
    DGE queues to shorten the end drain.
"""

import sys
import numpy as np
import ml_dtypes

for _p in ("/opt/trn_rl_repo", "/root/.axon_site/_ro/trn_rl_repo"):
    if _p not in sys.path:
        sys.path.append(_p)

import concourse.bass as bass  # noqa: E402
import concourse.mybir as mybir  # noqa: E402
import concourse.tile as tile  # noqa: E402
from concourse import bacc, bass_isa  # noqa: E402
from concourse.bass_utils import run_bass_kernel_spmd  # noqa: E402

F32 = mybir.dt.float32
F16 = mybir.dt.float16
BF16 = mybir.dt.bfloat16
F8 = mybir.dt.float8e4
DR = mybir.MatmulPerfMode.DoubleRow
NE4 = ml_dtypes.float8_e4m3
NBF = ml_dtypes.bfloat16

B = 2
N = 2048          # query length (global)
J = 2048          # kv length
E = 2048          # model dim
HEADS = 16
DH = 128          # head dim
NCORES = 8
NC_ROWS = N // NCORES        # 256 query rows per core per batch
R = B * NC_ROWS              # 512 rows per core, col = b*NC_ROWS + i
JT = J // 128                # 16 j-tiles
G = E // 256                 # 8 DoubleRow k-tiles over a 2048 contraction
SCALE = float(DH) ** -0.5

# host-side power-of-2 prescales for the fp8 hi/lo splits
XS = 8.0          # x
WQS = 32.0        # Wq
OS = 64.0         # normalized context (outn)
WOS = 32.0        # Wo
QDESCALE = 1.0 / (XS * WQS)      # folded into the ACT q copy
ODESCALE = 1.0 / (OS * WOS)      # folded into the final output copy

_CACHE = {}


def _build(reps: int = 1):
    nc = bacc.Bacc(name=f"mqa_xattn_dr_r{reps}")
    # x hi/lo: [p, cc(b), g, s, r256] with e = 256g + 128s + p
    xh_d = nc.declare_dram_parameter("xh", [128, 2, G, 2, NC_ROWS], F8,
                                     isOutput=False)
    xl_d = nc.declare_dram_parameter("xl", [128, 2, G, 2, NC_ROWS], F8,
                                     isOutput=False)
    wqh_d = nc.declare_dram_parameter("wqh", [HEADS, 128, G, 2, 128], F8,
                                      isOutput=False)
    wql_d = nc.declare_dram_parameter("wql", [HEADS, 128, G, 2, 128], F8,
                                      isOutput=False)
    kt_d = nc.declare_dram_parameter("kt", [128, B, J], BF16, isOutput=False)
    vt_d = nc.declare_dram_parameter("vt", [128, B, JT, DH], BF16,
                                     isOutput=False)
    woh_d = nc.declare_dram_parameter("woh", [4, 128, G, 2, 512], F8,
                                      isOutput=False)
    wol_d = nc.declare_dram_parameter("wol", [4, 128, G, 2, 512], F8,
                                      isOutput=False)
    o_d = nc.declare_dram_parameter("o", [R, E], F32, isOutput=True)

    with tile.TileContext(nc) as tc:
        for _ in range(reps):
            _emit_once(nc, tc, xh_d, xl_d, wqh_d, wql_d, kt_d, vt_d,
                       woh_d, wol_d, o_d)

    nc.compile()
    return nc


def _emit_once(nc, tc, xh_d, xl_d, wqh_d, wql_d, kt_d, vt_d,
               woh_d, wol_d, o_d):
    with tc.tile_pool(name="persist", bufs=1) as pp:
        kt_sb = pp.tile([128, B, J], BF16)
        v_sb = pp.tile([128, B, JT, DH], BF16)
        qt_all = pp.tile([128, HEADS, R], BF16)
        # context, normalized and fp8 hi/lo split, laid out for DoubleRow
        # outproj: [p, b, g, s, i] with f = 256*g + 128*s + p, i in [0,256)
        on_hi = pp.tile([128, B, G, 2, NC_ROWS], F8)
        on_lo = pp.tile([128, B, G, 2, NC_ROWS], F8)
        # Wo is fully resident; its DMAs stream on the gpsimd queue during
        # phase B so phase C starts without an SBUF/DMA stall.
        woh_sb = pp.tile([128, 4, G, 2, 512], F8)
        wol_sb = pp.tile([128, 4, G, 2, 512], F8)

        # ---- Phase B: q-projection + attention, per head pair ----
        with tc.tile_pool(name="xt_pool", bufs=1) as xtp, \
             tc.tile_pool(name="wq_pool", bufs=3) as wqp, \
             tc.tile_pool(name="es_pool", bufs=8) as esp, \
             tc.tile_pool(name="rb_pool", bufs=2) as rbp, \
             tc.tile_pool(name="qp_ps", bufs=2, space="PSUM") as qp_ps, \
             tc.tile_pool(name="sg_ps", bufs=2, space="PSUM") as sg_ps, \
             tc.tile_pool(name="acc_ps", bufs=2, space="PSUM") as acc_ps:
            xh_sb = xtp.tile([128, 2, G, 2, NC_ROWS], F8)
            xl_sb = xtp.tile([128, 2, G, 2, NC_ROWS], F8)

            # Wo prefetch chunks, paced into the sync DMA queue behind the
            # wq head stream (the scheduler keeps same-queue order, so these
            # can't hoist ahead of the startup-critical transfers).
            wo_chunks = [(dst, src, ec, g0)
                         for ec in range(4)
                         for dst, src in ((woh_sb, woh_d), (wol_sb, wol_d))
                         for g0 in (0, G // 2)]

            def load_wq(h):
                wh = wqp.tile([128, G, 2, 128], F8, tag="wqh",
                              name=f"wqh_sb{h}")
                wl = wqp.tile([128, G, 2, 128], F8, tag="wql",
                              name=f"wql_sb{h}")
                nc.sync.dma_start(wh[:], wqh_d[h])
                nc.sync.dma_start(wl[:], wql_d[h])
                if h >= 2:
                    for _ in range(2):
                        if wo_chunks:
                            dst, src, ec, g0 = wo_chunks.pop(0)
                            nc.sync.dma_start(
                                dst[:, ec, g0:g0 + G // 2],
                                src[ec, :, g0:g0 + G // 2])
                return wh, wl

            # DMA order tuned so the first qproj group starts ~1us in and
            # batch-0 attention is never input-starved.
            wqh0 = wqp.tile([128, G, 2, 128], F8, tag="wqh", name="wqh_sb0")
            wql0 = wqp.tile([128, G, 2, 128], F8, tag="wql", name="wql_sb0")
            # x stream on the scalar-engine DGE queue, weights/kv on sync:
            # transfers share one bandwidth pool but per-DMA issue dead
            # time overlaps across queues
            nc.sync.dma_start(wqh0[:, 0:2], wqh_d[0, :, 0:2])
            nc.sync.dma_start(xh_sb[:, 0, 0:2], xh_d[:, 0, 0:2])
            nc.sync.dma_start(wqh0[:, 2:G], wqh_d[0, :, 2:G])
            nc.sync.dma_start(xh_sb[:, 0, 2:G], xh_d[:, 0, 2:G])
            nc.sync.dma_start(wql0[:, 0:4], wql_d[0, :, 0:4])
            nc.sync.dma_start(xl_sb[:, 0, 0:4], xl_d[:, 0, 0:4])
            nc.sync.dma_start(wql0[:, 4:G], wql_d[0, :, 4:G])
            nc.sync.dma_start(xl_sb[:, 0, 4:G], xl_d[:, 0, 4:G])
            wq_next = (wqh0, wql0)
            wq_next2 = load_wq(1)
            nc.sync.dma_start(kt_sb[:, 0, 0:1024], kt_d[:, 0, 0:1024])
            nc.sync.dma_start(v_sb[:, 0, 0:8], vt_d[:, 0, 0:8])
            nc.sync.dma_start(xh_sb[:, 1], xh_d[:, 1])
            nc.sync.dma_start(xl_sb[:, 1], xl_d[:, 1])
            nc.sync.dma_start(kt_sb[:, 0, 1024:J], kt_d[:, 0, 1024:J])
            nc.sync.dma_start(v_sb[:, 0, 8:JT], vt_d[:, 0, 8:JT])
            nc.sync.dma_start(kt_sb[:, 1, :], kt_d[:, 1, :])
            nc.sync.dma_start(v_sb[:, 1], vt_d[:, 1])

            def qproj_head_cc(h, wh, wl, q_ps, cc):
                # 3-term hi/lo: Wh@xh + Wl@xh + Wh@xl, one 256-col half
                terms = ((wh, xh_sb), (wl, xh_sb), (wh, xl_sb))
                n_mm = len(terms) * G
                i = 0
                for wt, xt in terms:
                    for g in range(G):
                        nc.tensor.matmul(
                            q_ps[:, cc * 256:(cc + 1) * 256],
                            wt[:, g],
                            xt[:, cc, g],
                            start=(i == 0), stop=(i == n_mm - 1),
                            perf_mode=DR)
                        i += 1

            pending_cc1 = []    # pair-0 cc1 work, interleaved into b0 attn

            # qproj for pair hp+1 is not emitted as a burst (ACT starves
            # during bursts: sg double-buffering banks only 2 jg of sim
            # backlog, so the exp stream idles and loses its lead). It is
            # drip-fed 3 matmuls at a time into pair hp's attention stream,
            # matching the per-jg ACT deficit.
            qtrickle = {"q": []}

            def build_qtrickle(hp1):
                nonlocal wq_next, wq_next2
                pw = []
                for hh in range(2):
                    h = 2 * hp1 + hh
                    pw.append(wq_next)
                    wq_next = wq_next2
                    if h + 2 < HEADS:
                        wq_next2 = load_wq(h + 2)
                tiles_ = [qp_ps.tile([128, R], F32, tag="qp",
                                     name=f"qpt{hp1}_{i}") for i in range(2)]
                q = []
                # cc0 groups (both heads) first: the next pair's batch-0
                # sim needs only the cc0 halves of qt
                for cc in range(2):
                    for hh in range(2):
                        h = 2 * hp1 + hh
                        wh, wl = pw[hh]
                        terms = ((wh, xh_sb), (wl, xh_sb), (wh, xl_sb))
                        n = 0
                        for wt, xt in terms:
                            for g in range(G):
                                q.append(("mm", tiles_[hh], wt, xt, cc, g,
                                          n == 0, n == 3 * G - 1))
                                n += 1
                        q.append(("copy", tiles_[hh], h, cc))
                qtrickle["q"] = q

            def qdrip(nmm):
                done = 0
                while qtrickle["q"] and done < nmm:
                    e = qtrickle["q"].pop(0)
                    if e[0] == "copy":
                        _, t, h, cc = e
                        with nc.allow_low_precision(reason="q -> bf16"):
                            nc.vector.tensor_scalar_mul(
                                qt_all[:, h, cc * 256:(cc + 1) * 256],
                                t[:, cc * 256:(cc + 1) * 256], QDESCALE)
                        continue
                    _, t, wt, xt, cc, g, st, sp = e
                    nc.tensor.matmul(t[:, cc * 256:(cc + 1) * 256],
                                     wt[:, g], xt[:, cc, g],
                                     start=st, stop=sp, perf_mode=DR)
                    done += 1

            def qproj_pair(hp, defer_cc1=False):
                nonlocal wq_next, wq_next2
                pair_w = []
                for hh in range(2):
                    h = 2 * hp + hh
                    pair_w.append(wq_next)
                    wq_next = wq_next2
                    if h + 2 < HEADS:
                        wq_next2 = load_wq(h + 2)
                for hh in range(2):
                    h = 2 * hp + hh
                    wh, wl = pair_w[hh]
                    q_ps = qp_ps.tile([128, R], F32, tag="qp")
                    qproj_head_cc(h, wh, wl, q_ps, 0)
                    # copies on DVE, not ACT: the exp stream paces the
                    # attention tail, so ACT gets nothing extra
                    if defer_cc1:
                        with nc.allow_low_precision(reason="q -> bf16"):
                            nc.vector.tensor_scalar_mul(
                                qt_all[:, h, 0:256], q_ps[:, 0:256],
                                QDESCALE)
                        pending_cc1.append((h, wh, wl, q_ps))
                    else:
                        qproj_head_cc(h, wh, wl, q_ps, 1)
                        with nc.allow_low_precision(reason="q -> bf16"):
                            nc.vector.tensor_scalar_mul(
                                qt_all[:, h, :], q_ps[:], QDESCALE)

            def emit_ctile(ec, b, rt, ps_pool, sb_pool, ps_tag="op",
                           last=False):
                """One output-projection tile [r128, e512] (48 DR matmuls).

                last=True pipelines the epilogue per 256-col half (and
                splits the final half's DMA) to shorten the end drain.
                """
                o_ps = ps_pool.tile([128, 512], F32, tag=ps_tag)
                r0 = rt * 128
                terms = ((on_hi, woh_sb), (on_lo, woh_sb),
                         (on_hi, wol_sb))

                def group(dst, e0, ew):
                    n_mm = len(terms) * G
                    i = 0
                    for on_t, wo_t in terms:
                        for g in range(G):
                            nc.tensor.matmul(
                                dst[:, e0:e0 + ew],
                                on_t[:, b, g, :, r0:r0 + 128],
                                wo_t[:, ec, g, :, e0:e0 + ew],
                                start=(i == 0), stop=(i == n_mm - 1),
                                perf_mode=DR)
                            i += 1

                def epilogue(src, c0, cw, eng=None):
                    o_sb = sb_pool.tile([128, cw], F32, tag=f"ost{cw}")
                    nc.vector.tensor_scalar_mul(
                        o_sb[:], src[:, c0:c0 + cw], ODESCALE)
                    (eng or nc.sync).dma_start(
                        o_d[b * NC_ROWS + r0:b * NC_ROWS + r0 + 128,
                            ec * 512 + c0:ec * 512 + c0 + cw],
                        o_sb[:])

                if last:
                    # staircase drain: three column groups on separate
                    # PSUM banks, each epilogue overlapping the next
                    # group's matmuls
                    o_ps2 = ps_pool.tile([128, 512], F32, tag=ps_tag,
                                         name="ops_last2")
                    o_ps3 = ps_pool.tile([128, 512], F32, tag=ps_tag,
                                         name="ops_last3")
                    group(o_ps, 0, 256)
                    epilogue(o_ps, 0, 256, eng=nc.scalar)
                    group(o_ps2, 256, 128)
                    epilogue(o_ps2, 256, 128)
                    group(o_ps3, 384, 128)
                    epilogue(o_ps3, 384, 128, eng=nc.scalar)
                else:
                    group(o_ps, 0, 256)
                    group(o_ps, 256, 256)
                    epilogue(o_ps, 0, 512)

            # Pair-7 units have no qproj filler and run at the ACT exp pace:
            # trickle the first output-projection tile (ec0, b0, rt0) into
            # their PE slack, 3-5 matmuls per jg, g7 terms after pair-7's
            # b0 context exists. Keeps PE continuously busy (the cost
            # model's p-state ramp doubles the price of any PE idle gap).
            CTERMS = lambda: ((on_hi, woh_sb), (on_lo, woh_sb),  # noqa: E731
                              (on_hi, wol_sb))
            trickle = {"q": [], "ops": None}

            CTRICKLE_TILES = [(0, 0, 0), (0, 0, 1)]

            def trickle_init():
                trickle["tiles"] = []
                parts = []     # per tile: (eh0_main, eh0_g7, eh1_all)
                for ec, tb, rt in CTRICKLE_TILES:
                    ops = qp_ps.tile([128, 512], F32, tag="qp",
                                     name=f"ct_ops{ec}{tb}{rt}")
                    trickle["tiles"].append((ops, ec, tb, rt))
                    seg = []
                    for eh in range(2):
                        idx = [(t, g) for g in range(G - 1)
                               for t in range(3)]
                        idx += [(t, G - 1) for t in range(3)]
                        ent = [(ops, ec, tb, rt, eh, t, g,
                                i == 0, i == 3 * G - 1)
                               for i, (t, g) in enumerate(idx)]
                        seg.append(ent)
                    parts.append((seg[0][:21], seg[0][21:], seg[1]))
                # batch-0 slots drain only the first 21 entries (tile A's
                # g0-6): everything later reads pair-7's own context and
                # must be emitted after the b0 tail (batch-1 slots onward)
                (a0m, a0l, a1), (b0m, b0l, b1) = parts
                trickle["q"] = a0m + a0l + a1 + b0m + b0l + b1

            def trickle_emit(n):
                for _ in range(n):
                    if not trickle["q"]:
                        return
                    ops, ec, tb, rt, eh, t, g, st, sp = \
                        trickle["q"].pop(0)
                    on_t, wo_t = CTERMS()[t]
                    nc.tensor.matmul(
                        ops[:, eh * 256:eh * 256 + 256],
                        on_t[:, tb, g, :, rt * 128:rt * 128 + 128],
                        wo_t[:, ec, g, :, eh * 256:eh * 256 + 256],
                        start=st, stop=sp, perf_mode=DR)

            # b0 may drain at most 21 entries (tile A's eh0 g0-6): anything
            # later in the queue reads pair-7's own context, written by the
            # b0 tail which is EMITTED after b0's jg slots - an earlier
            # read would see uninitialized SBUF with no semaphore guard.
            TRICKLE_SLOTS = {(0, jg): 3 for jg in range(1, 8)}
            TRICKLE_SLOTS.update({(1, jg): 5 for jg in range(8)})

            qproj_pair(0, defer_cc1=True)
            for hp in range(HEADS // 2):
                for b in range(B):
                    if b == 0 and hp + 1 < HEADS // 2:
                        build_qtrickle(hp + 1)
                    if hp == HEADS // 2 - 1 and b == 0:
                        trickle_init()
                    # Both heads of the pair processed together: every matmul
                    # has a 512-wide moving operand laid out as [h2, i256].
                    acc = acc_ps.tile([128, 512], F32, tag="acc")
                    qt_pair = qt_all[:, 2 * hp:2 * hp + 2,
                                     b * NC_ROWS:(b + 1) * NC_ROWS]
                    s1024 = rbp.tile([128, 1024], F16, tag="s128")
                    # during the final attention unit the qproj PSUM banks
                    # are idle and all batch-0 context is split: inject
                    # early output-projection tiles to fill the ACT-paced
                    # tail of phase B
                    inject = False and (hp == HEADS // 2 - 1 and b == 1)
                    for jg in range(JT // 2):
                        if inject and jg in (1, 3, 5, 7):
                            ti = (1, 3, 5, 7).index(jg)
                            emit_ctile(ti // 2, 0, ti % 2, qp_ps, rbp,
                                       ps_tag="qp")
                        if pending_cc1 and hp == 0 and b == 0 \
                                and jg in (1, 3):
                            h, wh, wl, q_ps = pending_cc1.pop(0)
                            qproj_head_cc(h, wh, wl, q_ps, 1)
                            with nc.allow_low_precision(reason="q -> bf16"):
                                nc.vector.tensor_scalar_mul(
                                    qt_all[:, h, 256:512], q_ps[:, 256:512],
                                    QDESCALE)
                        sg = sg_ps.tile([128, 1024], F32, tag="sg")
                        for kk in range(2):
                            jt = jg * 2 + kk
                            nc.tensor.matmul(
                                sg[:, kk * 512:(kk + 1) * 512],
                                kt_sb[:, b, jt * 128:(jt + 1) * 128],
                                qt_pair,
                                start=True, stop=True)
                        es = esp.tile([128, 1024], BF16, tag="es")
                        with nc.allow_low_precision(reason="es bf16"):
                            nc.scalar.activation(
                                es[:], sg[:],
                                mybir.ActivationFunctionType.Exp,
                                scale=SCALE)
                            # softmax denominators: fp16 partial rowsums on
                            # DVE (2x 16-bit mode); partition reduce below
                            if jg == 0:
                                nc.vector.tensor_copy(s1024[:], es[:])
                            else:
                                nc.vector.tensor_add(s1024[:], s1024[:],
                                                     es[:])
                        if hp == HEADS // 2 - 1 and (b, jg) in TRICKLE_SLOTS:
                            trickle_emit(TRICKLE_SLOTS[(b, jg)])
                        qdrip(3)
                        for kk in range(2):
                            jt = jg * 2 + kk
                            esk = es[:, kk * 512:(kk + 1) * 512]
                            nc.tensor.matmul(acc[:], v_sb[:, b, jt, :],
                                             esk, start=(jt == 0),
                                             stop=(jt == JT - 1))
                        qdrip(3)
                    if b == 1:
                        qdrip(10 ** 6)  # force-drain before the next pair
                    # softmax-denominator tail + context fp8 hi/lo split
                    s512 = rbp.tile([128, 512], F32, tag="s512", bufs=1)
                    sB = rbp.tile([128, 512], F32, tag="sB", bufs=1)
                    rb_sb = rbp.tile([128, 512], F32, tag="rbs")
                    t32 = rbp.tile([128, 512], F32, tag="t32")
                    hi_ap = on_hi[:, b, hp].rearrange("p a b -> p (a b)")
                    lo_ap = on_lo[:, b, hp].rearrange("p a b -> p (a b)")
                    with nc.allow_low_precision(reason="denominator tail"):
                        nc.vector.tensor_add(s512[:], s1024[:, 0:512],
                                             s1024[:, 512:1024])
                        nc.gpsimd.partition_all_reduce(
                            sB[:], s512[:], channels=128,
                            reduce_op=bass_isa.ReduceOp.add)
                        nc.vector.reciprocal(rb_sb[:], sB[:])
                        nc.vector.tensor_mul(t32[:], acc[:], rb_sb[:])
                        nc.gpsimd.tensor_scalar_mul(hi_ap, t32[:], OS)
                        nc.vector.scalar_tensor_tensor(
                            lo_ap, t32[:], OS, hi_ap,
                            mybir.AluOpType.mult,
                            mybir.AluOpType.subtract)
                    if hp == HEADS // 2 - 1 and b == 1:
                        trickle_emit(10 ** 6)  # drain leftover tile work
                        for ops, ec, tb, rt in trickle["tiles"]:
                            o_sb = rbp.tile([128, 512], F32, tag="ost512")
                            nc.vector.tensor_scalar_mul(o_sb[:], ops[:],
                                                        ODESCALE)
                            nc.sync.dma_start(
                                o_d[tb * NC_ROWS + rt * 128:
                                    tb * NC_ROWS + rt * 128 + 128,
                                    ec * 512:(ec + 1) * 512],
                                o_sb[:])

        # ---- Phase C: remaining output-projection tiles ----
        # (ec0/ec1, b0, *) were injected into the tail of phase B above.
        with tc.tile_pool(name="ost_pool", bufs=4) as ostp, \
             tc.tile_pool(name="op_ps", bufs=4, space="PSUM") as op_ps:
            tiles = [(ec, 0, rt) for ec in range(4) for rt in (0, 1)
                     if (ec, 0, rt) not in ((0, 0, 0), (0, 0, 1))]
            tiles += [(ec, 1, rt) for ec in range(4) for rt in (0, 1)]
            for ti, (ec, b, rt) in enumerate(tiles):
                emit_ctile(ec, b, rt, op_ps, ostp,
                           last=(ti == len(tiles) - 1))


def _get_nc(reps: int = 1):
    if reps not in _CACHE:
        _CACHE[reps] = _build(reps)
    return _CACHE[reps]


def _hilo(a, pre):
    s = (a * pre).astype(np.float32)
    hi = s.astype(NE4)
    lo = (s - hi.astype(np.float32)).astype(NE4)
    return hi, lo


def _make_in_maps(x, k, v, Wq, Wo):
    # Wq [E, inner] -> [h, p, g, s, f] with e = 256g + 128s + p
    wq_t = Wq.reshape(G, 2, 128, HEADS, 128).transpose(3, 2, 0, 1, 4)
    wqh, wql = _hilo(np.ascontiguousarray(wq_t), WQS)
    # Wo [inner, E] -> [ec, p, g, s, e'] with f = 256g + 128s + p
    wo_t = Wo.reshape(G, 2, 128, 4, 512).transpose(3, 2, 0, 1, 4)
    woh, wol = _hilo(np.ascontiguousarray(wo_t), WOS)
    # k [B, J, DH] -> kT [d, b, j]
    kt = np.ascontiguousarray(k.transpose(2, 0, 1)).astype(NBF)
    # v [B, J, DH] -> [p, b, jt, d]
    vt = np.ascontiguousarray(
        v.reshape(B, JT, 128, DH).transpose(2, 0, 1, 3)).astype(NBF)
    in_maps = []
    for c in range(NCORES):
        xs = x[:, c * NC_ROWS:(c + 1) * NC_ROWS, :]
        # [E, cc, r256] -> [p, cc, g, s, r]
        xt = np.stack([xs[0].T, xs[1].T], axis=1)
        xt = np.ascontiguousarray(
            xt.reshape(G, 2, 128, 2, NC_ROWS).transpose(2, 3, 0, 1, 4))
        xh, xl = _hilo(xt, XS)
        in_maps.append({"xh": xh, "xl": xl, "wqh": wqh, "wql": wql,
                        "kt": kt, "vt": vt, "woh": woh, "wol": wol})
    return in_maps


def run_on_device(x, k, v, Wq, Wo, reps: int = 1):
    nc = _get_nc(reps)
    in_maps = _make_in_maps(x, k, v, Wq, Wo)
    res = run_bass_kernel_spmd(nc, in_maps, list(range(NCORES)))
    parts = [res.results[c]["o"].reshape(B, NC_ROWS, E) for c in range(NCORES)]
    return np.concatenate(parts, axis=1)


def kernel(x, k, v, Wq, Wo):
    x = np.asarray(x, dtype=np.float32)
    k = np.asarray(k, dtype=np.float32)
    v = np.asarray(v, dtype=np.float32)
    Wq = np.asarray(Wq, dtype=np.float32)
    Wo = np.asarray(Wo, dtype=np.float32)
    return run_on_device(x, k, v, Wq, Wo, reps=1)


# revision 96
# speedup vs baseline: 1.0383x; 1.0097x over previous
"""Trainium2 Bass kernel for MQA cross-attention (nn_CrossAttention).

Reference computation (fp32):
    q = (x @ Wq).reshape(b, n, 16, 128).transpose(0,2,1,3) * 128**-0.5
    sim = q @ k^T   (k/v shared across heads, MQA)
    out = softmax(sim) @ v
    y = out.merge_heads @ Wo

Sharding: pure sequence-parallel across 8 cores. Each core gets 256 rows
of x per batch (512 rows total), full Wq/Wo/k/v, and produces its 512 rows
of the output. No collectives, no host-side reduction.

Mixed precision (validated vs reference, rel err ~4e-3):
  - qproj / outproj run as fp8e4 DoubleRow matmuls (0.5 cycles/row,
    256-deep contraction) with hi+lo splits of both operands, dropping
    only the lo*lo term. Splits are power-of-2 prescaled on the host so
    the lo residuals clear e4m3's subnormal floor; the prescales are
    folded into on-chip scalars (ACT copy scale, final output scale).
  - sim / attn*v stay bf16 (K=128 per head makes DoubleRow useless for
    sim, and an es hi/lo split would cost a second full ACT/DVE pass).
  - softmax denominators: fp16 DVE partial rowsums (2x DVE mode) +
    gpsimd 128-way partition reduce; normalize+fp8-split of the context
    runs on DVE with the hi-cast offloaded to gpsimd.

Per-core PE cycles: qproj 98304 + sim 131072 + attn*v 131072 +
outproj 98304 = 458752 (vs 524288 all-f32r).

Overlap notes (modeled 214us vs 259us f32r baseline):
  - The ACT exp stream (1038ns per [128,1024] tile) paces the attention
    inner loop, so q PSUM->SBUF copies run on DVE, not ACT.
  - qproj for pair hp+1 is drip-fed a few matmuls per jg into pair hp's
    attention stream instead of bursting: during a burst ACT starves
    (sg double-buffering banks only 2 jg of sim backlog) and loses the
    lead it needs to cover the per-jg exp deficit.
  - Wo is SBUF-resident; its chunks ride the sync DMA queue behind the
    wq head stream (same-queue order stops the scheduler from hoisting
    them into the startup-critical window - DMA bandwidth is one shared
    ~335GB/s pool, so front-running Wo starves the x/wq/kv stream).
  - Pair-0 qproj defers its batch-1 column halves into the batch-0
    attention stream so the first sim starts ~4us earlier.
  - Pair-7 has no qproj filler: the first two output-projection tiles
    are trickled into its PE slack (3 matmuls per jg, g7 terms queued
    last since they need pair-7's own context). PE gaps are doubly
    expensive under the cost model's p-state ramp.
  - The final tile's epilogue is split per column block across the two
    DGE queues to shorten the end drain.
"""

import sys
import numpy as np
import ml_dtypes

for _p in ("/opt/trn_rl_repo", "/root/.axon_site/_ro/trn_rl_repo"):
    if _p not in sys.path:
        sys.path.append(_p)

import concourse.bass as bass  # noqa: E402
import concourse.mybir as mybir  # noqa: E402
import concourse.tile as tile  # noqa: E402
from concourse import bacc, bass_isa  # noqa: E402
from concourse.bass_utils import run_bass_kernel_spmd  # noqa: E402

F32 = mybir.dt.float32
F16 = mybir.dt.float16
BF16 = mybir.dt.bfloat16
F8 = mybir.dt.float8e4
DR = mybir.MatmulPerfMode.DoubleRow
NE4 = ml_dtypes.float8_e4m3
NBF = ml_dtypes.bfloat16

B = 2
N = 2048          # query length (global)
J = 2048          # kv length
E = 2048          # model dim
HEADS = 16
DH = 128          # head dim
NCORES = 8
NC_ROWS = N // NCORES        # 256 query rows per core per batch
R = B * NC_ROWS              # 512 rows per core, col = b*NC_ROWS + i
JT = J // 128                # 16 j-tiles
G = E // 256                 # 8 DoubleRow k-tiles over a 2048 contraction
SCALE = float(DH) ** -0.5

# host-side power-of-2 prescales for the fp8 hi/lo splits
XS = 8.0          # x
WQS = 32.0        # Wq
OS = 64.0         # normalized context (outn)
WOS = 32.0        # Wo
QDESCALE = 1.0 / (XS * WQS)      # folded into the ACT q copy
ODESCALE = 1.0 / (OS * WOS)      # folded into the final output copy

_CACHE = {}


def _build(reps: int = 1):
    nc = bacc.Bacc(name=f"mqa_xattn_dr_r{reps}")
    # x hi/lo: [p, cc(b), g, s, r256] with e = 256g + 128s + p
    xh_d = nc.declare_dram_parameter("xh", [128, 2, G, 2, NC_ROWS], F8,
                                     isOutput=False)
    xl_d = nc.declare_dram_parameter("xl", [128, 2, G, 2, NC_ROWS], F8,
                                     isOutput=False)
    wqh_d = nc.declare_dram_parameter("wqh", [HEADS, 128, G, 2, 128], F8,
                                      isOutput=False)
    wql_d = nc.declare_dram_parameter("wql", [HEADS, 128, G, 2, 128], F8,
                                      isOutput=False)
    kt_d = nc.declare_dram_parameter("kt", [128, B, J], BF16, isOutput=False)
    vt_d = nc.declare_dram_parameter("vt", [128, B, JT, DH], BF16,
                                     isOutput=False)
    woh_d = nc.declare_dram_parameter("woh", [4, 128, G, 2, 512], F8,
                                      isOutput=False)
    wol_d = nc.declare_dram_parameter("wol", [4, 128, G, 2, 512], F8,
                                      isOutput=False)
    o_d = nc.declare_dram_parameter("o", [R, E], F32, isOutput=True)

    with tile.TileContext(nc) as tc:
        for _ in range(reps):
            _emit_once(nc, tc, xh_d, xl_d, wqh_d, wql_d, kt_d, vt_d,
                       woh_d, wol_d, o_d)

    nc.compile()
    return nc


def _emit_once(nc, tc, xh_d, xl_d, wqh_d, wql_d, kt_d, vt_d,
               woh_d, wol_d, o_d):
    with tc.tile_pool(name="persist", bufs=1) as pp:
        kt_sb = pp.tile([128, B, J], BF16)
        v_sb = pp.tile([128, B, JT, DH], BF16)
        qt_all = pp.tile([128, HEADS, R], BF16)
        # context, normalized and fp8 hi/lo split, laid out for DoubleRow
        # outproj: [p, b, g, s, i] with f = 256*g + 128*s + p, i in [0,256)
        on_hi = pp.tile([128, B, G, 2, NC_ROWS], F8)
        on_lo = pp.tile([128, B, G, 2, NC_ROWS], F8)
        # Wo is fully resident; its DMAs stream on the gpsimd queue during
        # phase B so phase C starts without an SBUF/DMA stall.
        woh_sb = pp.tile([128, 4, G, 2, 512], F8)
        wol_sb = pp.tile([128, 4, G, 2, 512], F8)

        # ---- Phase B: q-projection + attention, per head pair ----
        with tc.tile_pool(name="xt_pool", bufs=1) as xtp, \
             tc.tile_pool(name="wq_pool", bufs=3) as wqp, \
             tc.tile_pool(name="es_pool", bufs=8) as esp, \
             tc.tile_pool(name="rb_pool", bufs=2) as rbp, \
             tc.tile_pool(name="qp_ps", bufs=2, space="PSUM") as qp_ps, \
             tc.tile_pool(name="sg_ps", bufs=2, space="PSUM") as sg_ps, \
             tc.tile_pool(name="acc_ps", bufs=2, space="PSUM") as acc_ps:
            xh_sb = xtp.tile([128, 2, G, 2, NC_ROWS], F8)
            xl_sb = xtp.tile([128, 2, G, 2, NC_ROWS], F8)

            # Wo prefetch chunks, paced into the sync DMA queue behind the
            # wq head stream (the scheduler keeps same-queue order, so these
            # can't hoist ahead of the startup-critical transfers).
            wo_chunks = [(dst, src, ec, g0)
                         for ec in range(4)
                         for dst, src in ((woh_sb, woh_d), (wol_sb, wol_d))
                         for g0 in (0, G // 2)]

            def load_wq(h):
                wh = wqp.tile([128, G, 2, 128], F8, tag="wqh",
                              name=f"wqh_sb{h}")
                wl = wqp.tile([128, G, 2, 128], F8, tag="wql",
                              name=f"wql_sb{h}")
                nc.sync.dma_start(wh[:], wqh_d[h])
                nc.sync.dma_start(wl[:], wql_d[h])
                if h >= 2:
                    for _ in range(2):
                        if wo_chunks:
                            dst, src, ec, g0 = wo_chunks.pop(0)
                            nc.sync.dma_start(
                                dst[:, ec, g0:g0 + G // 2],
                                src[ec, :, g0:g0 + G // 2])
                return wh, wl

            # DMA order tuned so the first qproj group starts ~1us in and
            # batch-0 attention is never input-starved.
            wqh0 = wqp.tile([128, G, 2, 128], F8, tag="wqh", name="wqh_sb0")
            wql0 = wqp.tile([128, G, 2, 128], F8, tag="wql", name="wql_sb0")
            # x stream on the scalar-engine DGE queue, weights/kv on sync:
            # transfers share one bandwidth pool but per-DMA issue dead
            # time overlaps across queues
            nc.sync.dma_start(wqh0[:, 0:2], wqh_d[0, :, 0:2])
            nc.sync.dma_start(xh_sb[:, 0, 0:2], xh_d[:, 0, 0:2])
            nc.sync.dma_start(wqh0[:, 2:G], wqh_d[0, :, 2:G])
            nc.sync.dma_start(xh_sb[:, 0, 2:G], xh_d[:, 0, 2:G])
            nc.sync.dma_start(wql0[:, 0:4], wql_d[0, :, 0:4])
            nc.sync.dma_start(xl_sb[:, 0, 0:4], xl_d[:, 0, 0:4])
            nc.sync.dma_start(wql0[:, 4:G], wql_d[0, :, 4:G])
            nc.sync.dma_start(xl_sb[:, 0, 4:G], xl_d[:, 0, 4:G])
            wq_next = (wqh0, wql0)
            wq_next2 = load_wq(1)
            nc.sync.dma_start(kt_sb[:, 0, 0:1024], kt_d[:, 0, 0:1024])
            nc.sync.dma_start(v_sb[:, 0, 0:8], vt_d[:, 0, 0:8])
            nc.sync.dma_start(xh_sb[:, 1], xh_d[:, 1])
            nc.sync.dma_start(xl_sb[:, 1], xl_d[:, 1])
            nc.sync.dma_start(kt_sb[:, 0, 1024:J], kt_d[:, 0, 1024:J])
            nc.sync.dma_start(v_sb[:, 0, 8:JT], vt_d[:, 0, 8:JT])
            nc.sync.dma_start(kt_sb[:, 1, :], kt_d[:, 1, :])
            nc.sync.dma_start(v_sb[:, 1], vt_d[:, 1])

            def qproj_head_cc(h, wh, wl, q_ps, cc):
                # 3-term hi/lo: Wh@xh + Wl@xh + Wh@xl, one 256-col half
                terms = ((wh, xh_sb), (wl, xh_sb), (wh, xl_sb))
                n_mm = len(terms) * G
                i = 0
                for wt, xt in terms:
                    for g in range(G):
                        nc.tensor.matmul(
                            q_ps[:, cc * 256:(cc + 1) * 256],
                            wt[:, g],
                            xt[:, cc, g],
                            start=(i == 0), stop=(i == n_mm - 1),
                            perf_mode=DR)
                        i += 1

            pending_cc1 = []    # pair-0 cc1 work, interleaved into b0 attn

            # qproj for pair hp+1 is not emitted as a burst (ACT starves
            # during bursts: sg double-buffering banks only 2 jg of sim
            # backlog, so the exp stream idles and loses its lead). It is
            # drip-fed 3 matmuls at a time into pair hp's attention stream,
            # matching the per-jg ACT deficit.
            qtrickle = {"q": []}

            def build_qtrickle(hp1):
                nonlocal wq_next, wq_next2
                pw = []
                for hh in range(2):
                    h = 2 * hp1 + hh
                    pw.append(wq_next)
                    wq_next = wq_next2
                    if h + 2 < HEADS:
                        wq_next2 = load_wq(h + 2)
                tiles_ = [qp_ps.tile([128, R], F32, tag="qp",
                                     name=f"qpt{hp1}_{i}") for i in range(2)]
                q = []
                # cc0 groups (both heads) first: the next pair's batch-0
                # sim needs only the cc0 halves of qt
                for cc in range(2):
                    for hh in range(2):
                        h = 2 * hp1 + hh
                        wh, wl = pw[hh]
                        terms = ((wh, xh_sb), (wl, xh_sb), (wh, xl_sb))
                        n = 0
                        for wt, xt in terms:
                            for g in range(G):
                                q.append(("mm", tiles_[hh], wt, xt, cc, g,
                                          n == 0, n == 3 * G - 1))
                                n += 1
                        q.append(("copy", tiles_[hh], h, cc))
                qtrickle["q"] = q

            def qdrip(nmm):
                done = 0
                while qtrickle["q"] and done < nmm:
                    e = qtrickle["q"].pop(0)
                    if e[0] == "copy":
                        _, t, h, cc = e
                        with nc.allow_low_precision(reason="q -> bf16"):
                            nc.vector.tensor_scalar_mul(
                                qt_all[:, h, cc * 256:(cc + 1) * 256],
                                t[:, cc * 256:(cc + 1) * 256], QDESCALE)
                        continue
                    _, t, wt, xt, cc, g, st, sp = e
                    nc.tensor.matmul(t[:, cc * 256:(cc + 1) * 256],
                                     wt[:, g], xt[:, cc, g],
                                     start=st, stop=sp, perf_mode=DR)
                    done += 1

            def qproj_pair(hp, defer_cc1=False):
                nonlocal wq_next, wq_next2
                pair_w = []
                for hh in range(2):
                    h = 2 * hp + hh
                    pair_w.append(wq_next)
                    wq_next = wq_next2
                    if h + 2 < HEADS:
                        wq_next2 = load_wq(h + 2)
                for hh in range(2):
                    h = 2 * hp + hh
                    wh, wl = pair_w[hh]
                    q_ps = qp_ps.tile([128, R], F32, tag="qp")
                    qproj_head_cc(h, wh, wl, q_ps, 0)
                    # copies on DVE, not ACT: the exp stream paces the
                    # attention tail, so ACT gets nothing extra
                    if defer_cc1:
                        with nc.allow_low_precision(reason="q -> bf16"):
                            nc.vector.tensor_scalar_mul(
                                qt_all[:, h, 0:256], q_ps[:, 0:256],
                                QDESCALE)
                        pending_cc1.append((h, wh, wl, q_ps))
                    else:
                        qproj_head_cc(h, wh, wl, q_ps, 1)
                        with nc.allow_low_precision(reason="q -> bf16"):
                            nc.vector.tensor_scalar_mul(
                                qt_all[:, h, :], q_ps[:], QDESCALE)

            def emit_ctile(ec, b, rt, ps_pool, sb_pool, ps_tag="op",
                           last=False):
                """One output-projection tile [r128, e512] (48 DR matmuls).

                last=True pipelines the epilogue per 256-col half (and
                splits the final half's DMA) to shorten the end drain.
                """
                o_ps = ps_pool.tile([128, 512], F32, tag=ps_tag)
                r0 = rt * 128
                terms = ((on_hi, woh_sb), (on_lo, woh_sb),
                         (on_hi, wol_sb))

                def group(dst, e0, ew):
                    n_mm = len(terms) * G
                    i = 0
                    for on_t, wo_t in terms:
                        for g in range(G):
                            nc.tensor.matmul(
                                dst[:, e0:e0 + ew],
                                on_t[:, b, g, :, r0:r0 + 128],
                                wo_t[:, ec, g, :, e0:e0 + ew],
                                start=(i == 0), stop=(i == n_mm - 1),
                                perf_mode=DR)
                            i += 1

                def epilogue(src, c0, cw, eng=None):
                    o_sb = sb_pool.tile([128, cw], F32, tag=f"ost{cw}")
                    nc.vector.tensor_scalar_mul(
                        o_sb[:], src[:, c0:c0 + cw], ODESCALE)
                    (eng or nc.sync).dma_start(
                        o_d[b * NC_ROWS + r0:b * NC_ROWS + r0 + 128,
                            ec * 512 + c0:ec * 512 + c0 + cw],
                        o_sb[:])

                if last:
                    # staircase drain: three column groups on separate
                    # PSUM banks, each epilogue overlapping the next
                    # group's matmuls
                    o_ps2 = ps_pool.tile([128, 512], F32, tag=ps_tag,
                                         name="ops_last2")
                    o_ps3 = ps_pool.tile([128, 512], F32, tag=ps_tag,
                                         name="ops_last3")
                    group(o_ps, 0, 256)
                    epilogue(o_ps, 0, 256, eng=nc.scalar)
                    group(o_ps2, 256, 128)
                    epilogue(o_ps2, 256, 128)
                    group(o_ps3, 384, 128)
                    epilogue(o_ps3, 384, 128, eng=nc.scalar)
                else:
                    group(o_ps, 0, 256)
                    group(o_ps, 256, 256)
                    epilogue(o_ps, 0, 512)

            # Pair-7 units have no qproj filler and run at the ACT exp pace:
            # trickle the first output-projection tile (ec0, b0, rt0) into
            # their PE slack, 3-5 matmuls per jg, g7 terms after pair-7's
            # b0 context exists. Keeps PE continuously busy (the cost
            # model's p-state ramp doubles the price of any PE idle gap).
            CTERMS = lambda: ((on_hi, woh_sb), (on_lo, woh_sb),  # noqa: E731
                              (on_hi, wol_sb))
            trickle = {"q": [], "ops": None}

            CTRICKLE_TILES = [(0, 0, 0), (0, 0, 1)]

            def trickle_init():
                trickle["tiles"] = []
                parts = []     # per tile: (eh0_main, eh0_g7, eh1_all)
                for ec, tb, rt in CTRICKLE_TILES:
                    ops = qp_ps.tile([128, 512], F32, tag="qp",
                                     name=f"ct_ops{ec}{tb}{rt}")
                    trickle["tiles"].append((ops, ec, tb, rt))
                    seg = []
                    for eh in range(2):
                        idx = [(t, g) for g in range(G - 1)
                               for t in range(3)]
                        idx += [(t, G - 1) for t in range(3)]
                        ent = [(ops, ec, tb, rt, eh, t, g,
                                i == 0, i == 3 * G - 1)
                               for i, (t, g) in enumerate(idx)]
                        seg.append(ent)
                    parts.append((seg[0][:21], seg[0][21:], seg[1]))
                # batch-0 slots drain only the first 21 entries (tile A's
                # g0-6): everything later reads pair-7's own context and
                # must be emitted after the b0 tail (batch-1 slots onward)
                (a0m, a0l, a1), (b0m, b0l, b1) = parts
                trickle["q"] = a0m + a0l + a1 + b0m + b0l + b1

            def trickle_emit(n):
                for _ in range(n):
                    if not trickle["q"]:
                        return
                    ops, ec, tb, rt, eh, t, g, st, sp = \
                        trickle["q"].pop(0)
                    on_t, wo_t = CTERMS()[t]
                    nc.tensor.matmul(
                        ops[:, eh * 256:eh * 256 + 256],
                        on_t[:, tb, g, :, rt * 128:rt * 128 + 128],
                        wo_t[:, ec, g, :, eh * 256:eh * 256 + 256],
                        start=st, stop=sp, perf_mode=DR)

            # b0 may drain at most 21 entries (tile A's eh0 g0-6): anything
            # later in the queue reads pair-7's own context, written by the
            # b0 tail which is EMITTED after b0's jg slots - an earlier
            # read would see uninitialized SBUF with no semaphore guard.
            TRICKLE_SLOTS = {(0, jg): 3 for jg in range(1, 8)}
            TRICKLE_SLOTS.update({(1, jg): 5 for jg in range(8)})

            qproj_pair(0, defer_cc1=True)
            for hp in range(HEADS // 2):
                for b in range(B):
                    if b == 0 and hp + 1 < HEADS // 2:
                        build_qtrickle(hp + 1)
                    if hp == HEADS // 2 - 1 and b == 0:
                        trickle_init()
                    # Both heads of the pair processed together: every matmul
                    # has a 512-wide moving operand laid out as [h2, i256].
                    acc = acc_ps.tile([128, 512], F32, tag="acc")
                    qt_pair = qt_all[:, 2 * hp:2 * hp + 2,
                                     b * NC_ROWS:(b + 1) * NC_ROWS]
                    s1024 = rbp.tile([128, 1024], F16, tag="s128")
                    # during the final attention unit the qproj PSUM banks
                    # are idle and all batch-0 context is split: inject
                    # early output-projection tiles to fill the ACT-paced
                    # tail of phase B
                    inject = False and (hp == HEADS // 2 - 1 and b == 1)
                    for jg in range(JT // 2):
                        if inject and jg in (1, 3, 5, 7):
                            ti = (1, 3, 5, 7).index(jg)
                            emit_ctile(ti // 2, 0, ti % 2, qp_ps, rbp,
                                       ps_tag="qp")
                        if pending_cc1 and hp == 0 and b == 0 \
                                and jg in (1, 3):
                            h, wh, wl, q_ps = pending_cc1.pop(0)
                            qproj_head_cc(h, wh, wl, q_ps, 1)
                            with nc.allow_low_precision(reason="q -> bf16"):
                                nc.vector.tensor_scalar_mul(
                                    qt_all[:, h, 256:512], q_ps[:, 256:512],
                                    QDESCALE)
                        sg = sg_ps.tile([128, 1024], F32, tag="sg")
                        for kk in range(2):
                            jt = jg * 2 + kk
                            nc.tensor.matmul(
                                sg[:, kk * 512:(kk + 1) * 512],
                                kt_sb[:, b, jt * 128:(jt + 1) * 128],
                                qt_pair,
                                start=True, stop=True)
                        es = esp.tile([128, 1024], BF16, tag="es")
                        with nc.allow_low_precision(reason="es bf16"):
                            nc.scalar.activation(
                                es[:], sg[:],
                                mybir.ActivationFunctionType.Exp,
                                scale=SCALE)
                            # softmax denominators: fp16 partial rowsums on
                            # DVE (2x 16-bit mode); partition reduce below
                            if jg == 0:
                                nc.vector.tensor_copy(s1024[:], es[:])
                            else:
                                nc.vector.tensor_add(s1024[:], s1024[:],
                                                     es[:])
                        if hp == HEADS // 2 - 1 and (b, jg) in TRICKLE_SLOTS:
                            trickle_emit(TRICKLE_SLOTS[(b, jg)])
                        qdrip(3)
                        # software-pipeline by one jg: consume the PREVIOUS
                        # jg's es, so the exp->attnv semaphore has fired
                        # long before PE arrives (no catch-up seam)
                        if jg > 0:
                            pj = jg - 1
                            for kk in range(2):
                                jt = pj * 2 + kk
                                nc.tensor.matmul(
                                    acc[:], v_sb[:, b, jt, :],
                                    es_prev[:, kk * 512:(kk + 1) * 512],
                                    start=(jt == 0), stop=False)
                        es_prev = es
                        qdrip(3)
                    # drain the pipelined final jg's attn*v
                    for kk in range(2):
                        jt = (JT // 2 - 1) * 2 + kk
                        nc.tensor.matmul(acc[:], v_sb[:, b, jt, :],
                                         es_prev[:, kk * 512:(kk + 1) * 512],
                                         start=False, stop=(jt == JT - 1))
                    if b == 1:
                        qdrip(10 ** 6)  # force-drain before the next pair
                    # softmax-denominator tail + context fp8 hi/lo split
                    s512 = rbp.tile([128, 512], F32, tag="s512", bufs=1)
                    sB = rbp.tile([128, 512], F32, tag="sB", bufs=1)
                    rb_sb = rbp.tile([128, 512], F32, tag="rbs")
                    t32 = rbp.tile([128, 512], F32, tag="t32")
                    hi_ap = on_hi[:, b, hp].rearrange("p a b -> p (a b)")
                    lo_ap = on_lo[:, b, hp].rearrange("p a b -> p (a b)")
                    with nc.allow_low_precision(reason="denominator tail"):
                        nc.vector.tensor_add(s512[:], s1024[:, 0:512],
                                             s1024[:, 512:1024])
                        nc.gpsimd.partition_all_reduce(
                            sB[:], s512[:], channels=128,
                            reduce_op=bass_isa.ReduceOp.add)
                        nc.vector.reciprocal(rb_sb[:], sB[:])
                        nc.vector.tensor_mul(t32[:], acc[:], rb_sb[:])
                        nc.gpsimd.tensor_scalar_mul(hi_ap, t32[:], OS)
                        nc.vector.scalar_tensor_tensor(
                            lo_ap, t32[:], OS, hi_ap,
                            mybir.AluOpType.mult,
                            mybir.AluOpType.subtract)
                    if hp == HEADS // 2 - 1 and b == 1:
                        trickle_emit(10 ** 6)  # drain leftover tile work
                        for ops, ec, tb, rt in trickle["tiles"]:
                            o_sb = rbp.tile([128, 512], F32, tag="ost512")
                            nc.vector.tensor_scalar_mul(o_sb[:], ops[:],
                                                        ODESCALE)
                            nc.sync.dma_start(
                                o_d[tb * NC_ROWS + rt * 128:
                                    tb * NC_ROWS + rt * 128 + 128,
                                    ec * 512:(ec + 1) * 512],
                                o_sb[:])

        # ---- Phase C: remaining output-projection tiles ----
        # (ec0/ec1, b0, *) were injected into the tail of phase B above.
        with tc.tile_pool(name="ost_pool", bufs=4) as ostp, \
             tc.tile_pool(name="op_ps", bufs=4, space="PSUM") as op_ps:
            tiles = [(ec, 0, rt) for ec in range(4) for rt in (0, 1)
                     if (ec, 0, rt) not in ((0, 0, 0), (0, 0, 1))]
            tiles += [(ec, 1, rt) for ec in range(4) for rt in (0, 1)]
            for ti, (ec, b, rt) in enumerate(tiles):
                emit_ctile(ec, b, rt, op_ps, ostp,
                           last=(ti == len(tiles) - 1))


def _get_nc(reps: int = 1):
    if reps not in _CACHE:
        _CACHE[reps] = _build(reps)
    return _CACHE[reps]


def _hilo(a, pre):
    s = (a * pre).astype(np.float32)
    hi = s.astype(NE4)
    lo = (s - hi.astype(np.float32)).astype(NE4)
    return hi, lo


def _make_in_maps(x, k, v, Wq, Wo):
    # Wq [E, inner] -> [h, p, g, s, f] with e = 256g + 128s + p
    wq_t = Wq.reshape(G, 2, 128, HEADS, 128).transpose(3, 2, 0, 1, 4)
    wqh, wql = _hilo(np.ascontiguousarray(wq_t), WQS)
    # Wo [inner, E] -> [ec, p, g, s, e'] with f = 256g + 128s + p
    wo_t = Wo.reshape(G, 2, 128, 4, 512).transpose(3, 2, 0, 1, 4)
    woh, wol = _hilo(np.ascontiguousarray(wo_t), WOS)
    # k [B, J, DH] -> kT [d, b, j]
    kt = np.ascontiguousarray(k.transpose(2, 0, 1)).astype(NBF)
    # v [B, J, DH] -> [p, b, jt, d]
    vt = np.ascontiguousarray(
        v.reshape(B, JT, 128, DH).transpose(2, 0, 1, 3)).astype(NBF)
    in_maps = []
    for c in range(NCORES):
        xs = x[:, c * NC_ROWS:(c + 1) * NC_ROWS, :]
        # [E, cc, r256] -> [p, cc, g, s, r]
        xt = np.stack([xs[0].T, xs[1].T], axis=1)
        xt = np.ascontiguousarray(
            xt.reshape(G, 2, 128, 2, NC_ROWS).transpose(2, 3, 0, 1, 4))
        xh, xl = _hilo(xt, XS)
        in_maps.append({"xh": xh, "xl": xl, "wqh": wqh, "wql": wql,
                        "kt": kt, "vt": vt, "woh": woh, "wol": wol})
    return in_maps


def run_on_device(x, k, v, Wq, Wo, reps: int = 1):
    nc = _get_nc(reps)
    in_maps = _make_in_maps(x, k, v, Wq, Wo)
    res = run_bass_kernel_spmd(nc, in_maps, list(range(NCORES)))
    parts = [res.results[c]["o"].reshape(B, NC_ROWS, E) for c in range(NCORES)]
    return np.concatenate(parts, axis=1)


def kernel(x, k, v, Wq, Wo):
    x = np.asarray(x, dtype=np.float32)
    k = np.asarray(k, dtype=np.float32)
    v = np.asarray(v, dtype=np.float32)
    Wq = np.asarray(Wq, dtype=np.float32)
    Wo = np.asarray(Wo, dtype=np.float32)
    return run_on_device(x, k, v, Wq, Wo, reps=1)


# revision 109
# speedup vs baseline: 1.0396x; 1.0013x over previous
"""Trainium2 Bass kernel for MQA cross-attention (nn_CrossAttention).

Reference computation (fp32):
    q = (x @ Wq).reshape(b, n, 16, 128).transpose(0,2,1,3) * 128**-0.5
    sim = q @ k^T   (k/v shared across heads, MQA)
    out = softmax(sim) @ v
    y = out.merge_heads @ Wo

Sharding: pure sequence-parallel across 8 cores. Each core gets 256 rows
of x per batch (512 rows total), full Wq/Wo/k/v, and produces its 512 rows
of the output. No collectives, no host-side reduction.

Mixed precision (validated vs reference, rel err ~4e-3):
  - qproj / outproj run as fp8e4 DoubleRow matmuls (0.5 cycles/row,
    256-deep contraction) with hi+lo splits of both operands, dropping
    only the lo*lo term. Splits are power-of-2 prescaled on the host so
    the lo residuals clear e4m3's subnormal floor; the prescales are
    folded into on-chip scalars (ACT copy scale, final output scale).
  - sim / attn*v stay bf16 (K=128 per head makes DoubleRow useless for
    sim, and an es hi/lo split would cost a second full ACT/DVE pass).
  - softmax denominators: fp16 DVE partial rowsums (2x DVE mode) +
    gpsimd 128-way partition reduce; normalize+fp8-split of the context
    runs on DVE with the hi-cast offloaded to gpsimd.

Per-core PE cycles: qproj 98304 + sim 131072 + attn*v 131072 +
outproj 98304 = 458752 (vs 524288 all-f32r).

Overlap notes (modeled 214us vs 259us f32r baseline):
  - The ACT exp stream (1038ns per [128,1024] tile) paces the attention
    inner loop, so q PSUM->SBUF copies run on DVE, not ACT.
  - qproj for pair hp+1 is drip-fed a few matmuls per jg into pair hp's
    attention stream instead of bursting: during a burst ACT starves
    (sg double-buffering banks only 2 jg of sim backlog) and loses the
    lead it needs to cover the per-jg exp deficit.
  - Wo is SBUF-resident; its chunks ride the sync DMA queue behind the
    wq head stream (same-queue order stops the scheduler from hoisting
    them into the startup-critical window - DMA bandwidth is one shared
    ~335GB/s pool, so front-running Wo starves the x/wq/kv stream).
  - Pair-0 qproj defers its batch-1 column halves into the batch-0
    attention stream so the first sim starts ~4us earlier.
  - Pair-7 has no qproj filler: the first two output-projection tiles
    are trickled into its PE slack (3 matmuls per jg, g7 terms queued
    last since they need pair-7's own context). PE gaps are doubly
    expensive under the cost model's p-state ramp.
  - The final tile's epilogue is split per column block across the two
    DGE queues to shorten the end drain.
"""

import sys
import numpy as np
import ml_dtypes

for _p in ("/opt/trn_rl_repo", "/root/.axon_site/_ro/trn_rl_repo"):
    if _p not in sys.path:
        sys.path.append(_p)

import concourse.bass as bass  # noqa: E402
import concourse.mybir as mybir  # noqa: E402
import concourse.tile as tile  # noqa: E402
from concourse import bacc, bass_isa  # noqa: E402
from concourse.bass_utils import run_bass_kernel_spmd  # noqa: E402

F32 = mybir.dt.float32
F16 = mybir.dt.float16
BF16 = mybir.dt.bfloat16
F8 = mybir.dt.float8e4
DR = mybir.MatmulPerfMode.DoubleRow
NE4 = ml_dtypes.float8_e4m3
NBF = ml_dtypes.bfloat16

B = 2
N = 2048          # query length (global)
J = 2048          # kv length
E = 2048          # model dim
HEADS = 16
DH = 128          # head dim
NCORES = 8
NC_ROWS = N // NCORES        # 256 query rows per core per batch
R = B * NC_ROWS              # 512 rows per core, col = b*NC_ROWS + i
JT = J // 128                # 16 j-tiles
G = E // 256                 # 8 DoubleRow k-tiles over a 2048 contraction
SCALE = float(DH) ** -0.5

# host-side power-of-2 prescales for the fp8 hi/lo splits
XS = 8.0          # x
WQS = 32.0        # Wq
OS = 64.0         # normalized context (outn)
WOS = 32.0        # Wo
QDESCALE = 1.0 / (XS * WQS)      # folded into the ACT q copy
ODESCALE = 1.0 / (OS * WOS)      # folded into the final output copy

_CACHE = {}


def _build(reps: int = 1):
    nc = bacc.Bacc(name=f"mqa_xattn_dr_r{reps}")
    # x hi/lo: [p, cc(b), g, s, r256] with e = 256g + 128s + p
    xh_d = nc.declare_dram_parameter("xh", [128, 2, G, 2, NC_ROWS], F8,
                                     isOutput=False)
    xl_d = nc.declare_dram_parameter("xl", [128, 2, G, 2, NC_ROWS], F8,
                                     isOutput=False)
    wqh_d = nc.declare_dram_parameter("wqh", [HEADS, 128, G, 2, 128], F8,
                                      isOutput=False)
    wql_d = nc.declare_dram_parameter("wql", [HEADS, 128, G, 2, 128], F8,
                                      isOutput=False)
    kt_d = nc.declare_dram_parameter("kt", [128, B, J], BF16, isOutput=False)
    vt_d = nc.declare_dram_parameter("vt", [128, B, JT, DH], BF16,
                                     isOutput=False)
    woh_d = nc.declare_dram_parameter("woh", [4, 128, G, 2, 512], F8,
                                      isOutput=False)
    wol_d = nc.declare_dram_parameter("wol", [4, 128, G, 2, 512], F8,
                                      isOutput=False)
    o_d = nc.declare_dram_parameter("o", [R, E], F32, isOutput=True)

    with tile.TileContext(nc) as tc:
        for _ in range(reps):
            _emit_once(nc, tc, xh_d, xl_d, wqh_d, wql_d, kt_d, vt_d,
                       woh_d, wol_d, o_d)

    nc.compile()
    return nc


def _emit_once(nc, tc, xh_d, xl_d, wqh_d, wql_d, kt_d, vt_d,
               woh_d, wol_d, o_d):
    with tc.tile_pool(name="persist", bufs=1) as pp:
        kt_sb = pp.tile([128, B, J], BF16)
        v_sb = pp.tile([128, B, JT, DH], BF16)
        qt_all = pp.tile([128, HEADS, R], BF16)
        # context, normalized and fp8 hi/lo split, laid out for DoubleRow
        # outproj: [p, b, g, s, i] with f = 256*g + 128*s + p, i in [0,256)
        on_hi = pp.tile([128, B, G, 2, NC_ROWS], F8)
        on_lo = pp.tile([128, B, G, 2, NC_ROWS], F8)
        # Wo is fully resident; its DMAs stream on the gpsimd queue during
        # phase B so phase C starts without an SBUF/DMA stall.
        woh_sb = pp.tile([128, 4, G, 2, 512], F8)
        wol_sb = pp.tile([128, 4, G, 2, 512], F8)

        # ---- Phase B: q-projection + attention, per head pair ----
        with tc.tile_pool(name="xt_pool", bufs=1) as xtp, \
             tc.tile_pool(name="wq_pool", bufs=3) as wqp, \
             tc.tile_pool(name="es_pool", bufs=8) as esp, \
             tc.tile_pool(name="rb_pool", bufs=2) as rbp, \
             tc.tile_pool(name="qp_ps", bufs=2, space="PSUM") as qp_ps, \
             tc.tile_pool(name="sg_ps", bufs=2, space="PSUM") as sg_ps, \
             tc.tile_pool(name="acc_ps", bufs=2, space="PSUM") as acc_ps:
            xh_sb = xtp.tile([128, 2, G, 2, NC_ROWS], F8)
            xl_sb = xtp.tile([128, 2, G, 2, NC_ROWS], F8)

            # Wo prefetch chunks, paced into the sync DMA queue behind the
            # wq head stream (the scheduler keeps same-queue order, so these
            # can't hoist ahead of the startup-critical transfers).
            wo_chunks = [(dst, src, ec, g0)
                         for ec in range(4)
                         for dst, src in ((woh_sb, woh_d), (wol_sb, wol_d))
                         for g0 in (0, G // 2)]

            def load_wq(h):
                wh = wqp.tile([128, G, 2, 128], F8, tag="wqh",
                              name=f"wqh_sb{h}")
                wl = wqp.tile([128, G, 2, 128], F8, tag="wql",
                              name=f"wql_sb{h}")
                nc.sync.dma_start(wh[:], wqh_d[h])
                nc.sync.dma_start(wl[:], wql_d[h])
                if h >= 2:
                    for _ in range(2):
                        if wo_chunks:
                            dst, src, ec, g0 = wo_chunks.pop(0)
                            nc.sync.dma_start(
                                dst[:, ec, g0:g0 + G // 2],
                                src[ec, :, g0:g0 + G // 2])
                return wh, wl

            # DMA order tuned so the first qproj group starts ~1us in and
            # batch-0 attention is never input-starved.
            wqh0 = wqp.tile([128, G, 2, 128], F8, tag="wqh", name="wqh_sb0")
            wql0 = wqp.tile([128, G, 2, 128], F8, tag="wql", name="wql_sb0")
            # x stream on the scalar-engine DGE queue, weights/kv on sync:
            # transfers share one bandwidth pool but per-DMA issue dead
            # time overlaps across queues
            nc.sync.dma_start(wqh0[:, 0:2], wqh_d[0, :, 0:2])
            nc.sync.dma_start(xh_sb[:, 0, 0:2], xh_d[:, 0, 0:2])
            nc.sync.dma_start(wqh0[:, 2:G], wqh_d[0, :, 2:G])
            nc.sync.dma_start(xh_sb[:, 0, 2:G], xh_d[:, 0, 2:G])
            nc.sync.dma_start(wql0[:, 0:4], wql_d[0, :, 0:4])
            nc.sync.dma_start(xl_sb[:, 0, 0:4], xl_d[:, 0, 0:4])
            nc.sync.dma_start(wql0[:, 4:G], wql_d[0, :, 4:G])
            nc.sync.dma_start(xl_sb[:, 0, 4:G], xl_d[:, 0, 4:G])
            wq_next = (wqh0, wql0)
            wq_next2 = load_wq(1)
            nc.sync.dma_start(kt_sb[:, 0, 0:1024], kt_d[:, 0, 0:1024])
            nc.sync.dma_start(v_sb[:, 0, 0:8], vt_d[:, 0, 0:8])
            nc.sync.dma_start(xh_sb[:, 1], xh_d[:, 1])
            nc.sync.dma_start(xl_sb[:, 1], xl_d[:, 1])
            nc.sync.dma_start(kt_sb[:, 0, 1024:J], kt_d[:, 0, 1024:J])
            nc.sync.dma_start(v_sb[:, 0, 8:JT], vt_d[:, 0, 8:JT])
            nc.sync.dma_start(kt_sb[:, 1, :], kt_d[:, 1, :])
            nc.sync.dma_start(v_sb[:, 1], vt_d[:, 1])

            def qproj_head_cc(h, wh, wl, q_ps, cc):
                # 3-term hi/lo: Wh@xh + Wl@xh + Wh@xl, one 256-col half
                terms = ((wh, xh_sb), (wl, xh_sb), (wh, xl_sb))
                n_mm = len(terms) * G
                i = 0
                for wt, xt in terms:
                    for g in range(G):
                        nc.tensor.matmul(
                            q_ps[:, cc * 256:(cc + 1) * 256],
                            wt[:, g],
                            xt[:, cc, g],
                            start=(i == 0), stop=(i == n_mm - 1),
                            perf_mode=DR)
                        i += 1

            pending_cc1 = []    # pair-0 cc1 work, interleaved into b0 attn

            # qproj for pair hp+1 is not emitted as a burst (ACT starves
            # during bursts: sg double-buffering banks only 2 jg of sim
            # backlog, so the exp stream idles and loses its lead). It is
            # drip-fed 3 matmuls at a time into pair hp's attention stream,
            # matching the per-jg ACT deficit.
            qtrickle = {"q": []}

            def build_qtrickle(hp1):
                nonlocal wq_next, wq_next2
                pw = []
                for hh in range(2):
                    h = 2 * hp1 + hh
                    pw.append(wq_next)
                    wq_next = wq_next2
                    if h + 2 < HEADS:
                        wq_next2 = load_wq(h + 2)
                tiles_ = [qp_ps.tile([128, R], F32, tag="qp",
                                     name=f"qpt{hp1}_{i}") for i in range(2)]
                q = []
                # cc0 groups (both heads) first: the next pair's batch-0
                # sim needs only the cc0 halves of qt
                for cc in range(2):
                    for hh in range(2):
                        h = 2 * hp1 + hh
                        wh, wl = pw[hh]
                        terms = ((wh, xh_sb), (wl, xh_sb), (wh, xl_sb))
                        n = 0
                        for wt, xt in terms:
                            for g in range(G):
                                q.append(("mm", tiles_[hh], wt, xt, cc, g,
                                          n == 0, n == 3 * G - 1))
                                n += 1
                        q.append(("copy", tiles_[hh], h, cc))
                qtrickle["q"] = q

            def qdrip(nmm):
                done = 0
                while qtrickle["q"] and done < nmm:
                    e = qtrickle["q"].pop(0)
                    if e[0] == "copy":
                        _, t, h, cc = e
                        with nc.allow_low_precision(reason="q -> bf16"):
                            nc.vector.tensor_scalar_mul(
                                qt_all[:, h, cc * 256:(cc + 1) * 256],
                                t[:, cc * 256:(cc + 1) * 256], QDESCALE)
                        continue
                    _, t, wt, xt, cc, g, st, sp = e
                    nc.tensor.matmul(t[:, cc * 256:(cc + 1) * 256],
                                     wt[:, g], xt[:, cc, g],
                                     start=st, stop=sp, perf_mode=DR)
                    done += 1

            def qproj_pair(hp, defer_cc1=False):
                nonlocal wq_next, wq_next2
                pair_w = []
                for hh in range(2):
                    h = 2 * hp + hh
                    pair_w.append(wq_next)
                    wq_next = wq_next2
                    if h + 2 < HEADS:
                        wq_next2 = load_wq(h + 2)
                for hh in range(2):
                    h = 2 * hp + hh
                    wh, wl = pair_w[hh]
                    q_ps = qp_ps.tile([128, R], F32, tag="qp")
                    qproj_head_cc(h, wh, wl, q_ps, 0)
                    # copies on DVE, not ACT: the exp stream paces the
                    # attention tail, so ACT gets nothing extra
                    if defer_cc1:
                        with nc.allow_low_precision(reason="q -> bf16"):
                            nc.vector.tensor_scalar_mul(
                                qt_all[:, h, 0:256], q_ps[:, 0:256],
                                QDESCALE)
                        pending_cc1.append((h, wh, wl, q_ps))
                    else:
                        qproj_head_cc(h, wh, wl, q_ps, 1)
                        with nc.allow_low_precision(reason="q -> bf16"):
                            nc.vector.tensor_scalar_mul(
                                qt_all[:, h, :], q_ps[:], QDESCALE)

            def emit_ctile(ec, b, rt, ps_pool, sb_pool, ps_tag="op",
                           last=False):
                """One output-projection tile [r128, e512] (48 DR matmuls).

                last=True pipelines the epilogue per 256-col half (and
                splits the final half's DMA) to shorten the end drain.
                """
                o_ps = ps_pool.tile([128, 512], F32, tag=ps_tag)
                r0 = rt * 128
                terms = ((on_hi, woh_sb), (on_lo, woh_sb),
                         (on_hi, wol_sb))

                def group(dst, e0, ew):
                    n_mm = len(terms) * G
                    i = 0
                    for on_t, wo_t in terms:
                        for g in range(G):
                            nc.tensor.matmul(
                                dst[:, e0:e0 + ew],
                                on_t[:, b, g, :, r0:r0 + 128],
                                wo_t[:, ec, g, :, e0:e0 + ew],
                                start=(i == 0), stop=(i == n_mm - 1),
                                perf_mode=DR)
                            i += 1

                def epilogue(src, c0, cw, eng=None):
                    o_sb = sb_pool.tile([128, cw], F32, tag=f"ost{cw}")
                    nc.vector.tensor_scalar_mul(
                        o_sb[:], src[:, c0:c0 + cw], ODESCALE)
                    (eng or nc.sync).dma_start(
                        o_d[b * NC_ROWS + r0:b * NC_ROWS + r0 + 128,
                            ec * 512 + c0:ec * 512 + c0 + cw],
                        o_sb[:])

                if last:
                    # staircase drain: three column groups on separate
                    # PSUM banks, each epilogue overlapping the next
                    # group's matmuls
                    o_ps2 = ps_pool.tile([128, 512], F32, tag=ps_tag,
                                         name="ops_last2")
                    o_ps3 = ps_pool.tile([128, 512], F32, tag=ps_tag,
                                         name="ops_last3")
                    group(o_ps, 0, 256)
                    epilogue(o_ps, 0, 256, eng=nc.scalar)
                    group(o_ps2, 256, 128)
                    epilogue(o_ps2, 256, 128)
                    group(o_ps3, 384, 128)
                    epilogue(o_ps3, 384, 128, eng=nc.scalar)
                else:
                    group(o_ps, 0, 256)
                    group(o_ps, 256, 256)
                    epilogue(o_ps, 0, 512)

            # Pair-7 units have no qproj filler and run at the ACT exp pace:
            # trickle the first output-projection tile (ec0, b0, rt0) into
            # their PE slack, 3-5 matmuls per jg, g7 terms after pair-7's
            # b0 context exists. Keeps PE continuously busy (the cost
            # model's p-state ramp doubles the price of any PE idle gap).
            CTERMS = lambda: ((on_hi, woh_sb), (on_lo, woh_sb),  # noqa: E731
                              (on_hi, wol_sb))
            trickle = {"q": [], "ops": None}

            CTRICKLE_TILES = [(0, 0, 0), (0, 0, 1)]

            def trickle_init():
                trickle["tiles"] = []
                parts = []     # per tile: (eh0_main, eh0_g7, eh1_all)
                for ec, tb, rt in CTRICKLE_TILES:
                    ops = qp_ps.tile([128, 512], F32, tag="qp",
                                     name=f"ct_ops{ec}{tb}{rt}")
                    trickle["tiles"].append((ops, ec, tb, rt))
                    seg = []
                    for eh in range(2):
                        idx = [(t, g) for g in range(G - 1)
                               for t in range(3)]
                        idx += [(t, G - 1) for t in range(3)]
                        ent = [(ops, ec, tb, rt, eh, t, g,
                                i == 0, i == 3 * G - 1)
                               for i, (t, g) in enumerate(idx)]
                        seg.append(ent)
                    parts.append((seg[0][:21], seg[0][21:], seg[1]))
                # batch-0 slots drain only the first 21 entries (tile A's
                # g0-6): everything later reads pair-7's own context and
                # must be emitted after the b0 tail (batch-1 slots onward)
                (a0m, a0l, a1), (b0m, b0l, b1) = parts
                trickle["q"] = a0m + b0m + a0l + a1 + b0l + b1

            def trickle_emit(n):
                for _ in range(n):
                    if not trickle["q"]:
                        return
                    ops, ec, tb, rt, eh, t, g, st, sp = \
                        trickle["q"].pop(0)
                    on_t, wo_t = CTERMS()[t]
                    nc.tensor.matmul(
                        ops[:, eh * 256:eh * 256 + 256],
                        on_t[:, tb, g, :, rt * 128:rt * 128 + 128],
                        wo_t[:, ec, g, :, eh * 256:eh * 256 + 256],
                        start=st, stop=sp, perf_mode=DR)

            # b0 may drain at most 21 entries (tile A's eh0 g0-6): anything
            # later in the queue reads pair-7's own context, written by the
            # b0 tail which is EMITTED after b0's jg slots - an earlier
            # read would see uninitialized SBUF with no semaphore guard.
            TRICKLE_SLOTS = {(0, jg): 3 for jg in range(1, 8)}
            TRICKLE_SLOTS.update({(1, jg): 5 for jg in range(8)})

            qproj_pair(0, defer_cc1=True)
            for hp in range(HEADS // 2):
                for b in range(B):
                    if b == 0 and hp + 1 < HEADS // 2:
                        build_qtrickle(hp + 1)
                    if hp == HEADS // 2 - 1 and b == 0:
                        trickle_init()
                    # Both heads of the pair processed together: every matmul
                    # has a 512-wide moving operand laid out as [h2, i256].
                    acc = acc_ps.tile([128, 512], F32, tag="acc")
                    qt_pair = qt_all[:, 2 * hp:2 * hp + 2,
                                     b * NC_ROWS:(b + 1) * NC_ROWS]
                    s1024 = rbp.tile([128, 1024], F16, tag="s128")
                    # during the final attention unit the qproj PSUM banks
                    # are idle and all batch-0 context is split: inject
                    # early output-projection tiles to fill the ACT-paced
                    # tail of phase B
                    inject = False and (hp == HEADS // 2 - 1 and b == 1)
                    for jg in range(JT // 2):
                        if inject and jg in (1, 3, 5, 7):
                            ti = (1, 3, 5, 7).index(jg)
                            emit_ctile(ti // 2, 0, ti % 2, qp_ps, rbp,
                                       ps_tag="qp")
                        if pending_cc1 and hp == 0 and b == 0 \
                                and jg in (1, 3):
                            h, wh, wl, q_ps = pending_cc1.pop(0)
                            qproj_head_cc(h, wh, wl, q_ps, 1)
                            with nc.allow_low_precision(reason="q -> bf16"):
                                nc.vector.tensor_scalar_mul(
                                    qt_all[:, h, 256:512], q_ps[:, 256:512],
                                    QDESCALE)
                        sg = sg_ps.tile([128, 1024], F32, tag="sg")
                        for kk in range(2):
                            jt = jg * 2 + kk
                            nc.tensor.matmul(
                                sg[:, kk * 512:(kk + 1) * 512],
                                kt_sb[:, b, jt * 128:(jt + 1) * 128],
                                qt_pair,
                                start=True, stop=True)
                        es = esp.tile([128, 1024], BF16, tag="es")
                        with nc.allow_low_precision(reason="es bf16"):
                            nc.scalar.activation(
                                es[:], sg[:],
                                mybir.ActivationFunctionType.Exp,
                                scale=SCALE)
                            # softmax denominators: fp16 partial rowsums on
                            # DVE (2x 16-bit mode); partition reduce below
                            if jg == 0:
                                nc.vector.tensor_copy(s1024[:], es[:])
                            else:
                                nc.vector.tensor_add(s1024[:], s1024[:],
                                                     es[:])
                        if hp == HEADS // 2 - 1 and (b, jg) in TRICKLE_SLOTS:
                            trickle_emit(TRICKLE_SLOTS[(b, jg)])
                        qdrip(3)
                        # software-pipeline by one jg: consume the PREVIOUS
                        # jg's es, so the exp->attnv semaphore has fired
                        # long before PE arrives (no catch-up seam)
                        if jg > 0:
                            pj = jg - 1
                            for kk in range(2):
                                jt = pj * 2 + kk
                                nc.tensor.matmul(
                                    acc[:], v_sb[:, b, jt, :],
                                    es_prev[:, kk * 512:(kk + 1) * 512],
                                    start=(jt == 0), stop=False)
                        es_prev = es
                        qdrip(3)
                    # drain the pipelined final jg's attn*v
                    for kk in range(2):
                        jt = (JT // 2 - 1) * 2 + kk
                        nc.tensor.matmul(acc[:], v_sb[:, b, jt, :],
                                         es_prev[:, kk * 512:(kk + 1) * 512],
                                         start=False, stop=(jt == JT - 1))
                    if b == 1:
                        qdrip(10 ** 6)  # force-drain before the next pair
                    # softmax-denominator tail + context fp8 hi/lo split
                    s512 = rbp.tile([128, 512], F32, tag="s512", bufs=1)
                    sB = rbp.tile([128, 512], F32, tag="sB", bufs=1)
                    rb_sb = rbp.tile([128, 512], F32, tag="rbs")
                    t32 = rbp.tile([128, 512], F32, tag="t32")
                    hi_ap = on_hi[:, b, hp].rearrange("p a b -> p (a b)")
                    lo_ap = on_lo[:, b, hp].rearrange("p a b -> p (a b)")
                    with nc.allow_low_precision(reason="denominator tail"):
                        nc.vector.tensor_add(s512[:], s1024[:, 0:512],
                                             s1024[:, 512:1024])
                        nc.gpsimd.partition_all_reduce(
                            sB[:], s512[:], channels=128,
                            reduce_op=bass_isa.ReduceOp.add)
                        nc.vector.reciprocal(rb_sb[:], sB[:])
                        nc.vector.tensor_mul(t32[:], acc[:], rb_sb[:])
                        nc.gpsimd.tensor_scalar_mul(hi_ap, t32[:], OS)
                        nc.vector.scalar_tensor_tensor(
                            lo_ap, t32[:], OS, hi_ap,
                            mybir.AluOpType.mult,
                            mybir.AluOpType.subtract)
                    if hp == HEADS // 2 - 1 and b == 1:
                        trickle_emit(10 ** 6)  # drain leftover tile work
                        for ops, ec, tb, rt in trickle["tiles"]:
                            o_sb = rbp.tile([128, 512], F32, tag="ost512")
                            nc.vector.tensor_scalar_mul(o_sb[:], ops[:],
                                                        ODESCALE)
                            nc.sync.dma_start(
                                o_d[tb * NC_ROWS + rt * 128:
                                    tb * NC_ROWS + rt * 128 + 128,
                                    ec * 512:(ec + 1) * 512],
                                o_sb[:])

        # ---- Phase C: remaining output-projection tiles ----
        # (ec0/ec1, b0, *) were injected into the tail of phase B above.
        with tc.tile_pool(name="ost_pool", bufs=4) as ostp, \
             tc.tile_pool(name="op_ps", bufs=4, space="PSUM") as op_ps:
            tiles = [(ec, 0, rt) for ec in range(4) for rt in (0, 1)
                     if (ec, 0, rt) not in ((0, 0, 0), (0, 0, 1))]
            tiles += [(ec, 1, rt) for ec in range(4) for rt in (0, 1)]
            for ti, (ec, b, rt) in enumerate(tiles):
                emit_ctile(ec, b, rt, op_ps, ostp,
                           last=(ti == len(tiles) - 1))


def _get_nc(reps: int = 1):
    if reps not in _CACHE:
        _CACHE[reps] = _build(reps)
    return _CACHE[reps]


def _hilo(a, pre):
    s = (a * pre).astype(np.float32)
    hi = s.astype(NE4)
    lo = (s - hi.astype(np.float32)).astype(NE4)
    return hi, lo


def _make_in_maps(x, k, v, Wq, Wo):
    # Wq [E, inner] -> [h, p, g, s, f] with e = 256g + 128s + p
    wq_t = Wq.reshape(G, 2, 128, HEADS, 128).transpose(3, 2, 0, 1, 4)
    wqh, wql = _hilo(np.ascontiguousarray(wq_t), WQS)
    # Wo [inner, E] -> [ec, p, g, s, e'] with f = 256g + 128s + p
    wo_t = Wo.reshape(G, 2, 128, 4, 512).transpose(3, 2, 0, 1, 4)
    woh, wol = _hilo(np.ascontiguousarray(wo_t), WOS)
    # k [B, J, DH] -> kT [d, b, j]
    kt = np.ascontiguousarray(k.transpose(2, 0, 1)).astype(NBF)
    # v [B, J, DH] -> [p, b, jt, d]
    vt = np.ascontiguousarray(
        v.reshape(B, JT, 128, DH).transpose(2, 0, 1, 3)).astype(NBF)
    in_maps = []
    for c in range(NCORES):
        xs = x[:, c * NC_ROWS:(c + 1) * NC_ROWS, :]
        # [E, cc, r256] -> [p, cc, g, s, r]
        xt = np.stack([xs[0].T, xs[1].T], axis=1)
        xt = np.ascontiguousarray(
            xt.reshape(G, 2, 128, 2, NC_ROWS).transpose(2, 3, 0, 1, 4))
        xh, xl = _hilo(xt, XS)
        in_maps.append({"xh": xh, "xl": xl, "wqh": wqh, "wql": wql,
                        "kt": kt, "vt": vt, "woh": woh, "wol": wol})
    return in_maps


def run_on_device(x, k, v, Wq, Wo, reps: int = 1):
    nc = _get_nc(reps)
    in_maps = _make_in_maps(x, k, v, Wq, Wo)
    res = run_bass_kernel_spmd(nc, in_maps, list(range(NCORES)))
    parts = [res.results[c]["o"].reshape(B, NC_ROWS, E) for c in range(NCORES)]
    return np.concatenate(parts, axis=1)


def kernel(x, k, v, Wq, Wo):
    x = np.asarray(x, dtype=np.float32)
    k = np.asarray(k, dtype=np.float32)
    v = np.asarray(v, dtype=np.float32)
    Wq = np.asarray(Wq, dtype=np.float32)
    Wo = np.asarray(Wo, dtype=np.float32)
    return run_on_device(x, k, v, Wq, Wo, reps=1)
